# revision 1
# baseline (speedup 1.0000x reference)
"""Trainium2 Bass kernel for the Equiformer-style GNN regressor.

Strategy (8 NeuronCores, SPMD):
  - Nodes padded 10000 -> 10240 (1280/core, 10 tiles of 128). Core c owns
    padded nodes [1280c, 1280(c+1)).
  - Edges assigned to the core owning their dst, grouped by dst node-tile,
    padded to a uniform chunks-per-tile (CPT) of 512-slot chunks.
  - Per layer, a DRAM "node table" holds per-node projected quantities
    (576 f32/row, head-grouped layout). Edge phase gathers rows with
    dma_gather (edge-major [128e, 576]), computes radial weights
    feature-major on PE, transposes them to edge-major, forms messages with
    DVE elementwise ops, and aggregates per dst node-tile with onehot
    matmuls into PSUM. Attention softmax is unshifted exp (logits are O(0.1))
    with the 1/den applied post-aggregation.
  - Update phase (per node-tile): out-projections + residual + equivariant
    LayerNorms + next-layer projections; new table rows are written to DRAM
    and AllGathered across the 8 cores.
  - Final readout (per-node energies) is DMA'd out; the trivial per-graph
    segment-sum + Linear(1,1) readout runs on host.
"""
import math
import sys
import types
from contextlib import ExitStack
from dataclasses import dataclass

import numpy as np

import concourse.bacc as bacc
import concourse.bass as bass
import concourse.tile as tile
from concourse import mybir
from concourse.bass_utils import run_bass_kernel_spmd

F32 = mybir.dt.float32
I16 = mybir.dt.int16
AF = mybir.ActivationFunctionType
OP = mybir.AluOpType

# ---------------- problem constants (hardcoded per spec) ----------------
N, E, G, L = 10000, 320000, 32, 6
C0, C1, C2, H, NB, RAD, FD, T = 128, 64, 32, 4, 128, 64, 512, 1
MAXR = 5.0
EPS = 1e-6
NCORE = 8
P = 128

GW = 576          # gather row width
MW = 704          # radial projection width (128 R_sa + 4*144)
BLK = 144         # per-head block width in G / M layouts

EM_DST, EM_MASK, EM_SH1, EM_SH2, EMW = 0, 1, 2, 5, 12


@dataclass
class Cfg:
    ncore: int = NCORE
    npc: int = 1280          # padded nodes per core (multiple of 128)
    cpt: int = 9             # chunks (512 slots) per node-tile
    layers: int = L
    nn: int = N              # real node count

    @property
    def ntile(self):
        return self.npc // P

    @property
    def np_total(self):
        return self.npc * self.ncore

    @property
    def slots(self):
        return self.ntile * self.cpt * 512


# ---------------- host-side packing helpers ----------------

def g_col_maps():
    """Column index maps for the 576-wide node-table row layout."""
    ps = np.zeros(C0, np.int64)      # col of P_s channel c
    psv = np.zeros(C1, np.int64)
    pst = np.zeros(C2, np.int64)
    pv = np.zeros((C1, 3), np.int64)
    pt = np.zeros((C2, 5), np.int64)
    for c in range(C0):
        h, j = divmod(c, 32)
        ps[c] = BLK * h + j
    for c in range(C1):
        h, j = divmod(c, 16)
        psv[c] = BLK * h + 32 + j
    for c in range(C2):
        h, j = divmod(c, 8)
        pst[c] = BLK * h + 48 + j
    for c in range(C1):
        h, j = divmod(c, 16)
        for i in range(3):
            pv[c, i] = BLK * h + 56 + 16 * i + j
    for c in range(C2):
        h, j = divmod(c, 8)
        for m in range(5):
            pt[c, m] = BLK * h + 104 + 8 * m + j
    return ps, psv, pst, pv, pt


G_PS, G_PSV, G_PST, G_PV, G_PT = g_col_maps()


def pack_node_table(P_s, P_sv, P_st, Pv, Pt):
    """[NPn,C0],[NPn,C1],[NPn,C2],[NPn,C1,3],[NPn,C2,5] -> [NPn, 576]."""
    n = P_s.shape[0]
    out = np.zeros((n, GW), np.float32)
    out[:, G_PS] = P_s
    out[:, G_PSV] = P_sv
    out[:, G_PST] = P_st
    out[:, G_PV.reshape(-1)] = Pv.reshape(n, -1)
    out[:, G_PT.reshape(-1)] = Pt.reshape(n, -1)
    return out


def pack_wwall(Ww_s, Ww_v, Ww_t, Ww_vv, Ww_tt, attn_a):
    """Per-layer [RAD, 704] radial-projection weight, matching M layout."""
    out = np.zeros((RAD, MW), np.float32)
    # R_sa: col c = Ww_s[:, c] * attn_a[h(c), j(c)]
    for c in range(C0):
        h, j = divmod(c, 32)
        out[:, c] = Ww_s[:, c] * attn_a[h, j]
    out[:, 128 + G_PS] = Ww_s
    out[:, 128 + G_PSV] = Ww_v
    out[:, 128 + G_PST] = Ww_t
    for c in range(C1):
        for i in range(3):
            out[:, 128 + G_PV[c, i]] = Ww_vv[:, c]
    for c in range(C2):
        for m in range(5):
            out[:, 128 + G_PT[c, m]] = Ww_tt[:, c]
    return out


def host_preprocess(inp, cfg: Cfg):
    """Build all per-core device input arrays + schedule."""
    npc, ncore = cfg.npc, cfg.ncore
    assert cfg.nn % ncore == 0
    real_pc = cfg.nn // ncore
    assert real_pc <= npc

    pos = np.asarray(inp["pos"], np.float32)
    node_atom = np.asarray(inp["node_atom"]).astype(np.int64)
    esrc = np.asarray(inp["edge_src"]).astype(np.int64)
    edst = np.asarray(inp["edge_dst"]).astype(np.int64)

    def pid(i):
        return (i // real_pc) * npc + (i % real_pc)

    src_p = pid(esrc)
    dst_p = pid(edst)

    # geometry (match reference formulas, f32)
    rel = pos[edst] - pos[esrc]
    d2 = (rel * rel).sum(-1) + np.float32(EPS)
    d = np.sqrt(d2)
    u = rel / d[:, None]
    s3, s5, s15 = [np.float32(np.sqrt(x)) for x in (3.0, 5.0, 15.0)]
    sh1 = s3 * u
    x_, y_, z_ = u[:, 0], u[:, 1], u[:, 2]
    sh2 = np.stack(
        [s15 * x_ * y_, s15 * y_ * z_, np.float32(0.5) * s5 * (3 * z_ * z_ - 1.0),
         s15 * x_ * z_, np.float32(0.5) * s15 * (x_ * x_ - y_ * y_)], -1)

    # per-(core,tile) edge lists
    tile_of_edge = dst_p // P          # global tile id 0..ncore*ntile-1
    ntile = cfg.ntile
    order = np.argsort(tile_of_edge, kind="stable")
    counts = np.bincount(tile_of_edge, minlength=ncore * ntile)
    cpt_need = int(np.ceil(counts.max() / 512))
    assert cpt_need <= cfg.cpt, f"need cpt {cpt_need} > cfg {cfg.cpt}"
    starts = np.zeros(ncore * ntile + 1, np.int64)
    np.cumsum(counts, out=starts[1:])

    S = cfg.slots
    per_core = []
    for c in range(ncore):
        slot_src = np.zeros(S, np.int64)
        slot_dstrel = np.zeros(S, np.float32)
        slot_mask = np.zeros(S, np.float32)
        slot_d2 = np.ones(S, np.float32)
        slot_d = np.ones(S, np.float32)
        slot_sh1 = np.zeros((S, 3), np.float32)
        slot_sh2 = np.zeros((S, 5), np.float32)
        for t in range(ntile):
            gt = c * ntile + t
            eids = order[starts[gt]:starts[gt + 1]]
            base = t * cfg.cpt * 512
            k = len(eids)
            sl = slice(base, base + k)
            slot_src[sl] = src_p[eids]
            slot_dstrel[sl] = (dst_p[eids] - (c * npc + t * P)).astype(np.float32)
            slot_mask[sl] = 1.0
            slot_d2[sl] = d2[eids]
            slot_d[sl] = d[eids]
            slot_sh1[sl] = sh1[eids]
            slot_sh2[sl] = sh2[eids]

        # device layouts
        nsub = S // P
        em = np.zeros((P, nsub, EMW), np.float32)
        j = np.arange(S)
        em[j % P, j // P, EM_DST] = slot_dstrel
        em[j % P, j // P, EM_MASK] = slot_mask
        em[j % P, j // P, EM_SH1:EM_SH1 + 3] = slot_sh1
        em[j % P, j // P, EM_SH2:EM_SH2 + 5] = slot_sh2
        geom = np.ones((3, S), np.float32)
        geom[0] = slot_d2
        geom[1] = slot_d
        idx_em = np.zeros((P, nsub), np.int32)
        idx_em[j % P, j // P] = slot_src.astype(np.int32)
        per_core.append(dict(em=em, geom=geom, idxw=idx_em))

    # ---- weights ----
    wd = {}
    centers = np.linspace(0.0, MAXR, NB).astype(np.float32)
    width = np.float32(MAXR / NB)
    wq = np.zeros((3, NB), np.float32)
    wq[0] = -0.5 / width**2
    wq[1] = centers / width**2
    wq[2] = -0.5 * centers**2 / width**2
    wd["wq"] = wq
    wd["wrad1"] = np.asarray(inp["Wrad1"], np.float32)
    wd["wrad2"] = np.asarray(inp["Wrad2"], np.float32)
    wd["brad1"] = np.asarray(inp["brad1"], np.float32).reshape(cfg.layers, RAD, 1)
    wd["brad2"] = np.asarray(inp["brad2"], np.float32).reshape(cfg.layers, RAD, 1)
    wd["wwall"] = np.stack([
        pack_wwall(inp["Ww_s"][l], inp["Ww_v"][l], inp["Ww_t"][l],
                   inp["Ww_vv"][l], inp["Ww_tt"][l], inp["attn_a"][l])
        for l in range(cfg.layers)])
    wd["wo_s"] = np.asarray(inp["Wo_s"], np.float32)
    wd["wo_v"] = np.asarray(inp["Wo_v"], np.float32)
    wd["wo_t"] = np.asarray(inp["Wo_t"], np.float32)
    wd["ws_src"] = np.asarray(inp["Ws_src"], np.float32)
    wd["ws_v"] = np.asarray(inp["Ws_v"], np.float32)
    wd["ws_t"] = np.asarray(inp["Ws_t"], np.float32)
    wd["wv_v"] = np.asarray(inp["Wv_v"], np.float32)
    wd["wt_t"] = np.asarray(inp["Wt_t"], np.float32)
    rep = lambda a: np.broadcast_to(a[:, None, :], (a.shape[0], P, a.shape[1])).copy()
    wd["lngs"] = rep(np.asarray(inp["g_s"], np.float32))
    wd["lnbs"] = rep(np.asarray(inp["b_s"], np.float32))
    wd["lngv"] = rep(np.asarray(inp["g_v"], np.float32))
    wd["lngt"] = rep(np.asarray(inp["g_t"], np.float32))
    wd["wfeat"] = np.asarray(inp["W_feat"], np.float32)
    wd["bfeatp"] = np.asarray(inp["b_feat"], np.float32).reshape(4, 128).T.copy()
    wd["wout1p"] = np.asarray(inp["W_out1"], np.float32).reshape(4, 128).T.copy()
    wd["nidx"] = np.tile(np.arange(P, dtype=np.float32), (P, 1))
    wd["ident"] = np.eye(P, dtype=np.float32)

    # ---- initial node table (layer 0 projections) + s0 feature-major ----
    s0 = np.asarray(inp["atom_emb"], np.float32)[node_atom]     # [N, C0]
    s0p = np.zeros((cfg.np_total, C0), np.float32)
    for c in range(ncore):
        s0p[c * npc: c * npc + real_pc] = s0[c * real_pc:(c + 1) * real_pc]
    ntab0 = pack_node_table(
        s0p @ inp["Ws_src"][0], s0p @ inp["Ws_v"][0], s0p @ inp["Ws_t"][0],
        np.zeros((cfg.np_total, C1, 3), np.float32),
        np.zeros((cfg.np_total, C2, 5), np.float32))

    in_maps = []
    for c in range(ncore):
        m = dict(per_core[c])
        m["ntab0"] = ntab0
        m["s0fm"] = s0p[c * npc:(c + 1) * npc].T.copy()   # [C0, npc]
        for k, v in wd.items():
            m[k] = v
        in_maps.append(m)
    return in_maps, real_pc


# ---------------- device program ----------------

def reap(sliced: bass.AP, dims) -> bass.AP:
    """Rebuild the free-dims of a (narrow) sliced AP with explicit
    [step, count] pairs, keeping its partition dim and offset."""
    return bass.AP(sliced.tensor, sliced.offset,
                   [list(sliced.ap[0])] + [[int(s), int(c)] for s, c in dims])


def build_program(cfg: Cfg, dbg: bool = False):
    nc = bacc.Bacc("TRN2", target_bir_lowering=False, debug=False,
                   enable_asserts=True, num_devices=cfg.ncore)
    npc, ntile, cpt = cfg.npc, cfg.ntile, cfg.cpt
    S = cfg.slots
    nsub = S // P
    NPT = cfg.np_total
    LYR = cfg.layers

    dp = nc.declare_dram_parameter
    t_ntab0 = dp("ntab0", [NPT, GW], F32, isOutput=False)
    t_s0fm = dp("s0fm", [C0, npc], F32, isOutput=False)
    t_em = dp("em", [P, nsub, EMW], F32, isOutput=False)
    t_geom = dp("geom", [3, S], F32, isOutput=False)
    t_idxw = dp("idxw", [P, nsub], mybir.dt.int32, isOutput=False)
    t_wq = dp("wq", [3, NB], F32, isOutput=False)
    t_wrad1 = dp("wrad1", [LYR, NB, RAD], F32, isOutput=False)
    t_wrad2 = dp("wrad2", [LYR, RAD, RAD], F32, isOutput=False)
    t_brad1 = dp("brad1", [LYR, RAD, 1], F32, isOutput=False)
    t_brad2 = dp("brad2", [LYR, RAD, 1], F32, isOutput=False)
    t_wwall = dp("wwall", [LYR, RAD, MW], F32, isOutput=False)
    t_wo_s = dp("wo_s", [LYR, C0, C0], F32, isOutput=False)
    t_wo_v = dp("wo_v", [LYR, C1, C1], F32, isOutput=False)
    t_wo_t = dp("wo_t", [LYR, C2, C2], F32, isOutput=False)
    t_ws_src = dp("ws_src", [LYR, C0, C0], F32, isOutput=False)
    t_ws_v = dp("ws_v", [LYR, C0, C1], F32, isOutput=False)
    t_ws_t = dp("ws_t", [LYR, C0, C2], F32, isOutput=False)
    t_wv_v = dp("wv_v", [LYR, C1, C1], F32, isOutput=False)
    t_wt_t = dp("wt_t", [LYR, C2, C2], F32, isOutput=False)
    t_lngs = dp("lngs", [LYR, P, C0], F32, isOutput=False)
    t_lnbs = dp("lnbs", [LYR, P, C0], F32, isOutput=False)
    t_lngv = dp("lngv", [LYR, P, C1], F32, isOutput=False)
    t_lngt = dp("lngt", [LYR, P, C2], F32, isOutput=False)
    t_wfeat = dp("wfeat", [C0, FD], F32, isOutput=False)
    t_bfeatp = dp("bfeatp", [P, 4], F32, isOutput=False)
    t_wout1p = dp("wout1p", [P, 4], F32, isOutput=False)
    t_nidx = dp("nidx", [P, P], F32, isOutput=False)
    t_ident = dp("ident", [P, P], F32, isOutput=False)
    t_nodee = dp("node_e", [npc], F32, isOutput=True)
    if dbg:
        t_dbg_g = dp("dbg_g", [P, 4, GW], F32, isOutput=True)
        t_dbg_rpt = dp("dbg_rpt", [P, MW], F32, isOutput=True)
        t_dbg_hh = dp("dbg_hh", [P, 4, GW], F32, isOutput=True)
        t_dbg_ex = dp("dbg_ex", [P, 16], F32, isOutput=True)
        t_dbg_w = dp("dbg_w", [RAD, 512], F32, isOutput=True)
        t_dbg_agg = dp("dbg_agg", [P, 484], F32, isOutput=True)
        t_dbg_aggnm = dp("dbg_aggnm", [P, 480], F32, isOutput=True)
        t_dbg_snm = dp("dbg_snm", [P, C0], F32, isOutput=True)
        t_dbg_vnm = dp("dbg_vnm", [P, C1, 3], F32, isOutput=True)

    own = [nc.dram_tensor(f"own{l}", [npc, GW], F32) for l in range(LYR - 1)]
    ntab = [nc.dram_tensor(f"ntab{l + 1}", [NPT, GW], F32, addr_space="Shared")
            for l in range(LYR - 1)]

    with tile.TileContext(nc) as tc, ExitStack() as ctx:
        pool1 = ctx.enter_context(tc.tile_pool(name="const", bufs=1))
        poolL = ctx.enter_context(tc.tile_pool(name="layerw", bufs=1))
        poolT = ctx.enter_context(tc.tile_pool(name="tilec", bufs=2))
        poolg = ctx.enter_context(tc.tile_pool(name="gath", bufs=2))
        poole = ctx.enter_context(tc.tile_pool(name="edge", bufs=2))
        poolx = ctx.enter_context(tc.tile_pool(name="edge1", bufs=1))
        poolr = ctx.enter_context(tc.tile_pool(name="rad", bufs=1))
        poolu = ctx.enter_context(tc.tile_pool(name="upd", bufs=1))
        psA = ctx.enter_context(tc.tile_pool(name="psA", bufs=2, space="PSUM"))
        psT = ctx.enter_context(tc.tile_pool(name="psT", bufs=2, space="PSUM"))
        psAgg = ctx.enter_context(tc.tile_pool(name="psAgg", bufs=1, space="PSUM"))

        def load1(dram, shape, dtype=F32):
            t = pool1.tile(shape, dtype, tag=dram.name)
            nc.sync.dma_start(out=t[:], in_=dram[:])
            return t

        # resident constants
        wq_t = load1(t_wq, [3, NB])
        nidx_t = load1(t_nidx, [P, P])
        ident_t = load1(t_ident, [P, P])
        wfeat_t = load1(t_wfeat, [C0, FD])
        bfeatp_t = load1(t_bfeatp, [P, 4])
        wout1p_t = load1(t_wout1p, [P, 4])

        eps_t = pool1.tile([P, 1], F32, tag="epsT")
        nc.vector.memset(eps_t[:], EPS)

        # feature-major stores for own nodes
        sfm = pool1.tile([C0, npc], F32, tag="sfm")
        nc.sync.dma_start(out=sfm[:], in_=t_s0fm[:])
        vfm_t = pool1.tile([C1, 3, npc], F32, tag="vfm")
        nc.vector.memset(vfm_t[:], 0.0)
        tfm_t = pool1.tile([C2, 5, npc], F32, tag="tfm")
        nc.vector.memset(tfm_t[:], 0.0)

        def vfm(i):
            return vfm_t[:, i, :]

        def tfm(m):
            return tfm_t[:, m, :]

        # per-tile batch buffers for the radial pipeline
        rbf_b = poolr.tile([NB, cpt * 512], F32, tag="rbfb")
        w_b = poolr.tile([RAD, cpt * 512], F32, tag="wb")

        def loadL(dram, l, p, f, tag):
            t = poolL.tile([p, f], F32, tag=tag)
            nc.sync.dma_start(out=t[:], in_=dram[l])
            return t

        def edge_tile(l, t, gsrc, lw, tct):
            """Edge phase for node-tile t of layer l. Returns agg psum tile."""
            em_s, idx_s, tbase = tct
            # PH1: rbf (exp table)
            for k in range(cpt):
                gsl = poolT.tile([3, 512], F32, tag="geom_c")
                nc.sync.dma_start(out=gsl[:], in_=t_geom[:, (tbase + k) * 512:(tbase + k + 1) * 512])
                ps = psA.tile([NB, 512], F32, tag="mmA", space="PSUM")
                nc.tensor.matmul(ps[:], wq_t[:], gsl[:], start=True, stop=True)
                nc.scalar.activation(out=rbf_b[:, k * 512:(k + 1) * 512], in_=ps[:], func=AF.Exp)
            # PH2: radial MLP (silu table)
            for k in range(cpt):
                sl = slice(k * 512, (k + 1) * 512)
                ps = psA.tile([RAD, 512], F32, tag="mmA", space="PSUM")
                nc.tensor.matmul(ps[:], lw["wrad1"][:], rbf_b[:, sl], start=True, stop=True)
                h1 = poolx.tile([RAD, 512], F32, tag="h1")
                nc.scalar.activation(out=h1[:], in_=ps[:], func=AF.Silu, bias=lw["brad1"][:])
                ps2 = psA.tile([RAD, 512], F32, tag="mmA", space="PSUM")
                nc.tensor.matmul(ps2[:], lw["wrad2"][:], h1[:], start=True, stop=True)
                nc.scalar.activation(out=w_b[:, sl], in_=ps2[:], func=AF.Silu, bias=lw["brad2"][:])
            # PH3 (exp table): projections, gather, messages, aggregation
            agg = psAgg.tile([P, 484], F32, tag="agg", space="PSUM")
            last_v = l < LYR - 1
            for k in range(cpt):
                wsl = w_b[:, k * 512:(k + 1) * 512]
                # gather G rows for this chunk (one row per partition/call)
                gt = poolg.tile([P, 4, GW], F32, tag="gt")
                for sub4 in range(4):
                    nc.gpsimd.indirect_dma_start(
                        out=gt[:, sub4, :], out_offset=None, in_=gsrc[:, :],
                        in_offset=bass.IndirectOffsetOnAxis(
                            ap=idx_s[:, k * 4 + sub4:k * 4 + sub4 + 1], axis=0))
                # radial projections Rp (feature-major), evac to sbuf
                rp = poolx.tile([P, 6, 512], F32, tag="rp")
                for b in range(6):
                    wcols = min(128, MW - b * 128)
                    ps = psA.tile([P, 512], F32, tag="mmA", space="PSUM")
                    nc.tensor.matmul(ps[:wcols, :], lw["wwall"][:, b * 128:b * 128 + wcols],
                                     wsl, start=True, stop=True)
                    nc.scalar.copy(out=rp[:wcols, b, :], in_=ps[:wcols, :])
                # per sub-tile of 128 edges
                logit = poole.tile([P, 16], F32, tag="logit")
                ex = poole.tile([P, 16], F32, tag="ex")
                hh = poole.tile([P, 4, GW], F32, tag="hh")
                t2v = poolx.tile([P, 4, 192], F32, tag="t2v")
                t2t = poolx.tile([P, 4, 160], F32, tag="t2t")
                scr = poolx.tile([P, 4, 128], F32, tag="scr")
                wlen = BLK if last_v else 32
                if dbg and l == 0 and t == 0 and k == 0:
                    dsb = poolu.tile([P, 4, GW], F32, tag="dsb")
                    nc.vector.tensor_copy(out=dsb[:], in_=gt[:])
                    nc.sync.dma_start(out=t_dbg_g[:], in_=dsb[:])
                    dsw = poolu.tile([RAD, 512], F32, tag="dsw")
                    nc.vector.tensor_copy(out=dsw[:], in_=wsl)
                    nc.sync.dma_start(out=t_dbg_w[:], in_=dsw[:])
                for sub in range(4):
                    cs = k * 4 + sub
                    esl = slice(sub * 128, (sub + 1) * 128)
                    rpT = psT.tile([P, MW], F32, tag="rpT", space="PSUM")
                    for b in range(6):
                        wcols = min(128, MW - b * 128)
                        nc.tensor.transpose(rpT[:, b * 128:b * 128 + wcols],
                                            rp[:wcols, b, esl],
                                            ident_t[:wcols, :wcols])
                    # logits: (G_s * R_sa) summed over the 32 head channels
                    nc.vector.tensor_tensor(
                        out=scr[:, sub, :],
                        in0=reap(gt[:, sub:sub + 1, 0:1], [(BLK, 4), (1, 32)]),
                        in1=rpT[:, 0:128],
                        op=OP.mult)
                    nc.vector.tensor_reduce(
                        out=logit[:, sub * 4:sub * 4 + 4],
                        in_=scr[:, sub, :].rearrange("p (h c) -> p h c", h=H),
                        axis=mybir.AxisListType.X, op=OP.add)
                    msk = em_s[:, cs, EM_MASK:EM_MASK + 1]
                    exs = slice(sub * 4, sub * 4 + 4)
                    nc.vector.tensor_scalar(out=ex[:, exs], in0=logit[:, exs],
                                            scalar1=msk, scalar2=None, op0=OP.mult)
                    nc.scalar.activation(out=ex[:, exs], in_=ex[:, exs], func=AF.Exp)
                    nc.vector.tensor_scalar(out=ex[:, exs], in0=ex[:, exs],
                                            scalar1=msk, scalar2=None, op0=OP.mult)
                    for h in range(H):
                        nc.vector.scalar_tensor_tensor(
                            out=hh[:, sub, BLK * h:BLK * h + wlen],
                            in0=gt[:, sub, BLK * h:BLK * h + wlen],
                            scalar=ex[:, sub * 4 + h:sub * 4 + h + 1],
                            in1=rpT[:, 128 + BLK * h:128 + BLK * h + wlen],
                            op0=OP.mult, op1=OP.mult)
                    # messages for v/t irreps (skipped for the final layer)
                    if last_v:
                        sh1a = em_s[:, cs:cs + 1, EM_SH1:EM_SH1 + 1]
                        sh2a = em_s[:, cs:cs + 1, EM_SH2:EM_SH2 + 1]
                        # T2v = (ex*A_v) outer sh1 : iter (h, i, c')
                        nc.vector.tensor_tensor(
                            out=t2v[:, sub, :],
                            in0=reap(hh[:, sub:sub + 1, 32:33], [(BLK, 4), (0, 3), (1, 16)]),
                            in1=reap(sh1a, [(0, 4), (1, 3), (0, 16)]),
                            op=OP.mult)
                        nc.vector.tensor_tensor(
                            out=t2t[:, sub, :],
                            in0=reap(hh[:, sub:sub + 1, 48:49], [(BLK, 4), (0, 5), (1, 8)]),
                            in1=reap(sh2a, [(0, 4), (1, 5), (0, 8)]),
                            op=OP.mult)
                        # msg_v = T2v + (ex*Pv*R_vv);  msg_t similarly
                        nc.vector.tensor_tensor(
                            out=reap(hh[:, sub:sub + 1, 56:57], [(BLK, 4), (16, 3), (1, 16)]),
                            in0=t2v[:, sub, :],
                            in1=reap(hh[:, sub:sub + 1, 56:57], [(BLK, 4), (16, 3), (1, 16)]),
                            op=OP.add)
                        nc.vector.tensor_tensor(
                            out=reap(hh[:, sub:sub + 1, 104:105], [(BLK, 4), (8, 5), (1, 8)]),
                            in0=t2t[:, sub, :],
                            in1=reap(hh[:, sub:sub + 1, 104:105], [(BLK, 4), (8, 5), (1, 8)]),
                            op=OP.add)
                    if dbg and l == 0 and t == 0 and k == 0 and sub == 0:
                        drt = poolu.tile([P, MW], F32, tag="drt")
                        nc.vector.tensor_copy(out=drt[:], in_=rpT[:])
                        nc.sync.dma_start(out=t_dbg_rpt[:], in_=drt[:])
                    if dbg and l == 0 and t == 0 and k == 0 and sub == 3:
                        dhh = poolu.tile([P, 4, GW], F32, tag="dhh")
                        nc.vector.tensor_copy(out=dhh[:], in_=hh[:])
                        nc.sync.dma_start(out=t_dbg_hh[:], in_=dhh[:])
                        dex = poolu.tile([P, 16], F32, tag="dex")
                        nc.vector.tensor_copy(out=dex[:], in_=ex[:])
                        nc.sync.dma_start(out=t_dbg_ex[:], in_=dex[:])
                    # aggregation matmuls
                    first = (k == 0 and sub == 0)
                    last = (k == cpt - 1 and sub == 3)
                    oh = poole.tile([P, P], F32, tag="oh")
                    nc.vector.tensor_scalar(out=oh[:], in0=nidx_t[:],
                                            scalar1=em_s[:, cs, EM_DST:EM_DST + 1],
                                            scalar2=None, op0=OP.is_equal)
                    # NOTE: start=True zeroes the whole PSUM zero-region, so
                    # only the very first matmul of the tile may set it; the
                    # other regions' first writes land on pending-zero bytes.
                    nc.tensor.matmul(agg[:, 480:484], oh[:], ex[:, exs],
                                     start=first, stop=last, skip_group_check=True)
                    nc.tensor.matmul(
                        agg[:, 0:128], oh[:],
                        reap(hh[:, sub:sub + 1, 0:1], [(BLK, 4), (1, 32)]),
                        start=False, stop=last, skip_group_check=True)
                    if last_v:
                        nc.tensor.matmul(
                            agg[:, 128:320], oh[:],
                            reap(hh[:, sub:sub + 1, 56:57], [(16, 3), (BLK, 4), (1, 16)]),
                            start=False, stop=last, skip_group_check=True)
                        nc.tensor.matmul(
                            agg[:, 320:480], oh[:],
                            reap(hh[:, sub:sub + 1, 104:105], [(8, 5), (BLK, 4), (1, 8)]),
                            start=False, stop=last, skip_group_check=True)
            return agg

        def transpose_to(psum_pool, src_ap, kparts, ffree):
            """transpose src [kparts, ffree] sbuf -> psum [ffree, kparts]"""
            ps = psum_pool.tile([P, P], F32, tag="mmA", space="PSUM")
            nc.tensor.transpose(ps[:ffree, :kparts], src_ap,
                                ident_t[:kparts, :kparts])
            return ps

        def update_tile(l, t, agg, lw):
            tsl = slice(t * P, (t + 1) * P)
            if dbg and l == 0 and t == 0:
                dag = poolu.tile([P, 484], F32, tag="dag")
                nc.vector.tensor_copy(out=dag[:], in_=agg[:])
                nc.sync.dma_start(out=t_dbg_agg[:], in_=dag[:])
            last_v = l < LYR - 1
            # 1/(den + 1e-9)
            rden = poolu.tile([P, H], F32, tag="rden")
            nc.vector.tensor_scalar(out=rden[:], in0=agg[:, 480:484],
                                    scalar1=1e-9, scalar2=None, op0=OP.add)
            nc.vector.reciprocal(out=rden[:], in_=rden[:])
            aggnm = poolu.tile([P, 480], F32, tag="aggnm")
            for h in range(H):
                nc.vector.tensor_scalar(
                    out=aggnm[:, 32 * h:32 * h + 32], in0=agg[:, 32 * h:32 * h + 32],
                    scalar1=rden[:, h:h + 1], scalar2=None, op0=OP.mult)
                if last_v:
                    nc.vector.tensor_scalar(
                        out=reap(aggnm[:, 128 + 16 * h:128 + 16 * h + 1], [(64, 3), (1, 16)]),
                        in0=reap(agg[:, 128 + 16 * h:128 + 16 * h + 1], [(64, 3), (1, 16)]),
                        scalar1=rden[:, h:h + 1], scalar2=None, op0=OP.mult)
                    nc.vector.tensor_scalar(
                        out=reap(aggnm[:, 320 + 8 * h:320 + 8 * h + 1], [(32, 5), (1, 8)]),
                        in0=reap(agg[:, 320 + 8 * h:320 + 8 * h + 1], [(32, 5), (1, 8)]),
                        scalar1=rden[:, h:h + 1], scalar2=None, op0=OP.mult)

            if dbg and l == 0 and t == 0:
                nc.sync.dma_start(out=t_dbg_aggnm[:], in_=aggnm[:])
            # transpose agg to feature-major + out-projections + residual
            psS = transpose_to(psA, aggnm[:, 0:128], P, P)
            afm_s = poolu.tile([P, P], F32, tag="afm_s")
            nc.scalar.copy(out=afm_s[:], in_=psS[:, :P])
            pso = psA.tile([P, P], F32, tag="mmA", space="PSUM")
            nc.tensor.matmul(pso[:], lw["wo_s"][:], afm_s[:], start=True, stop=True)
            upd_s = poolu.tile([P, P], F32, tag="upd_s")
            nc.vector.tensor_tensor(out=upd_s[:], in0=sfm[:, tsl], in1=pso[:], op=OP.add)

            upd_v = poolu.tile([C1, 3, P], F32, tag="upd_v")
            upd_t = poolu.tile([C2, 5, P], F32, tag="upd_t")
            if last_v:
                for i in range(3):
                    psV = transpose_to(psA, aggnm[:, 128 + 64 * i:128 + 64 * i + 64], P, C1)
                    afm = poolu.tile([C1, P], F32, tag="afm_v")
                    nc.scalar.copy(out=afm[:], in_=psV[:C1, :P])
                    psv2 = psA.tile([C1, P], F32, tag="mmA", space="PSUM")
                    nc.tensor.matmul(psv2[:], lw["wo_v"][:], afm[:], start=True, stop=True)
                    nc.vector.tensor_tensor(out=upd_v[:, i, :], in0=vfm(i)[:, tsl],
                                            in1=psv2[:], op=OP.add)
                for m in range(5):
                    psT_ = transpose_to(psA, aggnm[:, 320 + 32 * m:320 + 32 * m + 32], P, C2)
                    afm = poolu.tile([C2, P], F32, tag="afm_t")
                    nc.scalar.copy(out=afm[:], in_=psT_[:C2, :P])
                    pst2 = psA.tile([C2, P], F32, tag="mmA", space="PSUM")
                    nc.tensor.matmul(pst2[:], lw["wo_t"][:], afm[:], start=True, stop=True)
                    nc.vector.tensor_tensor(out=upd_t[:, m, :], in0=tfm(m)[:, tsl],
                                            in1=pst2[:], op=OP.add)

            # transpose updated features to node-major
            snm = poolu.tile([P, C0], F32, tag="snm")
            psn = transpose_to(psA, upd_s[:], P, P)
            nc.scalar.copy(out=snm[:], in_=psn[:, :P])
            vnm = poolu.tile([P, C1, 3], F32, tag="vnm")
            tnm = poolu.tile([P, C2, 5], F32, tag="tnm")
            if last_v:
                for i in range(3):
                    psn = transpose_to(psA, upd_v[:, i, :], C1, P)
                    nc.vector.tensor_copy(
                        out=reap(vnm[:, 0:1, i:i + 1], [(3, C1)]), in_=psn[:, :C1])
                for m in range(5):
                    psn = transpose_to(psA, upd_t[:, m, :], C2, P)
                    nc.vector.tensor_copy(
                        out=reap(tnm[:, 0:1, m:m + 1], [(5, C2)]), in_=psn[:, :C2])

            # LayerNorm on s
            stats = poolu.tile([P, 6], F32, tag="stats")
            nc.vector.bn_stats(out=stats[:], in_=snm[:])
            mv = poolu.tile([P, 2], F32, tag="mv")
            nc.vector.bn_aggr(out=mv[:], in_=stats[:])
            lnt = poolu.tile([P, 2], F32, tag="lnt")
            nc.scalar.activation(out=lnt[:, 0:1], in_=mv[:, 1:2], func=AF.Ln, bias=eps_t[:])
            nc.scalar.activation(out=lnt[:, 1:2], in_=lnt[:, 0:1], func=AF.Exp, scale=-0.5)
            nc.vector.tensor_scalar(out=snm[:], in0=snm[:], scalar1=mv[:, 0:1],
                                    scalar2=lnt[:, 1:2], op0=OP.subtract, op1=OP.mult)
            nc.vector.tensor_tensor(out=snm[:], in0=snm[:], in1=lw["lngs"][:], op=OP.mult)
            nc.vector.tensor_tensor(out=snm[:], in0=snm[:], in1=lw["lnbs"][:], op=OP.add)

            if last_v:
                # v norm
                vsq = poolu.tile([P, C1, 3], F32, tag="vsq")
                nc.vector.tensor_tensor(out=vsq[:], in0=vnm[:], in1=vnm[:], op=OP.mult)
                vr1 = poolu.tile([P, C1], F32, tag="vr1")
                nc.vector.tensor_reduce(out=vr1[:], in_=vsq[:], axis=mybir.AxisListType.X, op=OP.add)
                vr2 = poolu.tile([P, 1], F32, tag="vr2")
                nc.vector.tensor_reduce(out=vr2[:], in_=vr1[:], axis=mybir.AxisListType.X, op=OP.add)
                nc.scalar.activation(out=vr2[:], in_=vr2[:], func=AF.Ln, bias=eps_t[:], scale=1.0 / C1)
                nc.scalar.activation(out=vr2[:], in_=vr2[:], func=AF.Exp, scale=-0.5)
                nc.vector.tensor_scalar(out=vnm[:], in0=vnm[:], scalar1=vr2[:],
                                        scalar2=None, op0=OP.mult)
                nc.vector.tensor_tensor(
                    out=vnm[:], in0=vnm[:],
                    in1=reap(lw["lngv"][:, 0:1], [(1, C1), (0, 3)]), op=OP.mult)
                # t norm
                tsq = poolu.tile([P, C2, 5], F32, tag="tsq")
                nc.vector.tensor_tensor(out=tsq[:], in0=tnm[:], in1=tnm[:], op=OP.mult)
                tr1 = poolu.tile([P, C2], F32, tag="tr1")
                nc.vector.tensor_reduce(out=tr1[:], in_=tsq[:], axis=mybir.AxisListType.X, op=OP.add)
                tr2 = poolu.tile([P, 1], F32, tag="tr2")
                nc.vector.tensor_reduce(out=tr2[:], in_=tr1[:], axis=mybir.AxisListType.X, op=OP.add)
                nc.scalar.activation(out=tr2[:], in_=tr2[:], func=AF.Ln, bias=eps_t[:], scale=1.0 / C2)
                nc.scalar.activation(out=tr2[:], in_=tr2[:], func=AF.Exp, scale=-0.5)
                nc.vector.tensor_scalar(out=tnm[:], in0=tnm[:], scalar1=tr2[:],
                                        scalar2=None, op0=OP.mult)
                nc.vector.tensor_tensor(
                    out=tnm[:], in0=tnm[:],
                    in1=reap(lw["lngt"][:, 0:1], [(1, C2), (0, 5)]), op=OP.mult)

            if dbg and l == 0 and t == 0:
                nc.sync.dma_start(out=t_dbg_snm[:], in_=snm[:])
                if last_v:
                    nc.sync.dma_start(out=t_dbg_vnm[:], in_=vnm[:])
            # write back feature-major stores
            psn = transpose_to(psA, snm[:], P, P)
            nc.scalar.copy(out=sfm[:, tsl], in_=psn[:, :P])
            if last_v:
                for i in range(3):
                    psn = transpose_to(psA, reap(vnm[:, 0:1, i:i + 1], [(3, C1)]), P, C1)
                    nc.scalar.copy(out=vfm(i)[:, tsl], in_=psn[:C1, :P])
                for m in range(5):
                    psn = transpose_to(psA, reap(tnm[:, 0:1, m:m + 1], [(5, C2)]), P, C2)
                    nc.scalar.copy(out=tfm(m)[:, tsl], in_=psn[:C2, :P])

            if last_v:
                # next-layer node-table projections -> ntabrow (node-major)
                ntrow = poolu.tile([P, GW], F32, tag="ntrow")

                def proj_to_row(lhsT, rhs, rows, dims, off):
                    ps = psA.tile([P, P], F32, tag="mmA", space="PSUM")
                    nc.tensor.matmul(ps[:rows, :P], lhsT, rhs, start=True, stop=True)
                    sb = poolu.tile([P, P], F32, tag="projsb")
                    nc.scalar.copy(out=sb[:rows, :P], in_=ps[:rows, :P])
                    psn2 = psA.tile([P, P], F32, tag="mmA", space="PSUM")
                    nc.tensor.transpose(psn2[:P, :rows], sb[:rows, :P],
                                        ident_t[:rows, :rows])
                    nc.vector.tensor_copy(
                        out=reap(ntrow[:, off:off + 1], dims), in_=psn2[:P, :rows])

                proj_to_row(lw["ws_src2"][:], sfm[:, tsl], C0, [(BLK, 4), (1, 32)], 0)
                proj_to_row(lw["ws_v2"][:], sfm[:, tsl], C1, [(BLK, 4), (1, 16)], 32)
                proj_to_row(lw["ws_t2"][:], sfm[:, tsl], C2, [(BLK, 4), (1, 8)], 48)
                for i in range(3):
                    proj_to_row(lw["wv_v2"][:], vfm(i)[:, tsl], C1,
                                [(BLK, 4), (1, 16)], 56 + 16 * i)
                for m in range(5):
                    proj_to_row(lw["wt_t2"][:], tfm(m)[:, tsl], C2,
                                [(BLK, 4), (1, 8)], 104 + 8 * m)
                nc.sync.dma_start(out=own[l][tsl, :], in_=ntrow[:])
            else:
                # final readout head for this tile
                feat = poolu.tile([P, 4, P], F32, tag="feat")
                for b in range(4):
                    ps = psA.tile([P, P], F32, tag="mmA", space="PSUM")
                    nc.tensor.matmul(ps[:], wfeat_t[:, b * 128:(b + 1) * 128],
                                     sfm[:, tsl], start=True, stop=True)
                    nc.scalar.activation(out=feat[:, b, :], in_=ps[:],
                                         func=AF.Gelu_apprx_tanh, bias=bfeatp_t[:, b:b + 1])
                pse = psA.tile([1, P], F32, tag="mmA", space="PSUM")
                for b in range(4):
                    nc.tensor.matmul(pse[:], wout1p_t[:, b:b + 1], feat[:, b, :],
                                     start=(b == 0), stop=(b == 3))
                ne = poolu.tile([1, P], F32, tag="ne")
                nc.vector.tensor_copy(out=ne[:], in_=pse[:])
                nc.sync.dma_start(out=t_nodee[tsl], in_=ne[0:1, :])

        for l in range(LYR):
            gsrc = t_ntab0 if l == 0 else ntab[l - 1]
            lw = dict(
                wrad1=loadL(t_wrad1, l, NB, RAD, "wrad1"),
                wrad2=loadL(t_wrad2, l, RAD, RAD, "wrad2"),
                brad1=loadL(t_brad1, l, RAD, 1, "brad1"),
                brad2=loadL(t_brad2, l, RAD, 1, "brad2"),
                wwall=loadL(t_wwall, l, RAD, MW, "wwall"),
                wo_s=loadL(t_wo_s, l, C0, C0, "wo_s"),
                wo_v=loadL(t_wo_v, l, C1, C1, "wo_v"),
                wo_t=loadL(t_wo_t, l, C2, C2, "wo_t"),
                lngs=loadL(t_lngs, l, P, C0, "lngs"),
                lnbs=loadL(t_lnbs, l, P, C0, "lnbs"),
                lngv=loadL(t_lngv, l, P, C1, "lngv"),
                lngt=loadL(t_lngt, l, P, C2, "lngt"),
            )
            if l < LYR - 1:
                lw["ws_src2"] = loadL(t_ws_src, l + 1, C0, C0, "ws_src2")
                lw["ws_v2"] = loadL(t_ws_v, l + 1, C0, C1, "ws_v2")
                lw["ws_t2"] = loadL(t_ws_t, l + 1, C0, C2, "ws_t2")
                lw["wv_v2"] = loadL(t_wv_v, l + 1, C1, C1, "wv_v2")
                lw["wt_t2"] = loadL(t_wt_t, l + 1, C2, C2, "wt_t2")
            for t in range(ntile):
                em_s = poolT.tile([P, cpt * 4, EMW], F32, tag="em_s")
                nc.sync.dma_start(out=em_s[:], in_=t_em[:, t * cpt * 4:(t + 1) * cpt * 4, :])
                idx_s = poolT.tile([P, cpt * 4], mybir.dt.int32, tag="idx_s")
                nc.sync.dma_start(out=idx_s[:], in_=t_idxw[:, t * cpt * 4:(t + 1) * cpt * 4])
                agg = edge_tile(l, t, gsrc, lw, (em_s, idx_s, t * cpt))
                update_tile(l, t, agg, lw)
            if l < LYR - 1:
                nc.gpsimd.collective_compute(
                    "AllGather", OP.bypass,
                    replica_groups=[list(range(cfg.ncore))],
                    ins=[own[l][:]], outs=[ntab[l][:]])

    nc.compile()
    return nc


# ---------------- entry point ----------------

def _ensure_profile_hook():
    try:
        import antenv  # noqa
        import antenv.axon_hooks  # noqa
        return
    except Exception:
        pass
    try:
        import antenv
        from trn_agent_boot.trn_boot import _ntff_profile_via_ctypes
        hook = _ntff_profile_via_ctypes("/opt/axon/libaxon_pjrt.so")
        mod = types.ModuleType("antenv.axon_hooks")
        mod.get_axon_ntff_profile_hook = lambda: hook
        mod.set_axon_ntff_profile_hook = lambda h: None
        sys.modules["antenv.axon_hooks"] = mod
        antenv.axon_hooks = mod
    except Exception:
        pass


_PROGRAM_CACHE = {}


def run_cfg(inp, cfg: Cfg, trace=False):
    in_maps, real_pc = host_preprocess(inp, cfg)
    key = (cfg.ncore, cfg.npc, cfg.cpt, cfg.layers)
    if key not in _PROGRAM_CACHE:
        _PROGRAM_CACHE[key] = build_program(cfg)
    nc = _PROGRAM_CACHE[key]
    if trace:
        _ensure_profile_hook()
    res = run_bass_kernel_spmd(nc, in_maps, list(range(cfg.ncore)), trace=trace)
    node_e = np.concatenate(
        [res.results[c]["node_e"][:real_pc] for c in range(cfg.ncore)])
    return node_e, res


def kernel(**inputs):
    cfg = Cfg()
    node_e, _ = run_cfg(inputs, cfg)
    node_e = node_e[:, None] + np.asarray(inputs["b_out1"], np.float32)[None, :]
    batch = np.asarray(inputs["batch"]).astype(np.int64)
    graph = np.zeros((G, 1), np.float32)
    np.add.at(graph, batch, node_e)
    out = graph @ np.asarray(inputs["W_read"], np.float32) + np.asarray(
        inputs["b_read"], np.float32)
    return out.astype(np.float32)



# revision 4
# speedup vs baseline: 1.8598x; 1.8598x over previous
"""Trainium2 Bass kernel for the Equiformer-style GNN regressor (v2, bf16).

Strategy (8 NeuronCores, SPMD, data-parallel over nodes/edges):
  - Nodes are greedily permuted into 80 (core,tile) bins of 128 nodes each,
    balancing incoming-edge counts so every tile needs <= cpt*512 edge slots
    (cpt=8 with balancing vs 9 without).
  - Edges live in the slot array of the tile owning their dst node
    (partition = slot%128, sub-column = slot//128).
  - Per layer a DRAM node table holds per-node projected quantities in bf16:
    [P_s 128 | P_sv 64 | P_st 32 | Pv 192 (64i+c) | Pt 160 (32m+c)].
    Layer 0 uses a narrow 224-col table (v=t=0), layer 5 a 128-col table.
  - Edge phase per 512-edge chunk: gather 4x128 source rows (indirect DMA),
    radial weights w via PE (rbf -> 2-layer silu MLP), then the radial
    projections are computed EDGE-major in one matmul per 128-edge sub
    (stationary = w-slice [64,128], moving = packed wwall [64,448] holding
    [R_sa|R_s|R_v|R_vv|R_t|R_tt]).  Messages are formed with ~14 chunk-wide
    bf16 DVE ops into a contiguous hh layout [s 128 | v 192 | t 160 | ex 4]
    and aggregated per dst tile with ONE one-hot matmul per sub into PSUM.
  - Update phase: attention-denominator normalize, out-projections +
    residual + equivariant norms (bf16 PE transposes/matmuls), then the
    next layer's node-table rows are produced NODE-major directly
    (stationary = feature-major state slice) and AllGathered.
  - Final readout (per-node energies) is DMA'd out; the per-graph
    segment-sum + Linear readout runs on host.
"""
import sys
import types
from contextlib import ExitStack
from dataclasses import dataclass

import numpy as np
import ml_dtypes

import concourse.bacc as bacc
import concourse.bass as bass
import concourse.tile as tile
from concourse import mybir
from concourse.bass_utils import run_bass_kernel_spmd

F32 = mybir.dt.float32
BF16 = mybir.dt.bfloat16
I32 = mybir.dt.int32
BF = ml_dtypes.bfloat16
AF = mybir.ActivationFunctionType
OP = mybir.AluOpType

# ---------------- problem constants (hardcoded per spec) ----------------
N, E, G, L = 10000, 320000, 32, 6
C0, C1, C2, H, NB, RAD, FD, T = 128, 64, 32, 4, 128, 64, 512, 1
MAXR = 5.0
EPS = 1e-6
NCORE = 8
P = 128

GW = 576          # full node-table row width (bf16 elements)
GW0 = 224         # layer-0 row width
GW5 = 128         # layer-5 row width
MW = 448          # wwall width (R_sa|R_s|R_v|R_vv|R_t|R_tt)
HHW = 484         # hh width (s 128 | v 192 | t 160 | ex 4)
HHW5 = 132
EMW = 12          # em cols: [mask, sh1 x3, sh2 x5, dst, pad, pad]

# G row block offsets
G_SV, G_ST, G_PV, G_PT = 128, 192, 224, 416
# wwall block offsets
W_SA, W_S, W_V, W_VV, W_T, W_TT = 0, 128, 256, 320, 384, 416
# layer-0 wwall: [R_sa | R_s | R_v | R_t]
W0_V, W0_T = 256, 320
MW0 = 352
MW5 = 256


@dataclass
class Cfg:
    ncore: int = NCORE
    npc: int = 1280          # padded nodes per core (multiple of 128)
    cpt: int = 8             # chunks (512 slots) per node-tile
    layers: int = L
    nn: int = N              # real node count

    @property
    def ntile(self):
        return self.npc // P

    @property
    def np_total(self):
        return self.npc * self.ncore

    @property
    def slots(self):
        return self.ntile * self.cpt * 512


# ---------------- host-side packing ----------------

def balance_nodes(edge_dst, nbins, cap):
    """Greedy: sort nodes by in-degree desc, place into least-loaded
    non-full bin. Returns gid[node] = padded global id."""
    deg = np.bincount(edge_dst, minlength=N)
    order = np.argsort(-deg, kind="stable")
    load = np.zeros(nbins, np.int64)
    fill = np.zeros(nbins, np.int64)
    gid = np.zeros(N, np.int64)
    # heap-free greedy: argmin over non-full bins (nbins=80, N=10k -> fine)
    open_bins = np.arange(nbins)
    for n in order:
        b_i = np.argmin(load[open_bins])
        b = open_bins[b_i]
        gid[n] = b * P + fill[b]
        load[b] += deg[n]
        fill[b] += 1
        if fill[b] == cap:
            open_bins = open_bins[open_bins != b]
    return gid, load


def host_preprocess(inp, cfg: Cfg):
    npc, ncore, ntile = cfg.npc, cfg.ncore, cfg.ntile
    nbins = ncore * ntile

    pos = np.asarray(inp["pos"], np.float32)
    node_atom = np.asarray(inp["node_atom"]).astype(np.int64)
    esrc = np.asarray(inp["edge_src"]).astype(np.int64)
    edst = np.asarray(inp["edge_dst"]).astype(np.int64)

    gid, load = balance_nodes(edst, nbins, P)
    cpt_need = int(np.ceil(load.max() / 512))
    assert cpt_need <= cfg.cpt, f"need cpt {cpt_need} > cfg {cfg.cpt}"

    src_p = gid[esrc]
    dst_p = gid[edst]

    # geometry (f32, match reference formulas)
    rel = pos[edst] - pos[esrc]
    d2 = (rel * rel).sum(-1) + np.float32(EPS)
    d = np.sqrt(d2)
    u = rel / d[:, None]
    s3, s5, s15 = [np.float32(np.sqrt(x)) for x in (3.0, 5.0, 15.0)]
    sh1 = s3 * u
    x_, y_, z_ = u[:, 0], u[:, 1], u[:, 2]
    sh2 = np.stack(
        [s15 * x_ * y_, s15 * y_ * z_, np.float32(0.5) * s5 * (3 * z_ * z_ - 1.0),
         s15 * x_ * z_, np.float32(0.5) * s15 * (x_ * x_ - y_ * y_)], -1)

    tile_of_edge = dst_p // P              # global bin id
    order = np.argsort(tile_of_edge, kind="stable")
    counts = np.bincount(tile_of_edge, minlength=nbins)
    starts = np.zeros(nbins + 1, np.int64)
    np.cumsum(counts, out=starts[1:])

    S = cfg.slots
    nsub = S // P
    per_core = []
    for c in range(ncore):
        em = np.zeros((P, nsub, EMW), np.float32)
        geom = np.ones((3, S), np.float32)
        idx_em = np.zeros((P, nsub), np.int32)
        for t in range(ntile):
            gt_ = c * ntile + t
            eids = order[starts[gt_]:starts[gt_ + 1]]
            base = t * cfg.cpt * 512
            k = len(eids)
            j = base + np.arange(k)
            pp, ss = j % P, j // P
            em[pp, ss, 0] = 1.0
            em[pp, ss, 1:4] = sh1[eids]
            em[pp, ss, 4:9] = sh2[eids]
            em[pp, ss, 9] = (dst_p[eids] - (c * npc + t * P)).astype(np.float32)
            geom[0, j] = d2[eids]
            geom[1, j] = d[eids]
            idx_em[pp, ss] = src_p[eids].astype(np.int32)
        per_core.append(dict(em=em.astype(BF), geom=geom, idxw=idx_em))

    # ---- weights ----
    wd = {}
    centers = np.linspace(0.0, MAXR, NB).astype(np.float32)
    width = np.float32(MAXR / NB)
    wq = np.zeros((3, NB), np.float32)
    wq[0] = -0.5 / width**2
    wq[1] = centers / width**2
    wq[2] = -0.5 * centers**2 / width**2
    wd["wq"] = wq
    wd["wrad1"] = np.asarray(inp["Wrad1"], np.float32).astype(BF)
    wd["wrad2"] = np.asarray(inp["Wrad2"], np.float32).astype(BF)
    wd["brad1"] = np.asarray(inp["brad1"], np.float32).reshape(L, RAD, 1)
    wd["brad2"] = np.asarray(inp["brad2"], np.float32).reshape(L, RAD, 1)

    wwall = np.zeros((L, RAD, MW), np.float32)
    attn_a = np.asarray(inp["attn_a"], np.float32)
    for l in range(L):
        av = attn_a[l].reshape(C0)          # a[h(c), j(c)], c = h*32+j
        wwall[l, :, W_SA:W_SA + C0] = np.asarray(inp["Ww_s"][l]) * av[None, :]
        wwall[l, :, W_S:W_S + C0] = inp["Ww_s"][l]
        if l < L - 1:
            if l == 0:
                wwall[l, :, W0_V:W0_V + C1] = inp["Ww_v"][l]
                wwall[l, :, W0_T:W0_T + C2] = inp["Ww_t"][l]
            else:
                wwall[l, :, W_V:W_V + C1] = inp["Ww_v"][l]
                wwall[l, :, W_VV:W_VV + C1] = inp["Ww_vv"][l]
                wwall[l, :, W_T:W_T + C2] = inp["Ww_t"][l]
                wwall[l, :, W_TT:W_TT + C2] = inp["Ww_tt"][l]
    wd["wwall"] = wwall.astype(BF)

    wd["wo_s"] = np.asarray(inp["Wo_s"], np.float32).astype(BF)
    wd["wo_v"] = np.asarray(inp["Wo_v"], np.float32).astype(BF)
    wd["wo_t"] = np.asarray(inp["Wo_t"], np.float32).astype(BF)
    # packed next-layer s-projections [C0, 224] = [Ws_src | Ws_v | Ws_t]
    wsp = np.zeros((L, C0, GW0), np.float32)
    for l in range(L):
        wsp[l, :, 0:C0] = inp["Ws_src"][l]
        wsp[l, :, C0:C0 + C1] = inp["Ws_v"][l]
        wsp[l, :, C0 + C1:GW0] = inp["Ws_t"][l]
    wd["wspack"] = wsp.astype(BF)
    wd["wv_v"] = np.asarray(inp["Wv_v"], np.float32).astype(BF)
    wd["wt_t"] = np.asarray(inp["Wt_t"], np.float32).astype(BF)
    rep = lambda a: np.broadcast_to(a[:, None, :], (a.shape[0], P, a.shape[1])).copy()
    wd["lngs"] = rep(np.asarray(inp["g_s"], np.float32)).astype(BF)
    wd["lnbs"] = rep(np.asarray(inp["b_s"], np.float32)).astype(BF)
    wd["lngv"] = rep(np.asarray(inp["g_v"], np.float32)).astype(BF)
    wd["lngt"] = rep(np.asarray(inp["g_t"], np.float32)).astype(BF)
    wd["wfeat"] = np.asarray(inp["W_feat"], np.float32).astype(BF)
    wd["bfeatp"] = np.asarray(inp["b_feat"], np.float32).reshape(4, 128).T.copy()
    wd["wout1p"] = np.asarray(inp["W_out1"], np.float32).reshape(4, 128).T.copy()
    nidx = np.tile(np.arange(P, dtype=np.float32), (P, 1))
    wd["nidxb"] = nidx.astype(BF)
    wd["identb"] = np.eye(P, dtype=np.float32).astype(BF)

    # ---- initial node table (layer 0 projections, s only) ----
    s0 = np.asarray(inp["atom_emb"], np.float32)[node_atom]     # [N, C0]
    s0p = np.zeros((cfg.np_total, C0), np.float32)
    s0p[gid] = s0
    nt0 = np.zeros((cfg.np_total, GW0), np.float32)
    nt0[:, 0:C0] = s0p @ np.asarray(inp["Ws_src"][0], np.float32)
    nt0[:, C0:C0 + C1] = s0p @ np.asarray(inp["Ws_v"][0], np.float32)
    nt0[:, C0 + C1:GW0] = s0p @ np.asarray(inp["Ws_t"][0], np.float32)

    # feature-major s0 per core (bf16)
    in_maps = []
    for c in range(ncore):
        m = dict(per_core[c])
        m["ntab0"] = nt0.astype(BF)
        m["s0fm"] = s0p[c * npc:(c + 1) * npc].T.copy().astype(BF)
        for k, v in wd.items():
            m[k] = v
        in_maps.append(m)
    return in_maps, gid


# ---------------- device program ----------------

def reap(sliced: bass.AP, dims) -> bass.AP:
    """Rebuild free-dims of a sliced AP with explicit [step, count] pairs."""
    return bass.AP(sliced.tensor, sliced.offset,
                   [list(sliced.ap[0])] + [[int(s), int(c)] for s, c in dims])


def build_program(cfg: Cfg):
    nc = bacc.Bacc("TRN2", target_bir_lowering=False, debug=False,
                   enable_asserts=True, num_devices=cfg.ncore)
    npc, ntile, cpt = cfg.npc, cfg.ntile, cfg.cpt
    S = cfg.slots
    nsub = S // P
    NPT = cfg.np_total
    LYR = cfg.layers

    dp = nc.declare_dram_parameter
    t_ntab0 = dp("ntab0", [NPT, GW0], BF16, isOutput=False)
    t_s0fm = dp("s0fm", [C0, npc], BF16, isOutput=False)
    t_em = dp("em", [P, nsub, EMW], BF16, isOutput=False)
    t_geom = dp("geom", [3, S], F32, isOutput=False)
    t_idxw = dp("idxw", [P, nsub], I32, isOutput=False)
    t_wq = dp("wq", [3, NB], F32, isOutput=False)
    t_wrad1 = dp("wrad1", [LYR, NB, RAD], BF16, isOutput=False)
    t_wrad2 = dp("wrad2", [LYR, RAD, RAD], BF16, isOutput=False)
    t_brad1 = dp("brad1", [LYR, RAD, 1], F32, isOutput=False)
    t_brad2 = dp("brad2", [LYR, RAD, 1], F32, isOutput=False)
    t_wwall = dp("wwall", [LYR, RAD, MW], BF16, isOutput=False)
    t_wo_s = dp("wo_s", [LYR, C0, C0], BF16, isOutput=False)
    t_wo_v = dp("wo_v", [LYR, C1, C1], BF16, isOutput=False)
    t_wo_t = dp("wo_t", [LYR, C2, C2], BF16, isOutput=False)
    t_wspack = dp("wspack", [LYR, C0, GW0], BF16, isOutput=False)
    t_wv_v = dp("wv_v", [LYR, C1, C1], BF16, isOutput=False)
    t_wt_t = dp("wt_t", [LYR, C2, C2], BF16, isOutput=False)
    t_lngs = dp("lngs", [LYR, P, C0], BF16, isOutput=False)
    t_lnbs = dp("lnbs", [LYR, P, C0], BF16, isOutput=False)
    t_lngv = dp("lngv", [LYR, P, C1], BF16, isOutput=False)
    t_lngt = dp("lngt", [LYR, P, C2], BF16, isOutput=False)
    t_wfeat = dp("wfeat", [C0, FD], BF16, isOutput=False)
    t_bfeatp = dp("bfeatp", [P, 4], F32, isOutput=False)
    t_wout1p = dp("wout1p", [P, 4], F32, isOutput=False)
    t_nidxb = dp("nidxb", [P, P], BF16, isOutput=False)
    t_identb = dp("identb", [P, P], BF16, isOutput=False)
    t_nodee = dp("node_e", [npc], F32, isOutput=True)

    own = [nc.dram_tensor(f"own{l}", [npc, GW if l < LYR - 2 else GW5], BF16)
           for l in range(LYR - 1)]
    ntab = [nc.dram_tensor(f"ntab{l + 1}", [NPT, GW if l < LYR - 2 else GW5],
                           BF16, addr_space="Shared")
            for l in range(LYR - 1)]

    def gwid(l):
        return GW0 if l == 0 else (GW5 if l == LYR - 1 else GW)

    def mwid(l):
        return MW0 if l == 0 else (MW5 if l == LYR - 1 else MW)

    def hwid(l):
        return HHW5 if l == LYR - 1 else HHW

    with tile.TileContext(nc) as tc, ExitStack() as ctx:
        pool1 = ctx.enter_context(tc.tile_pool(name="const", bufs=1))
        poolL = ctx.enter_context(tc.tile_pool(name="layerw", bufs=1))
        poolT = ctx.enter_context(tc.tile_pool(name="tilec", bufs=2))
        poolg = ctx.enter_context(tc.tile_pool(name="gath", bufs=2))
        poole = ctx.enter_context(tc.tile_pool(name="edge", bufs=2))
        poolx = ctx.enter_context(tc.tile_pool(name="edge1", bufs=1))
        poolr = ctx.enter_context(tc.tile_pool(name="rad", bufs=2))
        poolu = ctx.enter_context(tc.tile_pool(name="upd", bufs=1))
        psA = ctx.enter_context(tc.tile_pool(name="psA", bufs=2, space="PSUM"))
        psRp = ctx.enter_context(tc.tile_pool(name="psRp", bufs=2, space="PSUM"))
        psNt = ctx.enter_context(tc.tile_pool(name="psNt", bufs=1, space="PSUM"))
        psAgg = ctx.enter_context(tc.tile_pool(name="psAgg", bufs=1, space="PSUM"))

        def load1(dram, shape, dtype=F32):
            t = pool1.tile(shape, dtype, tag=dram.name)
            nc.sync.dma_start(out=t[:], in_=dram[:])
            return t

        wq_t = load1(t_wq, [3, NB])
        nidx_t = load1(t_nidxb, [P, P], BF16)
        ident_t = load1(t_identb, [P, P], BF16)
        wfeat_t = load1(t_wfeat, [C0, FD], BF16)
        bfeatp_t = load1(t_bfeatp, [P, 4])
        wout1p_t = load1(t_wout1p, [P, 4])

        eps_t = pool1.tile([P, 1], F32, tag="epsT")
        nc.vector.memset(eps_t[:], EPS)

        # feature-major state (bf16)
        sfm = pool1.tile([C0, npc], BF16, tag="sfm")
        nc.sync.dma_start(out=sfm[:], in_=t_s0fm[:])
        vfm_t = pool1.tile([C1, 3, npc], BF16, tag="vfm")
        nc.vector.memset(vfm_t[:], 0.0)
        tfm_t = pool1.tile([C2, 5, npc], BF16, tag="tfm")
        nc.vector.memset(tfm_t[:], 0.0)

        def loadL(dram, l, p, f, tag, dtype=BF16):
            t = poolL.tile([p, f], dtype, tag=tag)
            nc.sync.dma_start(out=t[:], in_=dram[l])
            return t

        def edge_tile(l, t, gsrc, lw, em_s, idx_s):
            """Edge phase for node-tile t of layer l -> returns agg psum."""
            gw, mw, hw = gwid(l), mwid(l), hwid(l)
            tbase = t * cpt
            rbf_b = poolr.tile([NB, cpt * 512], BF16, tag="rbfb")
            w_b = poolr.tile([RAD, cpt * 512], BF16, tag="wb")
            # PH1: rbf
            for k in range(cpt):
                gsl = poolT.tile([3, 512], F32, tag="geom_c")
                nc.sync.dma_start(
                    out=gsl[:], in_=t_geom[:, (tbase + k) * 512:(tbase + k + 1) * 512])
                ps = psA.tile([NB, 512], F32, tag="mmA", space="PSUM")
                nc.tensor.matmul(ps[:], wq_t[:], gsl[:], start=True, stop=True)
                nc.scalar.activation(out=rbf_b[:, k * 512:(k + 1) * 512], in_=ps[:],
                                     func=AF.Exp)
            # PH2: radial MLP
            for k in range(cpt):
                sl = slice(k * 512, (k + 1) * 512)
                ps = psA.tile([RAD, 512], F32, tag="mmA", space="PSUM")
                nc.tensor.matmul(ps[:RAD, :], lw["wrad1"][:], rbf_b[:, sl],
                                 start=True, stop=True)
                h1 = poolx.tile([RAD, 512], BF16, tag="h1")
                nc.scalar.activation(out=h1[:], in_=ps[:RAD, :], func=AF.Silu,
                                     bias=lw["brad1"][:])
                ps2 = psA.tile([RAD, 512], F32, tag="mmA", space="PSUM")
                nc.tensor.matmul(ps2[:RAD, :], lw["wrad2"][:], h1[:],
                                 start=True, stop=True)
                nc.scalar.activation(out=w_b[:, sl], in_=ps2[:RAD, :], func=AF.Silu,
                                     bias=lw["brad2"][:])
            # PH3: gather + projections + messages + aggregation
            agg = psAgg.tile([P, HHW], F32, tag="agg", space="PSUM")
            full = 0 < l < LYR - 1
            for k in range(cpt):
                cs0 = k * 4
                # gather G rows (4 subs)
                gt = poolg.tile([P, 4, GW], BF16, tag="gt")
                for s in range(4):
                    nc.gpsimd.indirect_dma_start(
                        out=gt[:, s, 0:gw], out_offset=None, in_=gsrc[:, :],
                        in_offset=bass.IndirectOffsetOnAxis(
                            ap=idx_s[:, cs0 + s:cs0 + s + 1], axis=0))
                # edge-major radial projections + evac to sbuf bf16
                rsb = poole.tile([P, 4, MW], BF16, tag="rsb")
                for s in range(4):
                    psp = psRp.tile([P, 512], F32, tag="rp", space="PSUM")
                    nc.tensor.matmul(psp[:, 0:mw],
                                     w_b[:, k * 512 + s * 128:k * 512 + (s + 1) * 128],
                                     lw["wwall"][:, 0:mw], start=True, stop=True)
                    nc.scalar.copy(out=rsb[:, s, 0:mw], in_=psp[:, 0:mw])
                hh = poole.tile([P, 4, HHW], BF16, tag="hh")
                exoff = hw - 4
                # scr = G_s * R_sa
                scrt = poolx.tile([P, 4, C0], BF16, tag="scrt")
                nc.vector.tensor_tensor(
                    out=scrt[:], in0=reap(gt[:, 0:1, 0:1], [(GW, 4), (1, C0)]),
                    in1=reap(rsb[:, 0:1, W_SA:W_SA + 1], [(MW, 4), (1, C0)]),
                    op=OP.mult)
                # logits (reduce over 32 channels per head)
                lgt = poolx.tile([P, 4, H], F32, tag="lgt")
                nc.vector.tensor_reduce(
                    out=lgt[:],
                    in_=scrt[:].rearrange("p s (h c) -> p s h c", h=H),
                    axis=mybir.AxisListType.X, op=OP.add)
                # exp
                ext = poolx.tile([P, 4 * H], F32, tag="ext")
                nc.scalar.activation(out=ext[:], in_=lgt[:].rearrange("p s h -> p (s h)"),
                                     func=AF.Exp)
                # masked ex -> hh ex cols
                nc.vector.tensor_tensor(
                    out=reap(hh[:, 0:1, exoff:exoff + 1], [(HHW, 4), (1, H)]),
                    in0=ext[:].rearrange("p (s h) -> p s h", h=H),
                    in1=reap(em_s[:, cs0:cs0 + 1, 0:1], [(EMW, 4), (0, H)]),
                    op=OP.mult)
                # gxs = G_s * ex
                gxs = poolx.tile([P, 4, C0], BF16, tag="gxs")
                nc.vector.tensor_tensor(
                    out=reap(gxs[:, 0:1, 0:1], [(C0, 4), (32, H), (1, 32)]),
                    in0=reap(gt[:, 0:1, 0:1], [(GW, 4), (32, H), (1, 32)]),
                    in1=reap(hh[:, 0:1, exoff:exoff + 1], [(HHW, 4), (1, H), (0, 32)]),
                    op=OP.mult)
                # hh_s = gxs * R_s
                nc.vector.tensor_tensor(
                    out=reap(hh[:, 0:1, 0:1], [(HHW, 4), (1, C0)]),
                    in0=reap(gxs[:, 0:1, 0:1], [(C0, 4), (1, C0)]),
                    in1=reap(rsb[:, 0:1, W_S:W_S + 1], [(MW, 4), (1, C0)]),
                    op=OP.mult)
                if l < LYR - 1:
                    wv_off = W0_V if l == 0 else W_V
                    wt_off = W0_T if l == 0 else W_T
                    rxw = 192 if full else 96
                    rx = poolx.tile([P, 4, 192], BF16, tag="rx")
                    # rx_v = R_v * ex ; rx_t = R_t * ex
                    rxv_o, rxt_o = 0, 128 if full else 64
                    nc.vector.tensor_tensor(
                        out=reap(rx[:, 0:1, rxv_o:rxv_o + 1], [(192, 4), (16, H), (1, 16)]),
                        in0=reap(rsb[:, 0:1, wv_off:wv_off + 1], [(MW, 4), (16, H), (1, 16)]),
                        in1=reap(hh[:, 0:1, exoff:exoff + 1], [(HHW, 4), (1, H), (0, 16)]),
                        op=OP.mult)
                    nc.vector.tensor_tensor(
                        out=reap(rx[:, 0:1, rxt_o:rxt_o + 1], [(192, 4), (8, H), (1, 8)]),
                        in0=reap(rsb[:, 0:1, wt_off:wt_off + 1], [(MW, 4), (8, H), (1, 8)]),
                        in1=reap(hh[:, 0:1, exoff:exoff + 1], [(HHW, 4), (1, H), (0, 8)]),
                        op=OP.mult)
                    if full:
                        nc.vector.tensor_tensor(
                            out=reap(rx[:, 0:1, 64:65], [(192, 4), (16, H), (1, 16)]),
                            in0=reap(rsb[:, 0:1, W_VV:W_VV + 1], [(MW, 4), (16, H), (1, 16)]),
                            in1=reap(hh[:, 0:1, exoff:exoff + 1], [(HHW, 4), (1, H), (0, 16)]),
                            op=OP.mult)
                        nc.vector.tensor_tensor(
                            out=reap(rx[:, 0:1, 160:161], [(192, 4), (8, H), (1, 8)]),
                            in0=reap(rsb[:, 0:1, W_TT:W_TT + 1], [(MW, 4), (8, H), (1, 8)]),
                            in1=reap(hh[:, 0:1, exoff:exoff + 1], [(HHW, 4), (1, H), (0, 8)]),
                            op=OP.mult)
                    # m_svx = G_sv * rx_v ; m_stx = G_st * rx_t
                    msv = poolx.tile([P, 4, C1], BF16, tag="msv")
                    nc.vector.tensor_tensor(
                        out=reap(msv[:, 0:1, 0:1], [(C1, 4), (1, C1)]),
                        in0=reap(gt[:, 0:1, G_SV:G_SV + 1], [(GW, 4), (1, C1)]),
                        in1=reap(rx[:, 0:1, rxv_o:rxv_o + 1], [(192, 4), (1, C1)]),
                        op=OP.mult)
                    mst = poolx.tile([P, 4, C2], BF16, tag="mst")
                    nc.vector.tensor_tensor(
                        out=reap(mst[:, 0:1, 0:1], [(C2, 4), (1, C2)]),
                        in0=reap(gt[:, 0:1, G_ST:G_ST + 1], [(GW, 4), (1, C2)]),
                        in1=reap(rx[:, 0:1, rxt_o:rxt_o + 1], [(192, 4), (1, C2)]),
                        op=OP.mult)
                    if full:
                        # t2 terms -> hh, t1 terms -> scr2, then add
                        nc.vector.tensor_tensor(
                            out=reap(hh[:, 0:1, C0:C0 + 1], [(HHW, 4), (C1, 3), (1, C1)]),
                            in0=reap(gt[:, 0:1, G_PV:G_PV + 1], [(GW, 4), (C1, 3), (1, C1)]),
                            in1=reap(rx[:, 0:1, 64:65], [(192, 4), (0, 3), (1, C1)]),
                            op=OP.mult)
                        nc.vector.tensor_tensor(
                            out=reap(hh[:, 0:1, 320:321], [(HHW, 4), (C2, 5), (1, C2)]),
                            in0=reap(gt[:, 0:1, G_PT:G_PT + 1], [(GW, 4), (C2, 5), (1, C2)]),
                            in1=reap(rx[:, 0:1, 160:161], [(192, 4), (0, 5), (1, C2)]),
                            op=OP.mult)
                        scr2 = poolx.tile([P, 4, 352], BF16, tag="scr2")
                        nc.vector.tensor_tensor(
                            out=reap(scr2[:, 0:1, 0:1], [(352, 4), (C1, 3), (1, C1)]),
                            in0=reap(msv[:, 0:1, 0:1], [(C1, 4), (0, 3), (1, C1)]),
                            in1=reap(em_s[:, cs0:cs0 + 1, 1:2], [(EMW, 4), (1, 3), (0, C1)]),
                            op=OP.mult)
                        nc.vector.tensor_tensor(
                            out=reap(scr2[:, 0:1, 192:193], [(352, 4), (C2, 5), (1, C2)]),
                            in0=reap(mst[:, 0:1, 0:1], [(C2, 4), (0, 5), (1, C2)]),
                            in1=reap(em_s[:, cs0:cs0 + 1, 4:5], [(EMW, 4), (1, 5), (0, C2)]),
                            op=OP.mult)
                        nc.vector.tensor_tensor(
                            out=reap(hh[:, 0:1, C0:C0 + 1], [(HHW, 4), (1, 352)]),
                            in0=reap(hh[:, 0:1, C0:C0 + 1], [(HHW, 4), (1, 352)]),
                            in1=reap(scr2[:, 0:1, 0:1], [(352, 4), (1, 352)]),
                            op=OP.add)
                    else:
                        # layer 0: no t2 terms; t1 writes hh directly
                        nc.vector.tensor_tensor(
                            out=reap(hh[:, 0:1, C0:C0 + 1], [(HHW, 4), (C1, 3), (1, C1)]),
                            in0=reap(msv[:, 0:1, 0:1], [(C1, 4), (0, 3), (1, C1)]),
                            in1=reap(em_s[:, cs0:cs0 + 1, 1:2], [(EMW, 4), (1, 3), (0, C1)]),
                            op=OP.mult)
                        nc.vector.tensor_tensor(
                            out=reap(hh[:, 0:1, 320:321], [(HHW, 4), (C2, 5), (1, C2)]),
                            in0=reap(mst[:, 0:1, 0:1], [(C2, 4), (0, 5), (1, C2)]),
                            in1=reap(em_s[:, cs0:cs0 + 1, 4:5], [(EMW, 4), (1, 5), (0, C2)]),
                            op=OP.mult)
                # one-hot (dst within tile)
                ohb = poole.tile([P, 4, P], BF16, tag="ohb")
                nc.vector.tensor_tensor(
                    out=ohb[:],
                    in0=reap(nidx_t[:, 0:1], [(0, 4), (1, P)]),
                    in1=reap(em_s[:, cs0:cs0 + 1, 9:10], [(EMW, 4), (0, P)]),
                    op=OP.is_equal)
                # aggregation
                for s in range(4):
                    first = (k == 0 and s == 0)
                    last = (k == cpt - 1 and s == 3)
                    nc.tensor.matmul(agg[:, 0:hw], ohb[:, s, :], hh[:, s, 0:hw],
                                     start=first, stop=last, skip_group_check=True)
            return agg

        def transpose_to(src_ap, kparts, ffree):
            """transpose bf16 src [kparts, ffree] sbuf -> psum [ffree, kparts]"""
            ps = psA.tile([P, 512], F32, tag="mmA", space="PSUM")
            psb = ps[:, 0:P].bitcast(BF16)
            nc.tensor.transpose(psb[:ffree, :kparts], src_ap,
                                ident_t[:kparts, :kparts])
            return psb

        def update_tile(l, t, agg, lw):
            tsl = slice(t * P, (t + 1) * P)
            hw = hwid(l)
            last_v = l < LYR - 1
            rden = poolu.tile([P, H], F32, tag="rden")
            nc.vector.tensor_scalar(out=rden[:], in0=agg[:, hw - 4:hw],
                                    scalar1=1e-9, scalar2=None, op0=OP.add)
            nc.vector.reciprocal(out=rden[:], in_=rden[:])
            aggnm = poolu.tile([P, 480], BF16, tag="aggnm")
            for h in range(H):
                nc.vector.tensor_scalar(
                    out=aggnm[:, 32 * h:32 * h + 32], in0=agg[:, 32 * h:32 * h + 32],
                    scalar1=rden[:, h:h + 1], scalar2=None, op0=OP.mult)
                if last_v:
                    nc.vector.tensor_scalar(
                        out=reap(aggnm[:, 128 + 16 * h:128 + 16 * h + 1], [(64, 3), (1, 16)]),
                        in0=reap(agg[:, 128 + 16 * h:128 + 16 * h + 1], [(64, 3), (1, 16)]),
                        scalar1=rden[:, h:h + 1], scalar2=None, op0=OP.mult)
                    nc.vector.tensor_scalar(
                        out=reap(aggnm[:, 320 + 8 * h:320 + 8 * h + 1], [(32, 5), (1, 8)]),
                        in0=reap(agg[:, 320 + 8 * h:320 + 8 * h + 1], [(32, 5), (1, 8)]),
                        scalar1=rden[:, h:h + 1], scalar2=None, op0=OP.mult)

            # s out-projection + residual (feature-major)
            psS = transpose_to(aggnm[:, 0:128], P, P)
            afs = poolu.tile([P, P], BF16, tag="afs")
            nc.scalar.copy(out=afs[:], in_=psS[:, :P])
            pso = psA.tile([P, 512], F32, tag="mmA", space="PSUM")
            nc.tensor.matmul(pso[:, 0:P], lw["wo_s"][:], afs[:], start=True, stop=True)
            upd_s = poolu.tile([P, P], BF16, tag="upd_s")
            nc.vector.tensor_tensor(out=upd_s[:], in0=sfm[:, tsl], in1=pso[:, 0:P],
                                    op=OP.add)

            upd_v = poolu.tile([C1, 3, P], BF16, tag="upd_v")
            upd_t = poolu.tile([C2, 5, P], BF16, tag="upd_t")
            if last_v:
                for i in range(3):
                    psV = transpose_to(aggnm[:, 128 + 64 * i:128 + 64 * i + 64], P, C1)
                    afv = poolu.tile([C1, P], BF16, tag="afv")
                    nc.scalar.copy(out=afv[:], in_=psV[:C1, :P])
                    psv2 = psA.tile([P, 512], F32, tag="mmA", space="PSUM")
                    nc.tensor.matmul(psv2[:C1, 0:P], lw["wo_v"][:], afv[:],
                                     start=True, stop=True)
                    nc.vector.tensor_tensor(out=upd_v[:, i, :], in0=vfm_t[:, i, tsl],
                                            in1=psv2[:C1, 0:P], op=OP.add)
                for m in range(5):
                    psT_ = transpose_to(aggnm[:, 320 + 32 * m:320 + 32 * m + 32], P, C2)
                    aft = poolu.tile([C2, P], BF16, tag="aft")
                    nc.scalar.copy(out=aft[:], in_=psT_[:C2, :P])
                    pst2 = psA.tile([P, 512], F32, tag="mmA", space="PSUM")
                    nc.tensor.matmul(pst2[:C2, 0:P], lw["wo_t"][:], aft[:],
                                     start=True, stop=True)
                    nc.vector.tensor_tensor(out=upd_t[:, m, :], in0=tfm_t[:, m, tsl],
                                            in1=pst2[:C2, 0:P], op=OP.add)

            # transpose to node-major
            snm = poolu.tile([P, C0], BF16, tag="snm")
            psn = transpose_to(upd_s[:], P, P)
            nc.scalar.copy(out=snm[:], in_=psn[:, :P])
            vnm = poolu.tile([P, C1, 3], BF16, tag="vnm")
            tnm = poolu.tile([P, C2, 5], BF16, tag="tnm")
            if last_v:
                for i in range(3):
                    psn = transpose_to(upd_v[:, i, :], C1, P)
                    nc.vector.tensor_copy(
                        out=reap(vnm[:, 0:1, i:i + 1], [(3, C1)]), in_=psn[:, :C1])
                for m in range(5):
                    psn = transpose_to(upd_t[:, m, :], C2, P)
                    nc.vector.tensor_copy(
                        out=reap(tnm[:, 0:1, m:m + 1], [(5, C2)]), in_=psn[:, :C2])

            # LayerNorm on s
            stats = poolu.tile([P, 6], F32, tag="stats")
            nc.vector.bn_stats(out=stats[:], in_=snm[:])
            mv = poolu.tile([P, 2], F32, tag="mv")
            nc.vector.bn_aggr(out=mv[:], in_=stats[:])
            lnt = poolu.tile([P, 2], F32, tag="lnt")
            nc.scalar.activation(out=lnt[:, 0:1], in_=mv[:, 1:2], func=AF.Ln,
                                 bias=eps_t[:])
            nc.scalar.activation(out=lnt[:, 1:2], in_=lnt[:, 0:1], func=AF.Exp,
                                 scale=-0.5)
            nc.vector.tensor_scalar(out=snm[:], in0=snm[:], scalar1=mv[:, 0:1],
                                    scalar2=lnt[:, 1:2], op0=OP.subtract, op1=OP.mult)
            nc.vector.tensor_tensor(out=snm[:], in0=snm[:], in1=lw["lngs"][:], op=OP.mult)
            nc.vector.tensor_tensor(out=snm[:], in0=snm[:], in1=lw["lnbs"][:], op=OP.add)

            if last_v:
                vsq = poolu.tile([P, C1, 3], F32, tag="vsq")
                nc.vector.tensor_tensor(out=vsq[:], in0=vnm[:], in1=vnm[:], op=OP.mult)
                vr1 = poolu.tile([P, C1], F32, tag="vr1")
                nc.vector.tensor_reduce(out=vr1[:], in_=vsq[:],
                                        axis=mybir.AxisListType.X, op=OP.add)
                vr2 = poolu.tile([P, 1], F32, tag="vr2")
                nc.vector.tensor_reduce(out=vr2[:], in_=vr1[:],
                                        axis=mybir.AxisListType.X, op=OP.add)
                nc.scalar.activation(out=vr2[:], in_=vr2[:], func=AF.Ln,
                                     bias=eps_t[:], scale=1.0 / C1)
                nc.scalar.activation(out=vr2[:], in_=vr2[:], func=AF.Exp, scale=-0.5)
                nc.vector.tensor_scalar(out=vnm[:], in0=vnm[:], scalar1=vr2[:],
                                        scalar2=None, op0=OP.mult)
                nc.vector.tensor_tensor(
                    out=vnm[:], in0=vnm[:],
                    in1=reap(lw["lngv"][:, 0:1], [(1, C1), (0, 3)]), op=OP.mult)
                tsq = poolu.tile([P, C2, 5], F32, tag="tsq")
                nc.vector.tensor_tensor(out=tsq[:], in0=tnm[:], in1=tnm[:], op=OP.mult)
                tr1 = poolu.tile([P, C2], F32, tag="tr1")
                nc.vector.tensor_reduce(out=tr1[:], in_=tsq[:],
                                        axis=mybir.AxisListType.X, op=OP.add)
                tr2 = poolu.tile([P, 1], F32, tag="tr2")
                nc.vector.tensor_reduce(out=tr2[:], in_=tr1[:],
                                        axis=mybir.AxisListType.X, op=OP.add)
                nc.scalar.activation(out=tr2[:], in_=tr2[:], func=AF.Ln,
                                     bias=eps_t[:], scale=1.0 / C2)
                nc.scalar.activation(out=tr2[:], in_=tr2[:], func=AF.Exp, scale=-0.5)
                nc.vector.tensor_scalar(out=tnm[:], in0=tnm[:], scalar1=tr2[:],
                                        scalar2=None, op0=OP.mult)
                nc.vector.tensor_tensor(
                    out=tnm[:], in0=tnm[:],
                    in1=reap(lw["lngt"][:, 0:1], [(1, C2), (0, 5)]), op=OP.mult)

            # write back feature-major state
            psn = transpose_to(snm[:], P, P)
            nc.scalar.copy(out=sfm[:, tsl], in_=psn[:, :P])
            if last_v:
                for i in range(3):
                    psn = transpose_to(reap(vnm[:, 0:1, i:i + 1], [(3, C1)]), P, C1)
                    nc.scalar.copy(out=vfm_t[:, i, tsl], in_=psn[:C1, :P])
                for m in range(5):
                    psn = transpose_to(reap(tnm[:, 0:1, m:m + 1], [(5, C2)]), P, C2)
                    nc.scalar.copy(out=tfm_t[:, m, tsl], in_=psn[:C2, :P])

            if last_v:
                # next-layer table rows, node-major (stationary = fm state)
                gwn = gwid(l + 1)
                ntrow = poolu.tile([P, GW], BF16, tag="ntrow")
                if l < LYR - 2:
                    ntA = psNt.tile([P, GW0], F32, tag="ntA", space="PSUM")
                    nc.tensor.matmul(ntA[:], sfm[:, tsl], lw["wspack"][:],
                                     start=True, stop=True)
                    nc.scalar.copy(out=ntrow[:, 0:GW0], in_=ntA[:])
                    ntB = psNt.tile([P, 352], F32, tag="ntB", space="PSUM")
                    for i in range(3):
                        nc.tensor.matmul(ntB[:, 64 * i:64 * i + 64],
                                         vfm_t[:, i, tsl], lw["wv_v"][:],
                                         start=True, stop=True, skip_group_check=True)
                    for m in range(5):
                        nc.tensor.matmul(ntB[:, 192 + 32 * m:192 + 32 * m + 32],
                                         tfm_t[:, m, tsl], lw["wt_t"][:],
                                         start=True, stop=True, skip_group_check=True)
                    nc.scalar.copy(out=ntrow[:, GW0:GW], in_=ntB[:])
                    nc.sync.dma_start(out=own[l][tsl, :], in_=ntrow[:, 0:GW])
                else:
                    # l == 4: only P_s for layer 5
                    ntA = psNt.tile([P, GW0], F32, tag="ntA", space="PSUM")
                    nc.tensor.matmul(ntA[:, 0:C0], sfm[:, tsl],
                                     lw["wspack"][:, 0:C0], start=True, stop=True)
                    nc.scalar.copy(out=ntrow[:, 0:C0], in_=ntA[:, 0:C0])
                    nc.sync.dma_start(out=own[l][tsl, :], in_=ntrow[:, 0:GW5])
            else:
                # final readout head
                feat = poolu.tile([P, 4, P], F32, tag="feat")
                for b in range(4):
                    ps = psA.tile([P, 512], F32, tag="mmA", space="PSUM")
                    nc.tensor.matmul(ps[:, 0:P], wfeat_t[:, b * 128:(b + 1) * 128],
                                     sfm[:, tsl], start=True, stop=True)
                    nc.scalar.activation(out=feat[:, b, :], in_=ps[:, 0:P],
                                         func=AF.Gelu_apprx_tanh,
                                         bias=bfeatp_t[:, b:b + 1])
                pse = psNt.tile([1, P], F32, tag="pse", space="PSUM")
                for b in range(4):
                    nc.tensor.matmul(pse[:], wout1p_t[:, b:b + 1], feat[:, b, :],
                                     start=(b == 0), stop=(b == 3))
                ne = poolu.tile([1, P], F32, tag="ne")
                nc.vector.tensor_copy(out=ne[:], in_=pse[:])
                nc.sync.dma_start(out=t_nodee[tsl], in_=ne[0:1, :])

        for l in range(LYR):
            gsrc = t_ntab0 if l == 0 else ntab[l - 1]
            lw = dict(
                wrad1=loadL(t_wrad1, l, NB, RAD, "wrad1"),
                wrad2=loadL(t_wrad2, l, RAD, RAD, "wrad2"),
                brad1=loadL(t_brad1, l, RAD, 1, "brad1", F32),
                brad2=loadL(t_brad2, l, RAD, 1, "brad2", F32),
                wwall=loadL(t_wwall, l, RAD, MW, "wwall"),
                wo_s=loadL(t_wo_s, l, C0, C0, "wo_s"),
                lngs=loadL(t_lngs, l, P, C0, "lngs"),
                lnbs=loadL(t_lnbs, l, P, C0, "lnbs"),
            )
            if l < LYR - 1:
                lw["wo_v"] = loadL(t_wo_v, l, C1, C1, "wo_v")
                lw["wo_t"] = loadL(t_wo_t, l, C2, C2, "wo_t")
                lw["lngv"] = loadL(t_lngv, l, P, C1, "lngv")
                lw["lngt"] = loadL(t_lngt, l, P, C2, "lngt")
                lw["wspack"] = loadL(t_wspack, l + 1, C0, GW0, "wspack")
                lw["wv_v"] = loadL(t_wv_v, l + 1, C1, C1, "wv_v")
                lw["wt_t"] = loadL(t_wt_t, l + 1, C2, C2, "wt_t")
            for t in range(ntile):
                em_s = poolT.tile([P, cpt * 4, EMW], BF16, tag="em_s")
                nc.sync.dma_start(out=em_s[:],
                                  in_=t_em[:, t * cpt * 4:(t + 1) * cpt * 4, :])
                idx_s = poolT.tile([P, cpt * 4], I32, tag="idx_s")
                nc.sync.dma_start(out=idx_s[:],
                                  in_=t_idxw[:, t * cpt * 4:(t + 1) * cpt * 4])
                agg = edge_tile(l, t, gsrc, lw, em_s, idx_s)
                update_tile(l, t, agg, lw)
            if l < LYR - 1:
                nc.gpsimd.collective_compute(
                    "AllGather", OP.bypass,
                    replica_groups=[list(range(cfg.ncore))],
                    ins=[own[l][:]], outs=[ntab[l][:]])

    nc.compile()
    return nc


# ---------------- entry point ----------------

def _ensure_profile_hook():
    try:
        import antenv  # noqa
        import antenv.axon_hooks  # noqa
        return
    except Exception:
        pass
    try:
        import antenv
        from trn_agent_boot.trn_boot import _ntff_profile_via_ctypes
        hook = _ntff_profile_via_ctypes("/opt/axon/libaxon_pjrt.so")
        mod = types.ModuleType("antenv.axon_hooks")
        mod.get_axon_ntff_profile_hook = lambda: hook
        mod.set_axon_ntff_profile_hook = lambda h: None
        sys.modules["antenv.axon_hooks"] = mod
        antenv.axon_hooks = mod
    except Exception:
        pass


_PROGRAM_CACHE = {}


def run_cfg(inp, cfg: Cfg, trace=False):
    in_maps, gid = host_preprocess(inp, cfg)
    key = (cfg.ncore, cfg.npc, cfg.cpt, cfg.layers)
    if key not in _PROGRAM_CACHE:
        _PROGRAM_CACHE[key] = build_program(cfg)
    nc = _PROGRAM_CACHE[key]
    if trace:
        _ensure_profile_hook()
    res = run_bass_kernel_spmd(nc, in_maps, list(range(cfg.ncore)), trace=trace)
    node_e_pad = np.concatenate(
        [np.asarray(res.results[c]["node_e"]) for c in range(cfg.ncore)])
    node_e = node_e_pad[gid]          # invert node permutation
    return node_e, res


def kernel(**inputs):
    cfg = Cfg()
    node_e, _ = run_cfg(inputs, cfg)
    node_e = node_e[:, None] + np.asarray(inputs["b_out1"], np.float32)[None, :]
    batch = np.asarray(inputs["batch"]).astype(np.int64)
    graph = np.zeros((G, 1), np.float32)
    np.add.at(graph, batch, node_e)
    out = graph @ np.asarray(inputs["W_read"], np.float32) + np.asarray(
        inputs["b_read"], np.float32)
    return out.astype(np.float32)


# revision 5
# speedup vs baseline: 1.9994x; 1.0751x over previous
"""Trainium2 Bass kernel for the Equiformer-style GNN regressor (v2, bf16).

Strategy (8 NeuronCores, SPMD, data-parallel over nodes/edges):
  - Nodes are greedily permuted into 80 (core,tile) bins of 128 nodes each,
    balancing incoming-edge counts so every tile needs <= cpt*512 edge slots
    (cpt=8 with balancing vs 9 without).
  - Edges live in the slot array of the tile owning their dst node
    (partition = slot%128, sub-column = slot//128).
  - Per layer a DRAM node table holds per-node projected quantities in bf16:
    [P_s 128 | P_sv 64 | P_st 32 | Pv 192 (64i+c) | Pt 160 (32m+c)].
    Layer 0 uses a narrow 224-col table (v=t=0), layer 5 a 128-col table.
  - Edge phase per 512-edge chunk: gather 4x128 source rows (indirect DMA),
    radial weights w via PE (rbf -> 2-layer silu MLP), then the radial
    projections are computed EDGE-major in one matmul per 128-edge sub
    (stationary = w-slice [64,128], moving = packed wwall [64,448] holding
    [R_sa|R_s|R_v|R_vv|R_t|R_tt]).  Messages are formed with ~14 chunk-wide
    bf16 DVE ops into a contiguous hh layout [s 128 | v 192 | t 160 | ex 4]
    and aggregated per dst tile with ONE one-hot matmul per sub into PSUM.
  - Update phase: attention-denominator normalize, out-projections +
    residual + equivariant norms (bf16 PE transposes/matmuls), then the
    next layer's node-table rows are produced NODE-major directly
    (stationary = feature-major state slice) and AllGathered.
  - Final readout (per-node energies) is DMA'd out; the per-graph
    segment-sum + Linear readout runs on host.
"""
import sys
import types
from contextlib import ExitStack
from dataclasses import dataclass

import numpy as np
import ml_dtypes

import concourse.bacc as bacc
import concourse.bass as bass
import concourse.tile as tile
from concourse import mybir
from concourse.bass_utils import run_bass_kernel_spmd

F32 = mybir.dt.float32
BF16 = mybir.dt.bfloat16
I32 = mybir.dt.int32
BF = ml_dtypes.bfloat16
AF = mybir.ActivationFunctionType
OP = mybir.AluOpType

# ---------------- problem constants (hardcoded per spec) ----------------
N, E, G, L = 10000, 320000, 32, 6
C0, C1, C2, H, NB, RAD, FD, T = 128, 64, 32, 4, 128, 64, 512, 1
MAXR = 5.0
EPS = 1e-6
NCORE = 8
P = 128

GW = 576          # full node-table row width (bf16 elements)
GW0 = 224         # layer-0 row width
GW5 = 128         # layer-5 row width
MW = 448          # wwall width (R_sa|R_s|R_v|R_vv|R_t|R_tt)
HHW = 484         # hh width (s 128 | v 192 | t 160 | ex 4)
HHW5 = 132
EMW = 16          # em cols: [mask@0, sh1@2:5, sh2@6:11, dst@12] (4B-aligned starts)

# G row block offsets
G_SV, G_ST, G_PV, G_PT = 128, 192, 224, 416
# wwall block offsets
W_SA, W_S, W_V, W_VV, W_T, W_TT = 0, 128, 256, 320, 384, 416
# layer-0 wwall: [R_sa | R_s | R_v | R_t]
W0_V, W0_T = 256, 320
MW0 = 352
MW5 = 256


@dataclass
class Cfg:
    ncore: int = NCORE
    npc: int = 1280          # padded nodes per core (multiple of 128)
    cpt: int = 8             # chunks (512 slots) per node-tile
    layers: int = L
    nn: int = N              # real node count

    @property
    def ntile(self):
        return self.npc // P

    @property
    def np_total(self):
        return self.npc * self.ncore

    @property
    def slots(self):
        return self.ntile * self.cpt * 512


# ---------------- host-side packing ----------------

def balance_nodes(edge_dst, nbins, cap):
    """Greedy: sort nodes by in-degree desc, place into least-loaded
    non-full bin. Returns gid[node] = padded global id."""
    deg = np.bincount(edge_dst, minlength=N)
    order = np.argsort(-deg, kind="stable")
    load = np.zeros(nbins, np.int64)
    fill = np.zeros(nbins, np.int64)
    gid = np.zeros(N, np.int64)
    # heap-free greedy: argmin over non-full bins (nbins=80, N=10k -> fine)
    open_bins = np.arange(nbins)
    for n in order:
        b_i = np.argmin(load[open_bins])
        b = open_bins[b_i]
        gid[n] = b * P + fill[b]
        load[b] += deg[n]
        fill[b] += 1
        if fill[b] == cap:
            open_bins = open_bins[open_bins != b]
    return gid, load


def host_preprocess(inp, cfg: Cfg):
    npc, ncore, ntile = cfg.npc, cfg.ncore, cfg.ntile
    nbins = ncore * ntile

    pos = np.asarray(inp["pos"], np.float32)
    node_atom = np.asarray(inp["node_atom"]).astype(np.int64)
    esrc = np.asarray(inp["edge_src"]).astype(np.int64)
    edst = np.asarray(inp["edge_dst"]).astype(np.int64)

    gid, load = balance_nodes(edst, nbins, P)
    cpt_need = int(np.ceil(load.max() / 512))
    assert cpt_need <= cfg.cpt, f"need cpt {cpt_need} > cfg {cfg.cpt}"

    src_p = gid[esrc]
    dst_p = gid[edst]

    # geometry (f32, match reference formulas)
    rel = pos[edst] - pos[esrc]
    d2 = (rel * rel).sum(-1) + np.float32(EPS)
    d = np.sqrt(d2)
    u = rel / d[:, None]
    s3, s5, s15 = [np.float32(np.sqrt(x)) for x in (3.0, 5.0, 15.0)]
    sh1 = s3 * u
    x_, y_, z_ = u[:, 0], u[:, 1], u[:, 2]
    sh2 = np.stack(
        [s15 * x_ * y_, s15 * y_ * z_, np.float32(0.5) * s5 * (3 * z_ * z_ - 1.0),
         s15 * x_ * z_, np.float32(0.5) * s15 * (x_ * x_ - y_ * y_)], -1)

    tile_of_edge = dst_p // P              # global bin id
    order = np.argsort(tile_of_edge, kind="stable")
    counts = np.bincount(tile_of_edge, minlength=nbins)
    starts = np.zeros(nbins + 1, np.int64)
    np.cumsum(counts, out=starts[1:])

    S = cfg.slots
    nsub = S // P
    per_core = []
    for c in range(ncore):
        em = np.zeros((P, nsub, EMW), np.float32)
        geom = np.ones((3, S), np.float32)
        idx_em = np.zeros((P, nsub), np.int32)
        for t in range(ntile):
            gt_ = c * ntile + t
            eids = order[starts[gt_]:starts[gt_ + 1]]
            base = t * cfg.cpt * 512
            k = len(eids)
            j = base + np.arange(k)
            pp, ss = j % P, j // P
            em[pp, ss, 0] = 1.0
            em[pp, ss, 2:5] = sh1[eids]
            em[pp, ss, 6:11] = sh2[eids]
            em[pp, ss, 12] = (dst_p[eids] - (c * npc + t * P)).astype(np.float32)
            geom[0, j] = d2[eids]
            geom[1, j] = d[eids]
            idx_em[pp, ss] = src_p[eids].astype(np.int32)
        per_core.append(dict(em=em.astype(BF), geom=geom, idxw=idx_em))

    # ---- weights ----
    wd = {}
    centers = np.linspace(0.0, MAXR, NB).astype(np.float32)
    width = np.float32(MAXR / NB)
    wq = np.zeros((3, NB), np.float32)
    wq[0] = -0.5 / width**2
    wq[1] = centers / width**2
    wq[2] = -0.5 * centers**2 / width**2
    wd["wq"] = wq
    wd["wrad1"] = np.asarray(inp["Wrad1"], np.float32).astype(BF)
    wd["wrad2"] = np.asarray(inp["Wrad2"], np.float32).astype(BF)
    wd["brad1"] = np.asarray(inp["brad1"], np.float32).reshape(L, RAD, 1)
    wd["brad2"] = np.asarray(inp["brad2"], np.float32).reshape(L, RAD, 1)

    wwall = np.zeros((L, RAD, MW), np.float32)
    attn_a = np.asarray(inp["attn_a"], np.float32)
    for l in range(L):
        av = attn_a[l].reshape(C0)          # a[h(c), j(c)], c = h*32+j
        wwall[l, :, W_SA:W_SA + C0] = np.asarray(inp["Ww_s"][l]) * av[None, :]
        wwall[l, :, W_S:W_S + C0] = inp["Ww_s"][l]
        if l < L - 1:
            if l == 0:
                wwall[l, :, W0_V:W0_V + C1] = inp["Ww_v"][l]
                wwall[l, :, W0_T:W0_T + C2] = inp["Ww_t"][l]
            else:
                wwall[l, :, W_V:W_V + C1] = inp["Ww_v"][l]
                wwall[l, :, W_VV:W_VV + C1] = inp["Ww_vv"][l]
                wwall[l, :, W_T:W_T + C2] = inp["Ww_t"][l]
                wwall[l, :, W_TT:W_TT + C2] = inp["Ww_tt"][l]
    wd["wwall"] = wwall.astype(BF)

    wd["wo_s"] = np.asarray(inp["Wo_s"], np.float32).astype(BF)
    wd["wo_v"] = np.asarray(inp["Wo_v"], np.float32).astype(BF)
    wd["wo_t"] = np.asarray(inp["Wo_t"], np.float32).astype(BF)
    # packed next-layer s-projections [C0, 224] = [Ws_src | Ws_v | Ws_t]
    wsp = np.zeros((L, C0, GW0), np.float32)
    for l in range(L):
        wsp[l, :, 0:C0] = inp["Ws_src"][l]
        wsp[l, :, C0:C0 + C1] = inp["Ws_v"][l]
        wsp[l, :, C0 + C1:GW0] = inp["Ws_t"][l]
    wd["wspack"] = wsp.astype(BF)
    wd["wv_v"] = np.asarray(inp["Wv_v"], np.float32).astype(BF)
    wd["wt_t"] = np.asarray(inp["Wt_t"], np.float32).astype(BF)
    rep = lambda a: np.broadcast_to(a[:, None, :], (a.shape[0], P, a.shape[1])).copy()
    wd["lngs"] = rep(np.asarray(inp["g_s"], np.float32)).astype(BF)
    wd["lnbs"] = rep(np.asarray(inp["b_s"], np.float32)).astype(BF)
    wd["lngv"] = rep(np.asarray(inp["g_v"], np.float32)).astype(BF)
    wd["lngt"] = rep(np.asarray(inp["g_t"], np.float32)).astype(BF)
    wd["wfeat"] = np.asarray(inp["W_feat"], np.float32).astype(BF)
    wd["bfeatp"] = np.asarray(inp["b_feat"], np.float32).reshape(4, 128).T.copy()
    wd["wout1p"] = np.asarray(inp["W_out1"], np.float32).reshape(4, 128).T.copy()
    nidx = np.tile(np.arange(P, dtype=np.float32), (P, 1))
    wd["nidxb"] = nidx.astype(BF)
    wd["identb"] = np.eye(P, dtype=np.float32).astype(BF)

    # ---- initial node table (layer 0 projections, s only) ----
    s0 = np.asarray(inp["atom_emb"], np.float32)[node_atom]     # [N, C0]
    s0p = np.zeros((cfg.np_total, C0), np.float32)
    s0p[gid] = s0
    nt0 = np.zeros((cfg.np_total, GW0), np.float32)
    nt0[:, 0:C0] = s0p @ np.asarray(inp["Ws_src"][0], np.float32)
    nt0[:, C0:C0 + C1] = s0p @ np.asarray(inp["Ws_v"][0], np.float32)
    nt0[:, C0 + C1:GW0] = s0p @ np.asarray(inp["Ws_t"][0], np.float32)

    # feature-major s0 per core (bf16)
    in_maps = []
    for c in range(ncore):
        m = dict(per_core[c])
        m["ntab0"] = nt0.astype(BF)
        m["s0fm"] = s0p[c * npc:(c + 1) * npc].T.copy().astype(BF)
        for k, v in wd.items():
            m[k] = v
        in_maps.append(m)
    return in_maps, gid


# ---------------- device program ----------------

def reap(sliced: bass.AP, dims) -> bass.AP:
    """Rebuild free-dims of a sliced AP with explicit [step, count] pairs."""
    return bass.AP(sliced.tensor, sliced.offset,
                   [list(sliced.ap[0])] + [[int(s), int(c)] for s, c in dims])


def build_program(cfg: Cfg):
    nc = bacc.Bacc("TRN2", target_bir_lowering=False, debug=False,
                   enable_asserts=True, num_devices=cfg.ncore)
    npc, ntile, cpt = cfg.npc, cfg.ntile, cfg.cpt
    S = cfg.slots
    nsub = S // P
    NPT = cfg.np_total
    LYR = cfg.layers

    dp = nc.declare_dram_parameter
    t_ntab0 = dp("ntab0", [NPT, GW0], BF16, isOutput=False)
    t_s0fm = dp("s0fm", [C0, npc], BF16, isOutput=False)
    t_em = dp("em", [P, nsub, EMW], BF16, isOutput=False)
    t_geom = dp("geom", [3, S], F32, isOutput=False)
    t_idxw = dp("idxw", [P, nsub], I32, isOutput=False)
    t_wq = dp("wq", [3, NB], F32, isOutput=False)
    t_wrad1 = dp("wrad1", [LYR, NB, RAD], BF16, isOutput=False)
    t_wrad2 = dp("wrad2", [LYR, RAD, RAD], BF16, isOutput=False)
    t_brad1 = dp("brad1", [LYR, RAD, 1], F32, isOutput=False)
    t_brad2 = dp("brad2", [LYR, RAD, 1], F32, isOutput=False)
    t_wwall = dp("wwall", [LYR, RAD, MW], BF16, isOutput=False)
    t_wo_s = dp("wo_s", [LYR, C0, C0], BF16, isOutput=False)
    t_wo_v = dp("wo_v", [LYR, C1, C1], BF16, isOutput=False)
    t_wo_t = dp("wo_t", [LYR, C2, C2], BF16, isOutput=False)
    t_wspack = dp("wspack", [LYR, C0, GW0], BF16, isOutput=False)
    t_wv_v = dp("wv_v", [LYR, C1, C1], BF16, isOutput=False)
    t_wt_t = dp("wt_t", [LYR, C2, C2], BF16, isOutput=False)
    t_lngs = dp("lngs", [LYR, P, C0], BF16, isOutput=False)
    t_lnbs = dp("lnbs", [LYR, P, C0], BF16, isOutput=False)
    t_lngv = dp("lngv", [LYR, P, C1], BF16, isOutput=False)
    t_lngt = dp("lngt", [LYR, P, C2], BF16, isOutput=False)
    t_wfeat = dp("wfeat", [C0, FD], BF16, isOutput=False)
    t_bfeatp = dp("bfeatp", [P, 4], F32, isOutput=False)
    t_wout1p = dp("wout1p", [P, 4], F32, isOutput=False)
    t_nidxb = dp("nidxb", [P, P], BF16, isOutput=False)
    t_identb = dp("identb", [P, P], BF16, isOutput=False)
    t_nodee = dp("node_e", [npc], F32, isOutput=True)

    own = [nc.dram_tensor(f"own{l}", [npc, GW if l < LYR - 2 else GW5], BF16)
           for l in range(LYR - 1)]
    ntab = [nc.dram_tensor(f"ntab{l + 1}", [NPT, GW if l < LYR - 2 else GW5],
                           BF16, addr_space="Shared")
            for l in range(LYR - 1)]

    def gwid(l):
        return GW0 if l == 0 else (GW5 if l == LYR - 1 else GW)

    def mwid(l):
        return MW0 if l == 0 else (MW5 if l == LYR - 1 else MW)

    def hwid(l):
        return HHW5 if l == LYR - 1 else HHW

    with tile.TileContext(nc) as tc, ExitStack() as ctx:
        pool1 = ctx.enter_context(tc.tile_pool(name="const", bufs=1))
        poolL = ctx.enter_context(tc.tile_pool(name="layerw", bufs=1))
        poolT = ctx.enter_context(tc.tile_pool(name="tilec", bufs=2))
        poolg = ctx.enter_context(tc.tile_pool(name="gath", bufs=3))
        poole = ctx.enter_context(tc.tile_pool(name="edge", bufs=3))
        poolx = ctx.enter_context(tc.tile_pool(name="edge1", bufs=1))
        poolr = ctx.enter_context(tc.tile_pool(name="rad", bufs=2))
        poolu = ctx.enter_context(tc.tile_pool(name="upd", bufs=1))
        psA = ctx.enter_context(tc.tile_pool(name="psA", bufs=4, space="PSUM"))
        psRp = ctx.enter_context(tc.tile_pool(name="psRp", bufs=3, space="PSUM"))
        psAgg = ctx.enter_context(tc.tile_pool(name="psAgg", bufs=1, space="PSUM"))

        def load1(dram, shape, dtype=F32):
            t = pool1.tile(shape, dtype, tag=dram.name)
            nc.sync.dma_start(out=t[:], in_=dram[:])
            return t

        wq_t = load1(t_wq, [3, NB])
        nidx_t = load1(t_nidxb, [P, P], BF16)
        ident_t = load1(t_identb, [P, P], BF16)
        wfeat_t = load1(t_wfeat, [C0, FD], BF16)
        bfeatp_t = load1(t_bfeatp, [P, 4])
        wout1p_t = load1(t_wout1p, [P, 4])

        eps_t = pool1.tile([P, 1], F32, tag="epsT")
        nc.vector.memset(eps_t[:], EPS)

        # feature-major state (bf16)
        sfm = pool1.tile([C0, npc], BF16, tag="sfm")
        nc.sync.dma_start(out=sfm[:], in_=t_s0fm[:])
        vfm_t = pool1.tile([C1, 3, npc], BF16, tag="vfm")
        nc.vector.memset(vfm_t[:], 0.0)
        tfm_t = pool1.tile([C2, 5, npc], BF16, tag="tfm")
        nc.vector.memset(tfm_t[:], 0.0)

        def loadL(dram, l, p, f, tag, dtype=BF16):
            t = poolL.tile([p, f], dtype, tag=tag)
            nc.sync.dma_start(out=t[:], in_=dram[l])
            return t

        def edge_tile(l, t, gsrc, lw, em_s, idx_s):
            """Edge phase for node-tile t of layer l -> returns agg psum."""
            gw, mw, hw = gwid(l), mwid(l), hwid(l)
            tbase = t * cpt
            rbf_b = poolr.tile([NB, cpt * 512], BF16, tag="rbfb")
            w_b = poolr.tile([RAD, cpt * 512], BF16, tag="wb")
            # PH1: rbf
            for k in range(cpt):
                gsl = poolT.tile([3, 512], F32, tag="geom_c")
                nc.sync.dma_start(
                    out=gsl[:], in_=t_geom[:, (tbase + k) * 512:(tbase + k + 1) * 512])
                ps = psA.tile([NB, 512], F32, tag="mmA", space="PSUM")
                nc.tensor.matmul(ps[:], wq_t[:], gsl[:], start=True, stop=True)
                nc.scalar.activation(out=rbf_b[:, k * 512:(k + 1) * 512], in_=ps[:],
                                     func=AF.Exp)
            # PH2: radial MLP
            for k in range(cpt):
                sl = slice(k * 512, (k + 1) * 512)
                ps = psA.tile([RAD, 512], F32, tag="mmA", space="PSUM")
                nc.tensor.matmul(ps[:RAD, :], lw["wrad1"][:], rbf_b[:, sl],
                                 start=True, stop=True)
                h1 = poolx.tile([RAD, 512], BF16, tag="h1")
                nc.scalar.activation(out=h1[:], in_=ps[:RAD, :], func=AF.Silu,
                                     bias=lw["brad1"][:])
                ps2 = psA.tile([RAD, 512], F32, tag="mmA", space="PSUM")
                nc.tensor.matmul(ps2[:RAD, :], lw["wrad2"][:], h1[:],
                                 start=True, stop=True)
                nc.scalar.activation(out=w_b[:, sl], in_=ps2[:RAD, :], func=AF.Silu,
                                     bias=lw["brad2"][:])
            # PH3: gather + projections + messages + aggregation
            agg = psAgg.tile([P, HHW], F32, tag="agg", space="PSUM")
            full = 0 < l < LYR - 1
            for k in range(cpt):
                cs0 = k * 4
                # gather G rows (4 subs)
                gt = poolg.tile([P, 4, GW], BF16, tag="gt")
                for s in range(4):
                    nc.gpsimd.indirect_dma_start(
                        out=gt[:, s, 0:gw], out_offset=None, in_=gsrc[:, :],
                        in_offset=bass.IndirectOffsetOnAxis(
                            ap=idx_s[:, cs0 + s:cs0 + s + 1], axis=0))
                # edge-major radial projections + evac to sbuf bf16
                rsb = poole.tile([P, 4, MW], BF16, tag="rsb")
                for s in range(4):
                    psp = psRp.tile([P, 512], F32, tag="rp", space="PSUM")
                    nc.tensor.matmul(psp[:, 0:mw],
                                     w_b[:, k * 512 + s * 128:k * 512 + (s + 1) * 128],
                                     lw["wwall"][:, 0:mw], start=True, stop=True)
                    nc.scalar.copy(out=rsb[:, s, 0:mw], in_=psp[:, 0:mw])
                hh = poole.tile([P, 4, HHW], BF16, tag="hh")
                exoff = hw - 4
                # scr = G_s * R_sa
                scrt = poolx.tile([P, 4, C0], BF16, tag="scrt")
                nc.vector.tensor_tensor(
                    out=scrt[:], in0=reap(gt[:, 0:1, 0:1], [(GW, 4), (1, C0)]),
                    in1=reap(rsb[:, 0:1, W_SA:W_SA + 1], [(MW, 4), (1, C0)]),
                    op=OP.mult)
                # logits (reduce over 32 channels per head)
                lgt = poolx.tile([P, 4, H], F32, tag="lgt")
                nc.vector.tensor_reduce(
                    out=lgt[:],
                    in_=scrt[:].rearrange("p s (h c) -> p s h c", h=H),
                    axis=mybir.AxisListType.X, op=OP.add)
                # exp
                ext = poolx.tile([P, 4 * H], F32, tag="ext")
                nc.scalar.activation(out=ext[:], in_=lgt[:].rearrange("p s h -> p (s h)"),
                                     func=AF.Exp)
                # masked ex -> hh ex cols
                nc.vector.tensor_tensor(
                    out=reap(hh[:, 0:1, exoff:exoff + 1], [(HHW, 4), (1, H)]),
                    in0=ext[:].rearrange("p (s h) -> p s h", h=H),
                    in1=reap(em_s[:, cs0:cs0 + 1, 0:1], [(EMW, 4), (0, H)]),
                    op=OP.mult)
                # gxs = G_s * ex
                gxs = poolx.tile([P, 4, C0], BF16, tag="gxs")
                nc.vector.tensor_tensor(
                    out=reap(gxs[:, 0:1, 0:1], [(C0, 4), (32, H), (1, 32)]),
                    in0=reap(gt[:, 0:1, 0:1], [(GW, 4), (32, H), (1, 32)]),
                    in1=reap(hh[:, 0:1, exoff:exoff + 1], [(HHW, 4), (1, H), (0, 32)]),
                    op=OP.mult)
                # hh_s = gxs * R_s
                nc.vector.tensor_tensor(
                    out=reap(hh[:, 0:1, 0:1], [(HHW, 4), (1, C0)]),
                    in0=reap(gxs[:, 0:1, 0:1], [(C0, 4), (1, C0)]),
                    in1=reap(rsb[:, 0:1, W_S:W_S + 1], [(MW, 4), (1, C0)]),
                    op=OP.mult)
                if l < LYR - 1:
                    wv_off = W0_V if l == 0 else W_V
                    wt_off = W0_T if l == 0 else W_T
                    rxw = 192 if full else 96
                    rx = poolx.tile([P, 4, 192], BF16, tag="rx")
                    # rx_v = R_v * ex ; rx_t = R_t * ex
                    rxv_o, rxt_o = 0, 128 if full else 64
                    nc.vector.tensor_tensor(
                        out=reap(rx[:, 0:1, rxv_o:rxv_o + 1], [(192, 4), (16, H), (1, 16)]),
                        in0=reap(rsb[:, 0:1, wv_off:wv_off + 1], [(MW, 4), (16, H), (1, 16)]),
                        in1=reap(hh[:, 0:1, exoff:exoff + 1], [(HHW, 4), (1, H), (0, 16)]),
                        op=OP.mult)
                    nc.vector.tensor_tensor(
                        out=reap(rx[:, 0:1, rxt_o:rxt_o + 1], [(192, 4), (8, H), (1, 8)]),
                        in0=reap(rsb[:, 0:1, wt_off:wt_off + 1], [(MW, 4), (8, H), (1, 8)]),
                        in1=reap(hh[:, 0:1, exoff:exoff + 1], [(HHW, 4), (1, H), (0, 8)]),
                        op=OP.mult)
                    if full:
                        nc.vector.tensor_tensor(
                            out=reap(rx[:, 0:1, 64:65], [(192, 4), (16, H), (1, 16)]),
                            in0=reap(rsb[:, 0:1, W_VV:W_VV + 1], [(MW, 4), (16, H), (1, 16)]),
                            in1=reap(hh[:, 0:1, exoff:exoff + 1], [(HHW, 4), (1, H), (0, 16)]),
                            op=OP.mult)
                        nc.vector.tensor_tensor(
                            out=reap(rx[:, 0:1, 160:161], [(192, 4), (8, H), (1, 8)]),
                            in0=reap(rsb[:, 0:1, W_TT:W_TT + 1], [(MW, 4), (8, H), (1, 8)]),
                            in1=reap(hh[:, 0:1, exoff:exoff + 1], [(HHW, 4), (1, H), (0, 8)]),
                            op=OP.mult)
                    # m_svx = G_sv * rx_v ; m_stx = G_st * rx_t
                    msv = poolx.tile([P, 4, C1], BF16, tag="msv")
                    nc.vector.tensor_tensor(
                        out=reap(msv[:, 0:1, 0:1], [(C1, 4), (1, C1)]),
                        in0=reap(gt[:, 0:1, G_SV:G_SV + 1], [(GW, 4), (1, C1)]),
                        in1=reap(rx[:, 0:1, rxv_o:rxv_o + 1], [(192, 4), (1, C1)]),
                        op=OP.mult)
                    mst = poolx.tile([P, 4, C2], BF16, tag="mst")
                    nc.vector.tensor_tensor(
                        out=reap(mst[:, 0:1, 0:1], [(C2, 4), (1, C2)]),
                        in0=reap(gt[:, 0:1, G_ST:G_ST + 1], [(GW, 4), (1, C2)]),
                        in1=reap(rx[:, 0:1, rxt_o:rxt_o + 1], [(192, 4), (1, C2)]),
                        op=OP.mult)
                    if full:
                        # t2 terms -> hh, t1 terms -> scr2, then add
                        nc.vector.tensor_tensor(
                            out=reap(hh[:, 0:1, C0:C0 + 1], [(HHW, 4), (C1, 3), (1, C1)]),
                            in0=reap(gt[:, 0:1, G_PV:G_PV + 1], [(GW, 4), (C1, 3), (1, C1)]),
                            in1=reap(rx[:, 0:1, 64:65], [(192, 4), (0, 3), (1, C1)]),
                            op=OP.mult)
                        nc.vector.tensor_tensor(
                            out=reap(hh[:, 0:1, 320:321], [(HHW, 4), (C2, 5), (1, C2)]),
                            in0=reap(gt[:, 0:1, G_PT:G_PT + 1], [(GW, 4), (C2, 5), (1, C2)]),
                            in1=reap(rx[:, 0:1, 160:161], [(192, 4), (0, 5), (1, C2)]),
                            op=OP.mult)
                        scr2 = poolx.tile([P, 4, 352], BF16, tag="scr2")
                        nc.vector.tensor_tensor(
                            out=reap(scr2[:, 0:1, 0:1], [(352, 4), (C1, 3), (1, C1)]),
                            in0=reap(msv[:, 0:1, 0:1], [(C1, 4), (0, 3), (1, C1)]),
                            in1=reap(em_s[:, cs0:cs0 + 1, 2:3], [(EMW, 4), (1, 3), (0, C1)]),
                            op=OP.mult)
                        nc.vector.tensor_tensor(
                            out=reap(scr2[:, 0:1, 192:193], [(352, 4), (C2, 5), (1, C2)]),
                            in0=reap(mst[:, 0:1, 0:1], [(C2, 4), (0, 5), (1, C2)]),
                            in1=reap(em_s[:, cs0:cs0 + 1, 6:7], [(EMW, 4), (1, 5), (0, C2)]),
                            op=OP.mult)
                        nc.vector.tensor_tensor(
                            out=reap(hh[:, 0:1, C0:C0 + 1], [(HHW, 4), (1, 352)]),
                            in0=reap(hh[:, 0:1, C0:C0 + 1], [(HHW, 4), (1, 352)]),
                            in1=reap(scr2[:, 0:1, 0:1], [(352, 4), (1, 352)]),
                            op=OP.add)
                    else:
                        # layer 0: no t2 terms; t1 writes hh directly
                        nc.vector.tensor_tensor(
                            out=reap(hh[:, 0:1, C0:C0 + 1], [(HHW, 4), (C1, 3), (1, C1)]),
                            in0=reap(msv[:, 0:1, 0:1], [(C1, 4), (0, 3), (1, C1)]),
                            in1=reap(em_s[:, cs0:cs0 + 1, 2:3], [(EMW, 4), (1, 3), (0, C1)]),
                            op=OP.mult)
                        nc.vector.tensor_tensor(
                            out=reap(hh[:, 0:1, 320:321], [(HHW, 4), (C2, 5), (1, C2)]),
                            in0=reap(mst[:, 0:1, 0:1], [(C2, 4), (0, 5), (1, C2)]),
                            in1=reap(em_s[:, cs0:cs0 + 1, 6:7], [(EMW, 4), (1, 5), (0, C2)]),
                            op=OP.mult)
                # one-hot (dst within tile)
                ohb = poole.tile([P, 4, P], BF16, tag="ohb")
                nc.vector.tensor_tensor(
                    out=ohb[:],
                    in0=reap(nidx_t[:, 0:1], [(0, 4), (1, P)]),
                    in1=reap(em_s[:, cs0:cs0 + 1, 12:13], [(EMW, 4), (0, P)]),
                    op=OP.is_equal)
                # aggregation
                for s in range(4):
                    first = (k == 0 and s == 0)
                    last = (k == cpt - 1 and s == 3)
                    nc.tensor.matmul(agg[:, 0:hw], ohb[:, s, :], hh[:, s, 0:hw],
                                     start=first, stop=last, skip_group_check=True)
            return agg

        def transpose_to(src_ap, kparts, ffree):
            """transpose bf16 src [kparts, ffree] sbuf -> psum [ffree, kparts]"""
            ps = psA.tile([P, 512], F32, tag="mmA", space="PSUM")
            psb = ps[:, 0:P].bitcast(BF16)
            nc.tensor.transpose(psb[:ffree, :kparts], src_ap,
                                ident_t[:kparts, :kparts])
            return psb

        def update_tile(l, t, agg, lw):
            tsl = slice(t * P, (t + 1) * P)
            hw = hwid(l)
            last_v = l < LYR - 1
            rden = poolu.tile([P, H], F32, tag="rden")
            nc.vector.tensor_scalar(out=rden[:], in0=agg[:, hw - 4:hw],
                                    scalar1=1e-9, scalar2=None, op0=OP.add)
            nc.vector.reciprocal(out=rden[:], in_=rden[:])
            aggnm = poolu.tile([P, 480], BF16, tag="aggnm")
            for h in range(H):
                nc.vector.tensor_scalar(
                    out=aggnm[:, 32 * h:32 * h + 32], in0=agg[:, 32 * h:32 * h + 32],
                    scalar1=rden[:, h:h + 1], scalar2=None, op0=OP.mult)
                if last_v:
                    nc.vector.tensor_scalar(
                        out=reap(aggnm[:, 128 + 16 * h:128 + 16 * h + 1], [(64, 3), (1, 16)]),
                        in0=reap(agg[:, 128 + 16 * h:128 + 16 * h + 1], [(64, 3), (1, 16)]),
                        scalar1=rden[:, h:h + 1], scalar2=None, op0=OP.mult)
                    nc.vector.tensor_scalar(
                        out=reap(aggnm[:, 320 + 8 * h:320 + 8 * h + 1], [(32, 5), (1, 8)]),
                        in0=reap(agg[:, 320 + 8 * h:320 + 8 * h + 1], [(32, 5), (1, 8)]),
                        scalar1=rden[:, h:h + 1], scalar2=None, op0=OP.mult)

            # s out-projection + residual (feature-major)
            psS = transpose_to(aggnm[:, 0:128], P, P)
            afs = poolu.tile([P, P], BF16, tag="afs")
            nc.scalar.copy(out=afs[:], in_=psS[:, :P])
            pso = psA.tile([P, 512], F32, tag="mmA", space="PSUM")
            nc.tensor.matmul(pso[:, 0:P], lw["wo_s"][:], afs[:], start=True, stop=True)
            upd_s = poolu.tile([P, P], BF16, tag="upd_s")
            nc.vector.tensor_tensor(out=upd_s[:], in0=sfm[:, tsl], in1=pso[:, 0:P],
                                    op=OP.add)

            upd_v = poolu.tile([C1, 3, P], BF16, tag="upd_v")
            upd_t = poolu.tile([C2, 5, P], BF16, tag="upd_t")
            if last_v:
                for i in range(3):
                    psV = transpose_to(aggnm[:, 128 + 64 * i:128 + 64 * i + 64], P, C1)
                    afv = poolu.tile([C1, P], BF16, tag="afv")
                    nc.scalar.copy(out=afv[:], in_=psV[:C1, :P])
                    psv2 = psA.tile([P, 512], F32, tag="mmA", space="PSUM")
                    nc.tensor.matmul(psv2[:C1, 0:P], lw["wo_v"][:], afv[:],
                                     start=True, stop=True)
                    nc.vector.tensor_tensor(out=upd_v[:, i, :], in0=vfm_t[:, i, tsl],
                                            in1=psv2[:C1, 0:P], op=OP.add)
                for m in range(5):
                    psT_ = transpose_to(aggnm[:, 320 + 32 * m:320 + 32 * m + 32], P, C2)
                    aft = poolu.tile([C2, P], BF16, tag="aft")
                    nc.scalar.copy(out=aft[:], in_=psT_[:C2, :P])
                    pst2 = psA.tile([P, 512], F32, tag="mmA", space="PSUM")
                    nc.tensor.matmul(pst2[:C2, 0:P], lw["wo_t"][:], aft[:],
                                     start=True, stop=True)
                    nc.vector.tensor_tensor(out=upd_t[:, m, :], in0=tfm_t[:, m, tsl],
                                            in1=pst2[:C2, 0:P], op=OP.add)

            # transpose to node-major
            snm = poolu.tile([P, C0], BF16, tag="snm")
            psn = transpose_to(upd_s[:], P, P)
            nc.scalar.copy(out=snm[:], in_=psn[:, :P])
            vnm = poolu.tile([P, C1, 3], BF16, tag="vnm")
            tnm = poolu.tile([P, C2, 5], BF16, tag="tnm")
            if last_v:
                for i in range(3):
                    psn = transpose_to(upd_v[:, i, :], C1, P)
                    nc.vector.tensor_copy(
                        out=reap(vnm[:, 0:1, i:i + 1], [(3, C1)]), in_=psn[:, :C1])
                for m in range(5):
                    psn = transpose_to(upd_t[:, m, :], C2, P)
                    nc.vector.tensor_copy(
                        out=reap(tnm[:, 0:1, m:m + 1], [(5, C2)]), in_=psn[:, :C2])

            # LayerNorm on s
            stats = poolu.tile([P, 6], F32, tag="stats")
            nc.vector.bn_stats(out=stats[:], in_=snm[:])
            mv = poolu.tile([P, 2], F32, tag="mv")
            nc.vector.bn_aggr(out=mv[:], in_=stats[:])
            lnt = poolu.tile([P, 2], F32, tag="lnt")
            nc.scalar.activation(out=lnt[:, 0:1], in_=mv[:, 1:2], func=AF.Ln,
                                 bias=eps_t[:])
            nc.scalar.activation(out=lnt[:, 1:2], in_=lnt[:, 0:1], func=AF.Exp,
                                 scale=-0.5)
            nc.vector.tensor_scalar(out=snm[:], in0=snm[:], scalar1=mv[:, 0:1],
                                    scalar2=lnt[:, 1:2], op0=OP.subtract, op1=OP.mult)
            nc.vector.tensor_tensor(out=snm[:], in0=snm[:], in1=lw["lngs"][:], op=OP.mult)
            nc.vector.tensor_tensor(out=snm[:], in0=snm[:], in1=lw["lnbs"][:], op=OP.add)

            if last_v:
                vsq = poolu.tile([P, C1, 3], F32, tag="vsq")
                nc.vector.tensor_tensor(out=vsq[:], in0=vnm[:], in1=vnm[:], op=OP.mult)
                vr1 = poolu.tile([P, C1], F32, tag="vr1")
                nc.vector.tensor_reduce(out=vr1[:], in_=vsq[:],
                                        axis=mybir.AxisListType.X, op=OP.add)
                vr2 = poolu.tile([P, 1], F32, tag="vr2")
                nc.vector.tensor_reduce(out=vr2[:], in_=vr1[:],
                                        axis=mybir.AxisListType.X, op=OP.add)
                nc.scalar.activation(out=vr2[:], in_=vr2[:], func=AF.Ln,
                                     bias=eps_t[:], scale=1.0 / C1)
                nc.scalar.activation(out=vr2[:], in_=vr2[:], func=AF.Exp, scale=-0.5)
                nc.vector.tensor_scalar(out=vnm[:], in0=vnm[:], scalar1=vr2[:],
                                        scalar2=None, op0=OP.mult)
                nc.vector.tensor_tensor(
                    out=vnm[:], in0=vnm[:],
                    in1=reap(lw["lngv"][:, 0:1], [(1, C1), (0, 3)]), op=OP.mult)
                tsq = poolu.tile([P, C2, 5], F32, tag="tsq")
                nc.vector.tensor_tensor(out=tsq[:], in0=tnm[:], in1=tnm[:], op=OP.mult)
                tr1 = poolu.tile([P, C2], F32, tag="tr1")
                nc.vector.tensor_reduce(out=tr1[:], in_=tsq[:],
                                        axis=mybir.AxisListType.X, op=OP.add)
                tr2 = poolu.tile([P, 1], F32, tag="tr2")
                nc.vector.tensor_reduce(out=tr2[:], in_=tr1[:],
                                        axis=mybir.AxisListType.X, op=OP.add)
                nc.scalar.activation(out=tr2[:], in_=tr2[:], func=AF.Ln,
                                     bias=eps_t[:], scale=1.0 / C2)
                nc.scalar.activation(out=tr2[:], in_=tr2[:], func=AF.Exp, scale=-0.5)
                nc.vector.tensor_scalar(out=tnm[:], in0=tnm[:], scalar1=tr2[:],
                                        scalar2=None, op0=OP.mult)
                nc.vector.tensor_tensor(
                    out=tnm[:], in0=tnm[:],
                    in1=reap(lw["lngt"][:, 0:1], [(1, C2), (0, 5)]), op=OP.mult)

            # write back feature-major state
            psn = transpose_to(snm[:], P, P)
            nc.scalar.copy(out=sfm[:, tsl], in_=psn[:, :P])
            if last_v:
                for i in range(3):
                    psn = transpose_to(reap(vnm[:, 0:1, i:i + 1], [(3, C1)]), P, C1)
                    nc.scalar.copy(out=vfm_t[:, i, tsl], in_=psn[:C1, :P])
                for m in range(5):
                    psn = transpose_to(reap(tnm[:, 0:1, m:m + 1], [(5, C2)]), P, C2)
                    nc.scalar.copy(out=tfm_t[:, m, tsl], in_=psn[:C2, :P])

            if last_v:
                # next-layer table rows, node-major (stationary = fm state)
                gwn = gwid(l + 1)
                ntrow = poolu.tile([P, GW], BF16, tag="ntrow")
                if l < LYR - 2:
                    ntA = psRp.tile([P, 512], F32, tag="rp", space="PSUM")
                    nc.tensor.matmul(ntA[:, 0:GW0], sfm[:, tsl], lw["wspack"][:],
                                     start=True, stop=True)
                    nc.scalar.copy(out=ntrow[:, 0:GW0], in_=ntA[:, 0:GW0])
                    ntB = psRp.tile([P, 512], F32, tag="rp", space="PSUM")
                    for i in range(3):
                        nc.tensor.matmul(ntB[:, 64 * i:64 * i + 64],
                                         vfm_t[:, i, tsl], lw["wv_v"][:],
                                         start=True, stop=True, skip_group_check=True)
                    for m in range(5):
                        nc.tensor.matmul(ntB[:, 192 + 32 * m:192 + 32 * m + 32],
                                         tfm_t[:, m, tsl], lw["wt_t"][:],
                                         start=True, stop=True, skip_group_check=True)
                    nc.scalar.copy(out=ntrow[:, GW0:GW], in_=ntB[:, 0:352])
                    nc.sync.dma_start(out=own[l][tsl, :], in_=ntrow[:, 0:GW])
                else:
                    # l == 4: only P_s for layer 5
                    ntA = psRp.tile([P, 512], F32, tag="rp", space="PSUM")
                    nc.tensor.matmul(ntA[:, 0:C0], sfm[:, tsl],
                                     lw["wspack"][:, 0:C0], start=True, stop=True)
                    nc.scalar.copy(out=ntrow[:, 0:C0], in_=ntA[:, 0:C0])
                    nc.sync.dma_start(out=own[l][tsl, :], in_=ntrow[:, 0:GW5])
            else:
                # final readout head
                feat = poolu.tile([P, 4, P], F32, tag="feat")
                for b in range(4):
                    ps = psA.tile([P, 512], F32, tag="mmA", space="PSUM")
                    nc.tensor.matmul(ps[:, 0:P], wfeat_t[:, b * 128:(b + 1) * 128],
                                     sfm[:, tsl], start=True, stop=True)
                    nc.scalar.activation(out=feat[:, b, :], in_=ps[:, 0:P],
                                         func=AF.Gelu_apprx_tanh,
                                         bias=bfeatp_t[:, b:b + 1])
                pseT = psRp.tile([P, 512], F32, tag="rp", space="PSUM")
                pse = pseT[0:1, 0:P]
                for b in range(4):
                    nc.tensor.matmul(pse, wout1p_t[:, b:b + 1], feat[:, b, :],
                                     start=(b == 0), stop=(b == 3))
                ne = poolu.tile([1, P], F32, tag="ne")
                nc.vector.tensor_copy(out=ne[:], in_=pse)
                nc.sync.dma_start(out=t_nodee[tsl], in_=ne[0:1, :])

        for l in range(LYR):
            gsrc = t_ntab0 if l == 0 else ntab[l - 1]
            lw = dict(
                wrad1=loadL(t_wrad1, l, NB, RAD, "wrad1"),
                wrad2=loadL(t_wrad2, l, RAD, RAD, "wrad2"),
                brad1=loadL(t_brad1, l, RAD, 1, "brad1", F32),
                brad2=loadL(t_brad2, l, RAD, 1, "brad2", F32),
                wwall=loadL(t_wwall, l, RAD, MW, "wwall"),
                wo_s=loadL(t_wo_s, l, C0, C0, "wo_s"),
                lngs=loadL(t_lngs, l, P, C0, "lngs"),
                lnbs=loadL(t_lnbs, l, P, C0, "lnbs"),
            )
            if l < LYR - 1:
                lw["wo_v"] = loadL(t_wo_v, l, C1, C1, "wo_v")
                lw["wo_t"] = loadL(t_wo_t, l, C2, C2, "wo_t")
                lw["lngv"] = loadL(t_lngv, l, P, C1, "lngv")
                lw["lngt"] = loadL(t_lngt, l, P, C2, "lngt")
                lw["wspack"] = loadL(t_wspack, l + 1, C0, GW0, "wspack")
                lw["wv_v"] = loadL(t_wv_v, l + 1, C1, C1, "wv_v")
                lw["wt_t"] = loadL(t_wt_t, l + 1, C2, C2, "wt_t")
            for t in range(ntile):
                em_s = poolT.tile([P, cpt * 4, EMW], BF16, tag="em_s")
                nc.sync.dma_start(out=em_s[:],
                                  in_=t_em[:, t * cpt * 4:(t + 1) * cpt * 4, :])
                idx_s = poolT.tile([P, cpt * 4], I32, tag="idx_s")
                nc.sync.dma_start(out=idx_s[:],
                                  in_=t_idxw[:, t * cpt * 4:(t + 1) * cpt * 4])
                agg = edge_tile(l, t, gsrc, lw, em_s, idx_s)
                update_tile(l, t, agg, lw)
            if l < LYR - 1:
                nc.gpsimd.collective_compute(
                    "AllGather", OP.bypass,
                    replica_groups=[list(range(cfg.ncore))],
                    ins=[own[l][:]], outs=[ntab[l][:]])

    nc.compile()
    return nc


# ---------------- entry point ----------------

def _ensure_profile_hook():
    try:
        import antenv  # noqa
        import antenv.axon_hooks  # noqa
        return
    except Exception:
        pass
    try:
        import antenv
        from trn_agent_boot.trn_boot import _ntff_profile_via_ctypes
        hook = _ntff_profile_via_ctypes("/opt/axon/libaxon_pjrt.so")
        mod = types.ModuleType("antenv.axon_hooks")
        mod.get_axon_ntff_profile_hook = lambda: hook
        mod.set_axon_ntff_profile_hook = lambda h: None
        sys.modules["antenv.axon_hooks"] = mod
        antenv.axon_hooks = mod
    except Exception:
        pass


_PROGRAM_CACHE = {}


def run_cfg(inp, cfg: Cfg, trace=False):
    in_maps, gid = host_preprocess(inp, cfg)
    key = (cfg.ncore, cfg.npc, cfg.cpt, cfg.layers)
    if key not in _PROGRAM_CACHE:
        _PROGRAM_CACHE[key] = build_program(cfg)
    nc = _PROGRAM_CACHE[key]
    if trace:
        _ensure_profile_hook()
    res = run_bass_kernel_spmd(nc, in_maps, list(range(cfg.ncore)), trace=trace)
    node_e_pad = np.concatenate(
        [np.asarray(res.results[c]["node_e"]) for c in range(cfg.ncore)])
    node_e = node_e_pad[gid]          # invert node permutation
    return node_e, res


def kernel(**inputs):
    cfg = Cfg()
    node_e, _ = run_cfg(inputs, cfg)
    node_e = node_e[:, None] + np.asarray(inputs["b_out1"], np.float32)[None, :]
    batch = np.asarray(inputs["batch"]).astype(np.int64)
    graph = np.zeros((G, 1), np.float32)
    np.add.at(graph, batch, node_e)
    out = graph @ np.asarray(inputs["W_read"], np.float32) + np.asarray(
        inputs["b_read"], np.float32)
    return out.astype(np.float32)


# revision 6
# speedup vs baseline: 2.8863x; 1.4436x over previous
"""Trainium2 Bass kernel for the Equiformer-style GNN regressor (v2, bf16).

Strategy (8 NeuronCores, SPMD, data-parallel over nodes/edges):
  - Nodes are greedily permuted into 80 (core,tile) bins of 128 nodes each,
    balancing incoming-edge counts so every tile needs <= cpt*512 edge slots
    (cpt=8 with balancing vs 9 without).
  - Edges live in the slot array of the tile owning their dst node
    (partition = slot%128, sub-column = slot//128).
  - Per layer a DRAM node table holds per-node projected quantities in bf16:
    [P_s 128 | P_sv 64 | P_st 32 | Pv 192 (64i+c) | Pt 160 (32m+c)].
    Layer 0 uses a narrow 224-col table (v=t=0), layer 5 a 128-col table.
  - Edge phase per 512-edge chunk: gather 4x128 source rows (indirect DMA),
    radial weights w via PE (rbf -> 2-layer silu MLP), then the radial
    projections are computed EDGE-major in one matmul per 128-edge sub
    (stationary = w-slice [64,128], moving = packed wwall [64,448] holding
    [R_sa|R_s|R_v|R_vv|R_t|R_tt]).  Messages are formed with ~14 chunk-wide
    bf16 DVE ops into a contiguous hh layout [s 128 | v 192 | t 160 | ex 4]
    and aggregated per dst tile with ONE one-hot matmul per sub into PSUM.
  - Update phase: attention-denominator normalize, out-projections +
    residual + equivariant norms (bf16 PE transposes/matmuls), then the
    next layer's node-table rows are produced NODE-major directly
    (stationary = feature-major state slice) and AllGathered.
  - Final readout (per-node energies) is DMA'd out; the per-graph
    segment-sum + Linear readout runs on host.
"""
import sys
import types
from contextlib import ExitStack
from dataclasses import dataclass

import numpy as np
import ml_dtypes

import concourse.bacc as bacc
import concourse.bass as bass
import concourse.tile as tile
from concourse import mybir
from concourse.bass_utils import run_bass_kernel_spmd

F32 = mybir.dt.float32
BF16 = mybir.dt.bfloat16
I32 = mybir.dt.int32
BF = ml_dtypes.bfloat16
AF = mybir.ActivationFunctionType
OP = mybir.AluOpType

# ---------------- problem constants (hardcoded per spec) ----------------
N, E, G, L = 10000, 320000, 32, 6
C0, C1, C2, H, NB, RAD, FD, T = 128, 64, 32, 4, 128, 64, 512, 1
MAXR = 5.0
EPS = 1e-6
NCORE = 8
P = 128

GW = 576          # full node-table row width (bf16 elements)
GW0 = 224         # layer-0 row width
GW5 = 128         # layer-5 row width
MW = 448          # wwall width (R_sa|R_s|R_v|R_vv|R_t|R_tt)
HHW = 484         # hh width (s 128 | v 192 | t 160 | ex 4)
HHW5 = 132
EMW = 16          # em cols: [mask@0, sh1@2:5, sh2@6:11, dst@12] (4B-aligned starts)

# G row block offsets
G_SV, G_ST, G_PV, G_PT = 128, 192, 224, 416
# wwall block offsets
W_SA, W_S, W_V, W_VV, W_T, W_TT = 0, 128, 256, 320, 384, 416
# layer-0 wwall: [R_sa | R_s | R_v | R_t]
W0_V, W0_T = 256, 320
MW0 = 352
MW5 = 256


@dataclass
class Cfg:
    ncore: int = NCORE
    npc: int = 1280          # padded nodes per core (multiple of 128)
    cpt: int = 8             # chunks (512 slots) per node-tile
    layers: int = L
    nn: int = N              # real node count

    @property
    def ntile(self):
        return self.npc // P

    @property
    def np_total(self):
        return self.npc * self.ncore

    @property
    def slots(self):
        return self.ntile * self.cpt * 512


# ---------------- host-side packing ----------------

def balance_nodes(edge_dst, nbins, cap):
    """Greedy: sort nodes by in-degree desc, place into least-loaded
    non-full bin. Returns gid[node] = padded global id."""
    deg = np.bincount(edge_dst, minlength=N)
    order = np.argsort(-deg, kind="stable")
    load = np.zeros(nbins, np.int64)
    fill = np.zeros(nbins, np.int64)
    gid = np.zeros(N, np.int64)
    # heap-free greedy: argmin over non-full bins (nbins=80, N=10k -> fine)
    open_bins = np.arange(nbins)
    for n in order:
        b_i = np.argmin(load[open_bins])
        b = open_bins[b_i]
        gid[n] = b * P + fill[b]
        load[b] += deg[n]
        fill[b] += 1
        if fill[b] == cap:
            open_bins = open_bins[open_bins != b]
    return gid, load


def host_preprocess(inp, cfg: Cfg):
    npc, ncore, ntile = cfg.npc, cfg.ncore, cfg.ntile
    nbins = ncore * ntile

    pos = np.asarray(inp["pos"], np.float32)
    node_atom = np.asarray(inp["node_atom"]).astype(np.int64)
    esrc = np.asarray(inp["edge_src"]).astype(np.int64)
    edst = np.asarray(inp["edge_dst"]).astype(np.int64)

    gid, load = balance_nodes(edst, nbins, P)
    cpt_need = int(np.ceil(load.max() / 512))
    assert cpt_need <= cfg.cpt, f"need cpt {cpt_need} > cfg {cfg.cpt}"

    src_p = gid[esrc]
    dst_p = gid[edst]

    # geometry (f32, match reference formulas)
    rel = pos[edst] - pos[esrc]
    d2 = (rel * rel).sum(-1) + np.float32(EPS)
    d = np.sqrt(d2)
    u = rel / d[:, None]
    s3, s5, s15 = [np.float32(np.sqrt(x)) for x in (3.0, 5.0, 15.0)]
    sh1 = s3 * u
    x_, y_, z_ = u[:, 0], u[:, 1], u[:, 2]
    sh2 = np.stack(
        [s15 * x_ * y_, s15 * y_ * z_, np.float32(0.5) * s5 * (3 * z_ * z_ - 1.0),
         s15 * x_ * z_, np.float32(0.5) * s15 * (x_ * x_ - y_ * y_)], -1)

    tile_of_edge = dst_p // P              # global bin id
    order = np.argsort(tile_of_edge, kind="stable")
    counts = np.bincount(tile_of_edge, minlength=nbins)
    starts = np.zeros(nbins + 1, np.int64)
    np.cumsum(counts, out=starts[1:])

    S = cfg.slots
    nsub = S // P
    per_core = []
    for c in range(ncore):
        em = np.zeros((P, nsub, EMW), np.float32)
        em[:, :, 12] = -1.0
        geom = np.ones((3, S), np.float32)
        idx_em = np.zeros((P, nsub), np.int32)
        for t in range(ntile):
            gt_ = c * ntile + t
            eids = order[starts[gt_]:starts[gt_ + 1]]
            base = t * cfg.cpt * 512
            k = len(eids)
            j = base + np.arange(k)
            pp, ss = j % P, j // P
            em[pp, ss, 0] = 1.0
            em[pp, ss, 2:5] = sh1[eids]
            em[pp, ss, 6:11] = sh2[eids]
            em[pp, ss, 12] = (dst_p[eids] - (c * npc + t * P)).astype(np.float32)
            geom[0, j] = d2[eids]
            geom[1, j] = d[eids]
            idx_em[pp, ss] = src_p[eids].astype(np.int32)
        per_core.append(dict(em=em.astype(BF), geom=geom, idxw=idx_em))

    # ---- weights ----
    wd = {}
    centers = np.linspace(0.0, MAXR, NB).astype(np.float32)
    width = np.float32(MAXR / NB)
    wq = np.zeros((3, NB), np.float32)
    wq[0] = -0.5 / width**2
    wq[1] = centers / width**2
    wq[2] = -0.5 * centers**2 / width**2
    wd["wq"] = wq
    wd["wrad1"] = np.asarray(inp["Wrad1"], np.float32).astype(BF)
    wd["wrad2"] = np.asarray(inp["Wrad2"], np.float32).astype(BF)
    wd["brad1"] = np.asarray(inp["brad1"], np.float32).reshape(L, RAD, 1)
    wd["brad2"] = np.asarray(inp["brad2"], np.float32).reshape(L, RAD, 1)

    wwall = np.zeros((L, RAD, MW), np.float32)
    attn_a = np.asarray(inp["attn_a"], np.float32)
    for l in range(L):
        av = attn_a[l].reshape(C0)          # a[h(c), j(c)], c = h*32+j
        wwall[l, :, W_SA:W_SA + C0] = np.asarray(inp["Ww_s"][l]) * av[None, :]
        wwall[l, :, W_S:W_S + C0] = inp["Ww_s"][l]
        if l < L - 1:
            if l == 0:
                wwall[l, :, W0_V:W0_V + C1] = inp["Ww_v"][l]
                wwall[l, :, W0_T:W0_T + C2] = inp["Ww_t"][l]
            else:
                wwall[l, :, W_V:W_V + C1] = inp["Ww_v"][l]
                wwall[l, :, W_VV:W_VV + C1] = inp["Ww_vv"][l]
                wwall[l, :, W_T:W_T + C2] = inp["Ww_t"][l]
                wwall[l, :, W_TT:W_TT + C2] = inp["Ww_tt"][l]
    wd["wwall"] = wwall.astype(BF)

    wd["wo_s"] = np.asarray(inp["Wo_s"], np.float32).astype(BF)
    wd["wo_v"] = np.asarray(inp["Wo_v"], np.float32).astype(BF)
    wd["wo_t"] = np.asarray(inp["Wo_t"], np.float32).astype(BF)
    # packed next-layer s-projections [C0, 224] = [Ws_src | Ws_v | Ws_t]
    wsp = np.zeros((L, C0, GW0), np.float32)
    for l in range(L):
        wsp[l, :, 0:C0] = inp["Ws_src"][l]
        wsp[l, :, C0:C0 + C1] = inp["Ws_v"][l]
        wsp[l, :, C0 + C1:GW0] = inp["Ws_t"][l]
    wd["wspack"] = wsp.astype(BF)
    wd["wv_v"] = np.asarray(inp["Wv_v"], np.float32).astype(BF)
    wd["wt_t"] = np.asarray(inp["Wt_t"], np.float32).astype(BF)
    rep = lambda a: np.broadcast_to(a[:, None, :], (a.shape[0], P, a.shape[1])).copy()
    wd["lngs"] = rep(np.asarray(inp["g_s"], np.float32)).astype(BF)
    wd["lnbs"] = rep(np.asarray(inp["b_s"], np.float32)).astype(BF)
    wd["lngv"] = rep(np.asarray(inp["g_v"], np.float32)).astype(BF)
    wd["lngt"] = rep(np.asarray(inp["g_t"], np.float32)).astype(BF)
    wd["wfeat"] = np.asarray(inp["W_feat"], np.float32).astype(BF)
    wd["bfeatp"] = np.asarray(inp["b_feat"], np.float32).reshape(4, 128).T.copy()
    wd["wout1p"] = np.asarray(inp["W_out1"], np.float32).reshape(4, 128).T.copy()
    nidx = np.tile(np.arange(P, dtype=np.float32), (P, 1))
    wd["nidxb"] = nidx.astype(BF)
    wd["identb"] = np.eye(P, dtype=np.float32).astype(BF)

    # ---- initial node table (layer 0 projections, s only) ----
    s0 = np.asarray(inp["atom_emb"], np.float32)[node_atom]     # [N, C0]
    s0p = np.zeros((cfg.np_total, C0), np.float32)
    s0p[gid] = s0
    nt0 = np.zeros((cfg.np_total, GW0), np.float32)
    nt0[:, 0:C0] = s0p @ np.asarray(inp["Ws_src"][0], np.float32)
    nt0[:, C0:C0 + C1] = s0p @ np.asarray(inp["Ws_v"][0], np.float32)
    nt0[:, C0 + C1:GW0] = s0p @ np.asarray(inp["Ws_t"][0], np.float32)

    # feature-major s0 per core (bf16)
    in_maps = []
    for c in range(ncore):
        m = dict(per_core[c])
        m["ntab0"] = nt0.astype(BF)
        m["s0fm"] = s0p[c * npc:(c + 1) * npc].T.copy().astype(BF)
        for k, v in wd.items():
            m[k] = v
        in_maps.append(m)
    return in_maps, gid


# ---------------- device program ----------------

def reap(sliced: bass.AP, dims) -> bass.AP:
    """Rebuild free-dims of a sliced AP with explicit [step, count] pairs."""
    return bass.AP(sliced.tensor, sliced.offset,
                   [list(sliced.ap[0])] + [[int(s), int(c)] for s, c in dims])


def build_program(cfg: Cfg):
    nc = bacc.Bacc("TRN2", target_bir_lowering=False, debug=False,
                   enable_asserts=True, num_devices=cfg.ncore)
    npc, ntile, cpt = cfg.npc, cfg.ntile, cfg.cpt
    S = cfg.slots
    nsub = S // P
    NPT = cfg.np_total
    LYR = cfg.layers

    dp = nc.declare_dram_parameter
    t_ntab0 = dp("ntab0", [NPT, GW0], BF16, isOutput=False)
    t_s0fm = dp("s0fm", [C0, npc], BF16, isOutput=False)
    t_em = dp("em", [P, nsub, EMW], BF16, isOutput=False)
    t_geom = dp("geom", [3, S], F32, isOutput=False)
    t_idxw = dp("idxw", [P, nsub], I32, isOutput=False)
    t_wq = dp("wq", [3, NB], F32, isOutput=False)
    t_wrad1 = dp("wrad1", [LYR, NB, RAD], BF16, isOutput=False)
    t_wrad2 = dp("wrad2", [LYR, RAD, RAD], BF16, isOutput=False)
    t_brad1 = dp("brad1", [LYR, RAD, 1], F32, isOutput=False)
    t_brad2 = dp("brad2", [LYR, RAD, 1], F32, isOutput=False)
    t_wwall = dp("wwall", [LYR, RAD, MW], BF16, isOutput=False)
    t_wo_s = dp("wo_s", [LYR, C0, C0], BF16, isOutput=False)
    t_wo_v = dp("wo_v", [LYR, C1, C1], BF16, isOutput=False)
    t_wo_t = dp("wo_t", [LYR, C2, C2], BF16, isOutput=False)
    t_wspack = dp("wspack", [LYR, C0, GW0], BF16, isOutput=False)
    t_wv_v = dp("wv_v", [LYR, C1, C1], BF16, isOutput=False)
    t_wt_t = dp("wt_t", [LYR, C2, C2], BF16, isOutput=False)
    t_lngs = dp("lngs", [LYR, P, C0], BF16, isOutput=False)
    t_lnbs = dp("lnbs", [LYR, P, C0], BF16, isOutput=False)
    t_lngv = dp("lngv", [LYR, P, C1], BF16, isOutput=False)
    t_lngt = dp("lngt", [LYR, P, C2], BF16, isOutput=False)
    t_wfeat = dp("wfeat", [C0, FD], BF16, isOutput=False)
    t_bfeatp = dp("bfeatp", [P, 4], F32, isOutput=False)
    t_wout1p = dp("wout1p", [P, 4], F32, isOutput=False)
    t_nidxb = dp("nidxb", [P, P], BF16, isOutput=False)
    t_identb = dp("identb", [P, P], BF16, isOutput=False)
    t_nodee = dp("node_e", [npc], F32, isOutput=True)

    own = [nc.dram_tensor(f"own{l}", [npc, GW if l < LYR - 2 else GW5], BF16)
           for l in range(LYR - 1)]
    ntab = [nc.dram_tensor(f"ntab{l + 1}", [NPT, GW if l < LYR - 2 else GW5],
                           BF16, addr_space="Shared")
            for l in range(LYR - 1)]

    def gwid(l):
        return GW0 if l == 0 else (GW5 if l == LYR - 1 else GW)

    def mwid(l):
        return MW0 if l == 0 else (MW5 if l == LYR - 1 else MW)

    def hwid(l):
        return HHW5 if l == LYR - 1 else HHW

    with tile.TileContext(nc) as tc, ExitStack() as ctx:
        pool1 = ctx.enter_context(tc.tile_pool(name="const", bufs=1))
        poolL = ctx.enter_context(tc.tile_pool(name="layerw", bufs=1))
        poolT = ctx.enter_context(tc.tile_pool(name="tilec", bufs=2))
        poolg = ctx.enter_context(tc.tile_pool(name="gath", bufs=3))
        poole = ctx.enter_context(tc.tile_pool(name="edge", bufs=3))
        poolx = ctx.enter_context(tc.tile_pool(name="edge1", bufs=1))
        poolr = ctx.enter_context(tc.tile_pool(name="rad", bufs=2))
        poolu = ctx.enter_context(tc.tile_pool(name="upd", bufs=1))
        psA = ctx.enter_context(tc.tile_pool(name="psA", bufs=3, space="PSUM"))
        psRp = ctx.enter_context(tc.tile_pool(name="psRp", bufs=1, space="PSUM"))
        psAgg = ctx.enter_context(tc.tile_pool(name="psAgg", bufs=1, space="PSUM"))

        def load1(dram, shape, dtype=F32):
            t = pool1.tile(shape, dtype, tag=dram.name)
            nc.sync.dma_start(out=t[:], in_=dram[:])
            return t

        wq_t = load1(t_wq, [3, NB])
        nidx_t = load1(t_nidxb, [P, P], BF16)
        ident_t = load1(t_identb, [P, P], BF16)
        wfeat_t = load1(t_wfeat, [C0, FD], BF16)
        bfeatp_t = load1(t_bfeatp, [P, 4])
        wout1p_t = load1(t_wout1p, [P, 4])

        eps_t = pool1.tile([P, 1], F32, tag="epsT")
        nc.vector.memset(eps_t[:], EPS)

        # feature-major state (bf16)
        sfm = pool1.tile([C0, npc], BF16, tag="sfm")
        nc.sync.dma_start(out=sfm[:], in_=t_s0fm[:])
        vfm_t = pool1.tile([C1, 3, npc], BF16, tag="vfm")
        nc.vector.memset(vfm_t[:], 0.0)
        tfm_t = pool1.tile([C2, 5, npc], BF16, tag="tfm")
        nc.vector.memset(tfm_t[:], 0.0)

        def loadL(dram, l, p, f, tag, dtype=BF16):
            t = poolL.tile([p, f], dtype, tag=tag)
            nc.sync.dma_start(out=t[:], in_=dram[l])
            return t

        rbf_ring = {}

        def radial_ph1(ti):
            """PH1 (layer-independent rbf) for tile slot ti."""
            rbf_b = poolr.tile([NB, cpt * 512], BF16, tag="rbfb")
            for k in range(cpt):
                gsl = poolT.tile([3, 512], F32, tag="geom_c")
                nc.sync.dma_start(
                    out=gsl[:],
                    in_=t_geom[:, (ti * cpt + k) * 512:(ti * cpt + k + 1) * 512])
                ps = psA.tile([NB, 512], F32, tag="mmA", space="PSUM")
                nc.tensor.matmul(ps[:], wq_t[:], gsl[:], start=True, stop=True)
                nc.scalar.activation(out=rbf_b[:, k * 512:(k + 1) * 512],
                                     in_=ps[:], func=AF.Exp)
            rbf_ring[ti % 2] = rbf_b

        def edge_tile(l, t, gsrc, lw, em_s, idx_s, ph1_next):
            """Edge phase for node-tile t of layer l -> returns agg psum.
            Emission is software-pipelined: stage_a(k+1) (gather+proj+evac)
            is emitted before stage_b(k) (DVE messages + agg matmuls) so the
            PE never head-of-line blocks behind the DVE chain."""
            gw, mw, hw = gwid(l), mwid(l), hwid(l)
            full = 0 < l < LYR - 1
            exoff = hw - 4
            rbf_b = rbf_ring[t % 2]
            w_b = poolr.tile([RAD, cpt * 512], BF16, tag="wb")
            for k in range(cpt):
                sl = slice(k * 512, (k + 1) * 512)
                ps = psA.tile([RAD, 512], F32, tag="mmA", space="PSUM")
                nc.tensor.matmul(ps[:RAD, :], lw["wrad1"][:], rbf_b[:, sl],
                                 start=True, stop=True)
                h1 = poolx.tile([RAD, 512], BF16, tag="h1")
                nc.scalar.activation(out=h1[:], in_=ps[:RAD, :], func=AF.Silu,
                                     bias=lw["brad1"][:])
                ps2 = psA.tile([RAD, 512], F32, tag="mmA", space="PSUM")
                nc.tensor.matmul(ps2[:RAD, :], lw["wrad2"][:], h1[:],
                                 start=True, stop=True)
                nc.scalar.activation(out=w_b[:, sl], in_=ps2[:RAD, :], func=AF.Silu,
                                     bias=lw["brad2"][:])
            agg = psAgg.tile([P, HHW], F32, tag="agg", space="PSUM")
            live = {}

            def stage_a(k):
                cs0 = k * 4
                gt = poolg.tile([P, 4, GW], BF16, tag="gt")
                for s in range(4):
                    nc.gpsimd.indirect_dma_start(
                        out=gt[:, s, 0:gw], out_offset=None, in_=gsrc[:, :],
                        in_offset=bass.IndirectOffsetOnAxis(
                            ap=idx_s[:, cs0 + s:cs0 + s + 1], axis=0))
                psp = psRp.tile([P, 4, 512], F32, tag="rp", space="PSUM")
                for s in range(4):
                    nc.tensor.matmul(
                        psp[:, s, 0:mw],
                        w_b[:, k * 512 + s * 128:k * 512 + (s + 1) * 128],
                        lw["wwall"][:, 0:mw], start=True, stop=True,
                        skip_group_check=True)
                rsb = poole.tile([P, 4, MW], BF16, tag="rsb")
                nc.scalar.copy(out=rsb[:, :, 0:mw],
                               in_=reap(psp[:, 0:1, 0:1], [(512, 4), (1, mw)]))
                live[k] = (gt, rsb)

            def stage_b(k):
                cs0 = k * 4
                gt, rsb = live.pop(k)
                hh = poole.tile([P, 4, HHW], BF16, tag="hh")
                # scr = G_s * R_sa ; logits ; ex -> hh ex cols (unmasked; pad
                # edges are killed by the one-hot dst=-1 row instead)
                scrt = poolx.tile([P, 4, C0], BF16, tag="scrt")
                nc.vector.tensor_tensor(
                    out=scrt[:], in0=reap(gt[:, 0:1, 0:1], [(GW, 4), (1, C0)]),
                    in1=reap(rsb[:, 0:1, W_SA:W_SA + 1], [(MW, 4), (1, C0)]),
                    op=OP.mult)
                lgt = poolx.tile([P, 4, H], F32, tag="lgt")
                nc.vector.tensor_reduce(
                    out=lgt[:],
                    in_=scrt[:].rearrange("p s (h c) -> p s h c", h=H),
                    axis=mybir.AxisListType.X, op=OP.add)
                nc.scalar.activation(
                    out=reap(hh[:, 0:1, exoff:exoff + 1], [(HHW, 4), (1, H)]),
                    in_=lgt[:], func=AF.Exp)
                # gxs = G_s * ex ; hh_s = gxs * R_s
                gxs = poolx.tile([P, 4, C0], BF16, tag="gxs")
                nc.vector.tensor_tensor(
                    out=reap(gxs[:, 0:1, 0:1], [(C0, 4), (32, H), (1, 32)]),
                    in0=reap(gt[:, 0:1, 0:1], [(GW, 4), (32, H), (1, 32)]),
                    in1=reap(hh[:, 0:1, exoff:exoff + 1], [(HHW, 4), (1, H), (0, 32)]),
                    op=OP.mult)
                nc.vector.tensor_tensor(
                    out=reap(hh[:, 0:1, 0:1], [(HHW, 4), (1, C0)]),
                    in0=reap(gxs[:, 0:1, 0:1], [(C0, 4), (1, C0)]),
                    in1=reap(rsb[:, 0:1, W_S:W_S + 1], [(MW, 4), (1, C0)]),
                    op=OP.mult)
                if l < LYR - 1:
                    wv_off = W0_V if l == 0 else W_V
                    wt_off = W0_T if l == 0 else W_T
                    rx = poolx.tile([P, 4, 192], BF16, tag="rx")
                    rxv_o, rxt_o = 0, 128 if full else 64
                    nc.vector.tensor_tensor(
                        out=reap(rx[:, 0:1, rxv_o:rxv_o + 1], [(192, 4), (16, H), (1, 16)]),
                        in0=reap(rsb[:, 0:1, wv_off:wv_off + 1], [(MW, 4), (16, H), (1, 16)]),
                        in1=reap(hh[:, 0:1, exoff:exoff + 1], [(HHW, 4), (1, H), (0, 16)]),
                        op=OP.mult)
                    nc.vector.tensor_tensor(
                        out=reap(rx[:, 0:1, rxt_o:rxt_o + 1], [(192, 4), (8, H), (1, 8)]),
                        in0=reap(rsb[:, 0:1, wt_off:wt_off + 1], [(MW, 4), (8, H), (1, 8)]),
                        in1=reap(hh[:, 0:1, exoff:exoff + 1], [(HHW, 4), (1, H), (0, 8)]),
                        op=OP.mult)
                    if full:
                        nc.vector.tensor_tensor(
                            out=reap(rx[:, 0:1, 64:65], [(192, 4), (16, H), (1, 16)]),
                            in0=reap(rsb[:, 0:1, W_VV:W_VV + 1], [(MW, 4), (16, H), (1, 16)]),
                            in1=reap(hh[:, 0:1, exoff:exoff + 1], [(HHW, 4), (1, H), (0, 16)]),
                            op=OP.mult)
                        nc.vector.tensor_tensor(
                            out=reap(rx[:, 0:1, 160:161], [(192, 4), (8, H), (1, 8)]),
                            in0=reap(rsb[:, 0:1, W_TT:W_TT + 1], [(MW, 4), (8, H), (1, 8)]),
                            in1=reap(hh[:, 0:1, exoff:exoff + 1], [(HHW, 4), (1, H), (0, 8)]),
                            op=OP.mult)
                    msv = poolx.tile([P, 4, C1], BF16, tag="msv")
                    nc.vector.tensor_tensor(
                        out=reap(msv[:, 0:1, 0:1], [(C1, 4), (1, C1)]),
                        in0=reap(gt[:, 0:1, G_SV:G_SV + 1], [(GW, 4), (1, C1)]),
                        in1=reap(rx[:, 0:1, rxv_o:rxv_o + 1], [(192, 4), (1, C1)]),
                        op=OP.mult)
                    mst = poolx.tile([P, 4, C2], BF16, tag="mst")
                    nc.vector.tensor_tensor(
                        out=reap(mst[:, 0:1, 0:1], [(C2, 4), (1, C2)]),
                        in0=reap(gt[:, 0:1, G_ST:G_ST + 1], [(GW, 4), (1, C2)]),
                        in1=reap(rx[:, 0:1, rxt_o:rxt_o + 1], [(192, 4), (1, C2)]),
                        op=OP.mult)
                    if full:
                        nc.vector.tensor_tensor(
                            out=reap(hh[:, 0:1, C0:C0 + 1], [(HHW, 4), (C1, 3), (1, C1)]),
                            in0=reap(gt[:, 0:1, G_PV:G_PV + 1], [(GW, 4), (C1, 3), (1, C1)]),
                            in1=reap(rx[:, 0:1, 64:65], [(192, 4), (0, 3), (1, C1)]),
                            op=OP.mult)
                        nc.vector.tensor_tensor(
                            out=reap(hh[:, 0:1, 320:321], [(HHW, 4), (C2, 5), (1, C2)]),
                            in0=reap(gt[:, 0:1, G_PT:G_PT + 1], [(GW, 4), (C2, 5), (1, C2)]),
                            in1=reap(rx[:, 0:1, 160:161], [(192, 4), (0, 5), (1, C2)]),
                            op=OP.mult)
                        scr2 = poolx.tile([P, 4, 352], BF16, tag="scr2")
                        nc.vector.tensor_tensor(
                            out=reap(scr2[:, 0:1, 0:1], [(352, 4), (C1, 3), (1, C1)]),
                            in0=reap(msv[:, 0:1, 0:1], [(C1, 4), (0, 3), (1, C1)]),
                            in1=reap(em_s[:, cs0:cs0 + 1, 2:3], [(EMW, 4), (1, 3), (0, C1)]),
                            op=OP.mult)
                        nc.vector.tensor_tensor(
                            out=reap(scr2[:, 0:1, 192:193], [(352, 4), (C2, 5), (1, C2)]),
                            in0=reap(mst[:, 0:1, 0:1], [(C2, 4), (0, 5), (1, C2)]),
                            in1=reap(em_s[:, cs0:cs0 + 1, 6:7], [(EMW, 4), (1, 5), (0, C2)]),
                            op=OP.mult)
                        nc.vector.tensor_tensor(
                            out=reap(hh[:, 0:1, C0:C0 + 1], [(HHW, 4), (1, 352)]),
                            in0=reap(hh[:, 0:1, C0:C0 + 1], [(HHW, 4), (1, 352)]),
                            in1=reap(scr2[:, 0:1, 0:1], [(352, 4), (1, 352)]),
                            op=OP.add)
                    else:
                        nc.vector.tensor_tensor(
                            out=reap(hh[:, 0:1, C0:C0 + 1], [(HHW, 4), (C1, 3), (1, C1)]),
                            in0=reap(msv[:, 0:1, 0:1], [(C1, 4), (0, 3), (1, C1)]),
                            in1=reap(em_s[:, cs0:cs0 + 1, 2:3], [(EMW, 4), (1, 3), (0, C1)]),
                            op=OP.mult)
                        nc.vector.tensor_tensor(
                            out=reap(hh[:, 0:1, 320:321], [(HHW, 4), (C2, 5), (1, C2)]),
                            in0=reap(mst[:, 0:1, 0:1], [(C2, 4), (0, 5), (1, C2)]),
                            in1=reap(em_s[:, cs0:cs0 + 1, 6:7], [(EMW, 4), (1, 5), (0, C2)]),
                            op=OP.mult)
                ohb = poole.tile([P, 4, P], BF16, tag="ohb")
                nc.vector.tensor_tensor(
                    out=ohb[:],
                    in0=reap(nidx_t[:, 0:1], [(0, 4), (1, P)]),
                    in1=reap(em_s[:, cs0:cs0 + 1, 12:13], [(EMW, 4), (0, P)]),
                    op=OP.is_equal)
                for s in range(4):
                    first = (k == 0 and s == 0)
                    last = (k == cpt - 1 and s == 3)
                    nc.tensor.matmul(agg[:, 0:hw], ohb[:, s, :], hh[:, s, 0:hw],
                                     start=first, stop=last, skip_group_check=True)

            stage_a(0)
            for k in range(cpt):
                if k + 1 < cpt:
                    stage_a(k + 1)
                elif ph1_next is not None:
                    ph1_next()
                stage_b(k)
            return agg

        def transpose_to(src_ap, kparts, ffree):
            """transpose bf16 src [kparts, ffree] sbuf -> psum [ffree, kparts]"""
            ps = psA.tile([P, 512], F32, tag="mmA", space="PSUM")
            psb = ps[:, 0:P].bitcast(BF16)
            nc.tensor.transpose(psb[:ffree, :kparts], src_ap,
                                ident_t[:kparts, :kparts])
            return psb

        def update_tile(l, t, agg, lw):
            tsl = slice(t * P, (t + 1) * P)
            hw = hwid(l)
            last_v = l < LYR - 1
            rden = poolu.tile([P, H], F32, tag="rden")
            nc.vector.tensor_scalar(out=rden[:], in0=agg[:, hw - 4:hw],
                                    scalar1=1e-9, scalar2=None, op0=OP.add)
            nc.vector.reciprocal(out=rden[:], in_=rden[:])
            aggnm = poolu.tile([P, 480], BF16, tag="aggnm")
            nc.vector.tensor_tensor(
                out=reap(aggnm[:, 0:1], [(32, H), (1, 32)]),
                in0=reap(agg[:, 0:1], [(32, H), (1, 32)]),
                in1=reap(rden[:, 0:1], [(1, H), (0, 32)]), op=OP.mult)
            if last_v:
                nc.vector.tensor_tensor(
                    out=reap(aggnm[:, 128:129], [(64, 3), (16, H), (1, 16)]),
                    in0=reap(agg[:, 128:129], [(64, 3), (16, H), (1, 16)]),
                    in1=reap(rden[:, 0:1], [(0, 3), (1, H), (0, 16)]), op=OP.mult)
                nc.vector.tensor_tensor(
                    out=reap(aggnm[:, 320:321], [(32, 5), (8, H), (1, 8)]),
                    in0=reap(agg[:, 320:321], [(32, 5), (8, H), (1, 8)]),
                    in1=reap(rden[:, 0:1], [(0, 5), (1, H), (0, 8)]), op=OP.mult)

            # s out-projection + residual (feature-major)
            psS = transpose_to(aggnm[:, 0:128], P, P)
            afs = poolu.tile([P, P], BF16, tag="afs")
            nc.scalar.copy(out=afs[:], in_=psS[:, :P])
            pso = psA.tile([P, 512], F32, tag="mmA", space="PSUM")
            nc.tensor.matmul(pso[:, 0:P], lw["wo_s"][:], afs[:], start=True, stop=True)
            upd_s = poolu.tile([P, P], BF16, tag="upd_s")
            nc.vector.tensor_tensor(out=upd_s[:], in0=sfm[:, tsl], in1=pso[:, 0:P],
                                    op=OP.add)

            upd_v = poolu.tile([C1, 3, P], BF16, tag="upd_v")
            upd_t = poolu.tile([C2, 5, P], BF16, tag="upd_t")
            if last_v:
                for i in range(3):
                    psV = transpose_to(aggnm[:, 128 + 64 * i:128 + 64 * i + 64], P, C1)
                    afv = poolu.tile([C1, P], BF16, tag="afv")
                    nc.scalar.copy(out=afv[:], in_=psV[:C1, :P])
                    psv2 = psA.tile([P, 512], F32, tag="mmA", space="PSUM")
                    nc.tensor.matmul(psv2[:C1, 0:P], lw["wo_v"][:], afv[:],
                                     start=True, stop=True)
                    nc.vector.tensor_tensor(out=upd_v[:, i, :], in0=vfm_t[:, i, tsl],
                                            in1=psv2[:C1, 0:P], op=OP.add)
                for m in range(5):
                    psT_ = transpose_to(aggnm[:, 320 + 32 * m:320 + 32 * m + 32], P, C2)
                    aft = poolu.tile([C2, P], BF16, tag="aft")
                    nc.scalar.copy(out=aft[:], in_=psT_[:C2, :P])
                    pst2 = psA.tile([P, 512], F32, tag="mmA", space="PSUM")
                    nc.tensor.matmul(pst2[:C2, 0:P], lw["wo_t"][:], aft[:],
                                     start=True, stop=True)
                    nc.vector.tensor_tensor(out=upd_t[:, m, :], in0=tfm_t[:, m, tsl],
                                            in1=pst2[:C2, 0:P], op=OP.add)

            # transpose to node-major
            snm = poolu.tile([P, C0], BF16, tag="snm")
            psn = transpose_to(upd_s[:], P, P)
            nc.scalar.copy(out=snm[:], in_=psn[:, :P])
            vnm = poolu.tile([P, C1, 3], BF16, tag="vnm")
            tnm = poolu.tile([P, C2, 5], BF16, tag="tnm")
            if last_v:
                for i in range(3):
                    psn = transpose_to(upd_v[:, i, :], C1, P)
                    nc.vector.tensor_copy(
                        out=reap(vnm[:, 0:1, i:i + 1], [(3, C1)]), in_=psn[:, :C1])
                for m in range(5):
                    psn = transpose_to(upd_t[:, m, :], C2, P)
                    nc.vector.tensor_copy(
                        out=reap(tnm[:, 0:1, m:m + 1], [(5, C2)]), in_=psn[:, :C2])

            # LayerNorm on s
            stats = poolu.tile([P, 6], F32, tag="stats")
            nc.vector.bn_stats(out=stats[:], in_=snm[:])
            mv = poolu.tile([P, 2], F32, tag="mv")
            nc.vector.bn_aggr(out=mv[:], in_=stats[:])
            lnt = poolu.tile([P, 2], F32, tag="lnt")
            nc.scalar.activation(out=lnt[:, 0:1], in_=mv[:, 1:2], func=AF.Ln,
                                 bias=eps_t[:])
            nc.scalar.activation(out=lnt[:, 1:2], in_=lnt[:, 0:1], func=AF.Exp,
                                 scale=-0.5)
            nc.vector.tensor_scalar(out=snm[:], in0=snm[:], scalar1=mv[:, 0:1],
                                    scalar2=lnt[:, 1:2], op0=OP.subtract, op1=OP.mult)
            nc.vector.tensor_tensor(out=snm[:], in0=snm[:], in1=lw["lngs"][:], op=OP.mult)
            nc.vector.tensor_tensor(out=snm[:], in0=snm[:], in1=lw["lnbs"][:], op=OP.add)

            if last_v:
                vsq = poolu.tile([P, C1, 3], F32, tag="vsq")
                nc.vector.tensor_tensor(out=vsq[:], in0=vnm[:], in1=vnm[:], op=OP.mult)
                vr1 = poolu.tile([P, C1], F32, tag="vr1")
                nc.vector.tensor_reduce(out=vr1[:], in_=vsq[:],
                                        axis=mybir.AxisListType.X, op=OP.add)
                vr2 = poolu.tile([P, 1], F32, tag="vr2")
                nc.vector.tensor_reduce(out=vr2[:], in_=vr1[:],
                                        axis=mybir.AxisListType.X, op=OP.add)
                nc.scalar.activation(out=vr2[:], in_=vr2[:], func=AF.Ln,
                                     bias=eps_t[:], scale=1.0 / C1)
                nc.scalar.activation(out=vr2[:], in_=vr2[:], func=AF.Exp, scale=-0.5)
                nc.vector.tensor_scalar(out=vnm[:], in0=vnm[:], scalar1=vr2[:],
                                        scalar2=None, op0=OP.mult)
                nc.vector.tensor_tensor(
                    out=vnm[:], in0=vnm[:],
                    in1=reap(lw["lngv"][:, 0:1], [(1, C1), (0, 3)]), op=OP.mult)
                tsq = poolu.tile([P, C2, 5], F32, tag="tsq")
                nc.vector.tensor_tensor(out=tsq[:], in0=tnm[:], in1=tnm[:], op=OP.mult)
                tr1 = poolu.tile([P, C2], F32, tag="tr1")
                nc.vector.tensor_reduce(out=tr1[:], in_=tsq[:],
                                        axis=mybir.AxisListType.X, op=OP.add)
                tr2 = poolu.tile([P, 1], F32, tag="tr2")
                nc.vector.tensor_reduce(out=tr2[:], in_=tr1[:],
                                        axis=mybir.AxisListType.X, op=OP.add)
                nc.scalar.activation(out=tr2[:], in_=tr2[:], func=AF.Ln,
                                     bias=eps_t[:], scale=1.0 / C2)
                nc.scalar.activation(out=tr2[:], in_=tr2[:], func=AF.Exp, scale=-0.5)
                nc.vector.tensor_scalar(out=tnm[:], in0=tnm[:], scalar1=tr2[:],
                                        scalar2=None, op0=OP.mult)
                nc.vector.tensor_tensor(
                    out=tnm[:], in0=tnm[:],
                    in1=reap(lw["lngt"][:, 0:1], [(1, C2), (0, 5)]), op=OP.mult)

            # write back feature-major state
            psn = transpose_to(snm[:], P, P)
            nc.scalar.copy(out=sfm[:, tsl], in_=psn[:, :P])
            if last_v:
                for i in range(3):
                    psn = transpose_to(reap(vnm[:, 0:1, i:i + 1], [(3, C1)]), P, C1)
                    nc.scalar.copy(out=vfm_t[:, i, tsl], in_=psn[:C1, :P])
                for m in range(5):
                    psn = transpose_to(reap(tnm[:, 0:1, m:m + 1], [(5, C2)]), P, C2)
                    nc.scalar.copy(out=tfm_t[:, m, tsl], in_=psn[:C2, :P])

            if last_v:
                # next-layer table rows, node-major (stationary = fm state)
                gwn = gwid(l + 1)
                ntrow = poolu.tile([P, GW], BF16, tag="ntrow")
                if l < LYR - 2:
                    ntA = psRp.tile([P, 512], F32, tag="rp", space="PSUM")
                    nc.tensor.matmul(ntA[:, 0:GW0], sfm[:, tsl], lw["wspack"][:],
                                     start=True, stop=True)
                    nc.scalar.copy(out=ntrow[:, 0:GW0], in_=ntA[:, 0:GW0])
                    ntB = psRp.tile([P, 512], F32, tag="rp", space="PSUM")
                    for i in range(3):
                        nc.tensor.matmul(ntB[:, 64 * i:64 * i + 64],
                                         vfm_t[:, i, tsl], lw["wv_v"][:],
                                         start=True, stop=True, skip_group_check=True)
                    for m in range(5):
                        nc.tensor.matmul(ntB[:, 192 + 32 * m:192 + 32 * m + 32],
                                         tfm_t[:, m, tsl], lw["wt_t"][:],
                                         start=True, stop=True, skip_group_check=True)
                    nc.scalar.copy(out=ntrow[:, GW0:GW], in_=ntB[:, 0:352])
                    nc.sync.dma_start(out=own[l][tsl, :], in_=ntrow[:, 0:GW])
                else:
                    # l == 4: only P_s for layer 5
                    ntA = psRp.tile([P, 512], F32, tag="rp", space="PSUM")
                    nc.tensor.matmul(ntA[:, 0:C0], sfm[:, tsl],
                                     lw["wspack"][:, 0:C0], start=True, stop=True)
                    nc.scalar.copy(out=ntrow[:, 0:C0], in_=ntA[:, 0:C0])
                    nc.sync.dma_start(out=own[l][tsl, :], in_=ntrow[:, 0:GW5])
            else:
                # final readout head
                feat = poolu.tile([P, 4, P], F32, tag="feat")
                for b in range(4):
                    ps = psA.tile([P, 512], F32, tag="mmA", space="PSUM")
                    nc.tensor.matmul(ps[:, 0:P], wfeat_t[:, b * 128:(b + 1) * 128],
                                     sfm[:, tsl], start=True, stop=True)
                    nc.scalar.activation(out=feat[:, b, :], in_=ps[:, 0:P],
                                         func=AF.Gelu_apprx_tanh,
                                         bias=bfeatp_t[:, b:b + 1])
                pseT = psRp.tile([P, 512], F32, tag="rp", space="PSUM")
                pse = pseT[0:1, 0:P]
                for b in range(4):
                    nc.tensor.matmul(pse, wout1p_t[:, b:b + 1], feat[:, b, :],
                                     start=(b == 0), stop=(b == 3))
                ne = poolu.tile([1, P], F32, tag="ne")
                nc.vector.tensor_copy(out=ne[:], in_=pse)
                nc.sync.dma_start(out=t_nodee[tsl], in_=ne[0:1, :])

        for l in range(LYR):
            gsrc = t_ntab0 if l == 0 else ntab[l - 1]
            lw = dict(
                wrad1=loadL(t_wrad1, l, NB, RAD, "wrad1"),
                wrad2=loadL(t_wrad2, l, RAD, RAD, "wrad2"),
                brad1=loadL(t_brad1, l, RAD, 1, "brad1", F32),
                brad2=loadL(t_brad2, l, RAD, 1, "brad2", F32),
                wwall=loadL(t_wwall, l, RAD, MW, "wwall"),
                wo_s=loadL(t_wo_s, l, C0, C0, "wo_s"),
                lngs=loadL(t_lngs, l, P, C0, "lngs"),
                lnbs=loadL(t_lnbs, l, P, C0, "lnbs"),
            )
            if l < LYR - 1:
                lw["wo_v"] = loadL(t_wo_v, l, C1, C1, "wo_v")
                lw["wo_t"] = loadL(t_wo_t, l, C2, C2, "wo_t")
                lw["lngv"] = loadL(t_lngv, l, P, C1, "lngv")
                lw["lngt"] = loadL(t_lngt, l, P, C2, "lngt")
                lw["wspack"] = loadL(t_wspack, l + 1, C0, GW0, "wspack")
                lw["wv_v"] = loadL(t_wv_v, l + 1, C1, C1, "wv_v")
                lw["wt_t"] = loadL(t_wt_t, l + 1, C2, C2, "wt_t")
            for t in range(ntile):
                if l == 0 and t == 0:
                    radial_ph1(0)
                em_s = poolT.tile([P, cpt * 4, EMW], BF16, tag="em_s")
                nc.sync.dma_start(out=em_s[:],
                                  in_=t_em[:, t * cpt * 4:(t + 1) * cpt * 4, :])
                idx_s = poolT.tile([P, cpt * 4], I32, tag="idx_s")
                nc.sync.dma_start(out=idx_s[:],
                                  in_=t_idxw[:, t * cpt * 4:(t + 1) * cpt * 4])
                nt_t = t + 1 if t + 1 < ntile else 0
                more = (t + 1 < ntile) or (l + 1 < LYR)
                ph1_next = (lambda tt=nt_t: radial_ph1(tt)) if more else None
                agg = edge_tile(l, t, gsrc, lw, em_s, idx_s, ph1_next)
                update_tile(l, t, agg, lw)
            if l < LYR - 1:
                nc.gpsimd.collective_compute(
                    "AllGather", OP.bypass,
                    replica_groups=[list(range(cfg.ncore))],
                    ins=[own[l][:]], outs=[ntab[l][:]])

    nc.compile()
    return nc


# ---------------- entry point ----------------

def _ensure_profile_hook():
    try:
        import antenv  # noqa
        import antenv.axon_hooks  # noqa
        return
    except Exception:
        pass
    try:
        import antenv
        from trn_agent_boot.trn_boot import _ntff_profile_via_ctypes
        hook = _ntff_profile_via_ctypes("/opt/axon/libaxon_pjrt.so")
        mod = types.ModuleType("antenv.axon_hooks")
        mod.get_axon_ntff_profile_hook = lambda: hook
        mod.set_axon_ntff_profile_hook = lambda h: None
        sys.modules["antenv.axon_hooks"] = mod
        antenv.axon_hooks = mod
    except Exception:
        pass


_PROGRAM_CACHE = {}


def run_cfg(inp, cfg: Cfg, trace=False):
    in_maps, gid = host_preprocess(inp, cfg)
    key = (cfg.ncore, cfg.npc, cfg.cpt, cfg.layers)
    if key not in _PROGRAM_CACHE:
        _PROGRAM_CACHE[key] = build_program(cfg)
    nc = _PROGRAM_CACHE[key]
    if trace:
        _ensure_profile_hook()
    res = run_bass_kernel_spmd(nc, in_maps, list(range(cfg.ncore)), trace=trace)
    node_e_pad = np.concatenate(
        [np.asarray(res.results[c]["node_e"]) for c in range(cfg.ncore)])
    node_e = node_e_pad[gid]          # invert node permutation
    return node_e, res


def kernel(**inputs):
    cfg = Cfg()
    node_e, _ = run_cfg(inputs, cfg)
    node_e = node_e[:, None] + np.asarray(inputs["b_out1"], np.float32)[None, :]
    batch = np.asarray(inputs["batch"]).astype(np.int64)
    graph = np.zeros((G, 1), np.float32)
    np.add.at(graph, batch, node_e)
    out = graph @ np.asarray(inputs["W_read"], np.float32) + np.asarray(
        inputs["b_read"], np.float32)
    return out.astype(np.float32)


# revision 9
# speedup vs baseline: 3.0526x; 1.0576x over previous
"""Trainium2 Bass kernel for the Equiformer-style GNN regressor (v2, bf16).

Strategy (8 NeuronCores, SPMD, data-parallel over nodes/edges):
  - Nodes are greedily permuted into 80 (core,tile) bins of 128 nodes each,
    balancing incoming-edge counts so every tile needs <= cpt*512 edge slots
    (cpt=8 with balancing vs 9 without).
  - Edges live in the slot array of the tile owning their dst node
    (partition = slot%128, sub-column = slot//128).
  - Per layer a DRAM node table holds per-node projected quantities in bf16:
    [P_s 128 | P_sv 64 | P_st 32 | Pv 192 (64i+c) | Pt 160 (32m+c)].
    Layer 0 uses a narrow 224-col table (v=t=0), layer 5 a 128-col table.
  - Edge phase per 512-edge chunk: gather 4x128 source rows (indirect DMA),
    radial weights w via PE (rbf -> 2-layer silu MLP), then the radial
    projections are computed EDGE-major in one matmul per 128-edge sub
    (stationary = w-slice [64,128], moving = packed wwall [64,448] holding
    [R_sa|R_s|R_v|R_vv|R_t|R_tt]).  Messages are formed with ~14 chunk-wide
    bf16 DVE ops into a contiguous hh layout [s 128 | v 192 | t 160 | ex 4]
    and aggregated per dst tile with ONE one-hot matmul per sub into PSUM.
  - Update phase: attention-denominator normalize, out-projections +
    residual + equivariant norms (bf16 PE transposes/matmuls), then the
    next layer's node-table rows are produced NODE-major directly
    (stationary = feature-major state slice) and AllGathered.
  - Final readout (per-node energies) is DMA'd out; the per-graph
    segment-sum + Linear readout runs on host.
"""
import sys
import types
from contextlib import ExitStack
from dataclasses import dataclass

import numpy as np
import ml_dtypes

import concourse.bacc as bacc
import concourse.bass as bass
import concourse.tile as tile
from concourse import mybir
from concourse.bass_utils import run_bass_kernel_spmd

F32 = mybir.dt.float32
BF16 = mybir.dt.bfloat16
I32 = mybir.dt.int32
BF = ml_dtypes.bfloat16
AF = mybir.ActivationFunctionType
OP = mybir.AluOpType

# ---------------- problem constants (hardcoded per spec) ----------------
N, E, G, L = 10000, 320000, 32, 6
C0, C1, C2, H, NB, RAD, FD, T = 128, 64, 32, 4, 128, 64, 512, 1
MAXR = 5.0
EPS = 1e-6
NCORE = 8
P = 128

GW = 576          # full node-table row width (bf16 elements)
GW0 = 224         # layer-0 row width
GW5 = 128         # layer-5 row width
MW = 448          # wwall width (R_sa|R_s|R_v|R_vv|R_t|R_tt)
HHW = 484         # hh width (s 128 | v 192 | t 160 | ex 4)
HHW5 = 132
EMW = 16          # em cols: [mask@0, sh1@2:5, sh2@6:11, dst@12] (4B-aligned starts)

# G row block offsets
G_SV, G_ST, G_PV, G_PT = 128, 192, 224, 416
# wwall block offsets
W_SA, W_S, W_V, W_VV, W_T, W_TT = 0, 128, 256, 320, 384, 416
# layer-0 wwall: [R_sa | R_s | R_v | R_t]
W0_V, W0_T = 256, 320
MW0 = 352
MW5 = 256


@dataclass
class Cfg:
    ncore: int = NCORE
    npc: int = 1280          # padded nodes per core (multiple of 128)
    cpt: int = 8             # chunks (512 slots) per node-tile
    layers: int = L
    nn: int = N              # real node count

    @property
    def ntile(self):
        return self.npc // P

    @property
    def np_total(self):
        return self.npc * self.ncore

    @property
    def slots(self):
        return self.ntile * self.cpt * 512


# ---------------- host-side packing ----------------

def balance_nodes(edge_dst, nbins, cap):
    """Greedy: sort nodes by in-degree desc, place into least-loaded
    non-full bin. Returns gid[node] = padded global id."""
    deg = np.bincount(edge_dst, minlength=N)
    order = np.argsort(-deg, kind="stable")
    load = np.zeros(nbins, np.int64)
    fill = np.zeros(nbins, np.int64)
    gid = np.zeros(N, np.int64)
    # heap-free greedy: argmin over non-full bins (nbins=80, N=10k -> fine)
    open_bins = np.arange(nbins)
    for n in order:
        b_i = np.argmin(load[open_bins])
        b = open_bins[b_i]
        gid[n] = b * P + fill[b]
        load[b] += deg[n]
        fill[b] += 1
        if fill[b] == cap:
            open_bins = open_bins[open_bins != b]
    return gid, load


def host_preprocess(inp, cfg: Cfg):
    npc, ncore, ntile = cfg.npc, cfg.ncore, cfg.ntile
    nbins = ncore * ntile

    pos = np.asarray(inp["pos"], np.float32)
    node_atom = np.asarray(inp["node_atom"]).astype(np.int64)
    esrc = np.asarray(inp["edge_src"]).astype(np.int64)
    edst = np.asarray(inp["edge_dst"]).astype(np.int64)

    gid, load = balance_nodes(edst, nbins, P)
    cpt_need = int(np.ceil(load.max() / 512))
    assert cpt_need <= cfg.cpt, f"need cpt {cpt_need} > cfg {cfg.cpt}"

    src_p = gid[esrc]
    dst_p = gid[edst]

    # geometry (f32, match reference formulas)
    rel = pos[edst] - pos[esrc]
    d2 = (rel * rel).sum(-1) + np.float32(EPS)
    d = np.sqrt(d2)
    u = rel / d[:, None]
    s3, s5, s15 = [np.float32(np.sqrt(x)) for x in (3.0, 5.0, 15.0)]
    sh1 = s3 * u
    x_, y_, z_ = u[:, 0], u[:, 1], u[:, 2]
    sh2 = np.stack(
        [s15 * x_ * y_, s15 * y_ * z_, np.float32(0.5) * s5 * (3 * z_ * z_ - 1.0),
         s15 * x_ * z_, np.float32(0.5) * s15 * (x_ * x_ - y_ * y_)], -1)

    tile_of_edge = dst_p // P              # global bin id
    order = np.argsort(tile_of_edge, kind="stable")
    counts = np.bincount(tile_of_edge, minlength=nbins)
    starts = np.zeros(nbins + 1, np.int64)
    np.cumsum(counts, out=starts[1:])

    S = cfg.slots
    nsub = S // P
    per_core = []
    for c in range(ncore):
        em = np.zeros((P, nsub, EMW), np.float32)
        em[:, :, 12] = -1.0
        geom = np.ones((3, S), np.float32)
        idx_em = np.zeros((P, nsub), np.int32)
        for t in range(ntile):
            gt_ = c * ntile + t
            eids = order[starts[gt_]:starts[gt_ + 1]]
            base = t * cfg.cpt * 512
            k = len(eids)
            j = base + np.arange(k)
            pp, ss = j % P, j // P
            em[pp, ss, 0] = 1.0
            em[pp, ss, 2:5] = sh1[eids]
            em[pp, ss, 6:11] = sh2[eids]
            em[pp, ss, 12] = (dst_p[eids] - (c * npc + t * P)).astype(np.float32)
            geom[0, j] = d2[eids]
            geom[1, j] = d[eids]
            idx_em[pp, ss] = src_p[eids].astype(np.int32)
        per_core.append(dict(em=em.astype(BF), geom=geom, idxw=idx_em))

    # ---- weights ----
    wd = {}
    centers = np.linspace(0.0, MAXR, NB).astype(np.float32)
    width = np.float32(MAXR / NB)
    wq = np.zeros((3, NB), np.float32)
    wq[0] = -0.5 / width**2
    wq[1] = centers / width**2
    wq[2] = -0.5 * centers**2 / width**2
    wd["wq"] = wq
    wd["wrad1"] = np.asarray(inp["Wrad1"], np.float32).astype(BF)
    wd["wrad2"] = np.asarray(inp["Wrad2"], np.float32).astype(BF)
    wd["brad1"] = np.asarray(inp["brad1"], np.float32).reshape(L, RAD, 1)
    wd["brad2"] = np.asarray(inp["brad2"], np.float32).reshape(L, RAD, 1)

    wwall = np.zeros((L, RAD, MW), np.float32)
    attn_a = np.asarray(inp["attn_a"], np.float32)
    for l in range(L):
        av = attn_a[l].reshape(C0)          # a[h(c), j(c)], c = h*32+j
        wwall[l, :, W_SA:W_SA + C0] = np.asarray(inp["Ww_s"][l]) * av[None, :]
        wwall[l, :, W_S:W_S + C0] = inp["Ww_s"][l]
        if l < L - 1:
            if l == 0:
                wwall[l, :, W0_V:W0_V + C1] = inp["Ww_v"][l]
                wwall[l, :, W0_T:W0_T + C2] = inp["Ww_t"][l]
            else:
                wwall[l, :, W_V:W_V + C1] = inp["Ww_v"][l]
                wwall[l, :, W_VV:W_VV + C1] = inp["Ww_vv"][l]
                wwall[l, :, W_T:W_T + C2] = inp["Ww_t"][l]
                wwall[l, :, W_TT:W_TT + C2] = inp["Ww_tt"][l]
    wd["wwall"] = wwall.astype(BF)

    wd["wo_s"] = np.asarray(inp["Wo_s"], np.float32).astype(BF)
    wd["wo_v"] = np.asarray(inp["Wo_v"], np.float32).astype(BF)
    wd["wo_t"] = np.asarray(inp["Wo_t"], np.float32).astype(BF)
    # packed next-layer s-projections [C0, 224] = [Ws_src | Ws_v | Ws_t]
    wsp = np.zeros((L, C0, GW0), np.float32)
    for l in range(L):
        wsp[l, :, 0:C0] = inp["Ws_src"][l]
        wsp[l, :, C0:C0 + C1] = inp["Ws_v"][l]
        wsp[l, :, C0 + C1:GW0] = inp["Ws_t"][l]
    wd["wspack"] = wsp.astype(BF)
    wd["wv_v"] = np.asarray(inp["Wv_v"], np.float32).astype(BF)
    wd["wt_t"] = np.asarray(inp["Wt_t"], np.float32).astype(BF)
    rep = lambda a: np.broadcast_to(a[:, None, :], (a.shape[0], P, a.shape[1])).copy()
    wd["lngs"] = rep(np.asarray(inp["g_s"], np.float32)).astype(BF)
    wd["lnbs"] = rep(np.asarray(inp["b_s"], np.float32)).astype(BF)
    wd["lngv"] = rep(np.asarray(inp["g_v"], np.float32)).astype(BF)
    wd["lngt"] = rep(np.asarray(inp["g_t"], np.float32)).astype(BF)
    wd["wfeat"] = np.asarray(inp["W_feat"], np.float32).astype(BF)
    wd["bfeatp"] = np.asarray(inp["b_feat"], np.float32).reshape(4, 128).T.copy()
    wd["wout1p"] = np.asarray(inp["W_out1"], np.float32).reshape(4, 128).T.copy()
    nidx = np.tile(np.arange(P, dtype=np.float32), (P, 1))
    wd["nidxb"] = nidx.astype(BF)
    wd["identb"] = np.eye(P, dtype=np.float32).astype(BF)

    # ---- initial node table (layer 0 projections, s only) ----
    s0 = np.asarray(inp["atom_emb"], np.float32)[node_atom]     # [N, C0]
    s0p = np.zeros((cfg.np_total, C0), np.float32)
    s0p[gid] = s0
    nt0 = np.zeros((cfg.np_total, GW0), np.float32)
    nt0[:, 0:C0] = s0p @ np.asarray(inp["Ws_src"][0], np.float32)
    nt0[:, C0:C0 + C1] = s0p @ np.asarray(inp["Ws_v"][0], np.float32)
    nt0[:, C0 + C1:GW0] = s0p @ np.asarray(inp["Ws_t"][0], np.float32)

    # feature-major s0 per core (bf16)
    in_maps = []
    for c in range(ncore):
        m = dict(per_core[c])
        m["ntab0"] = nt0.astype(BF)
        m["s0fm"] = s0p[c * npc:(c + 1) * npc].T.copy().astype(BF)
        for k, v in wd.items():
            m[k] = v
        in_maps.append(m)
    return in_maps, gid


# ---------------- device program ----------------

def reap(sliced: bass.AP, dims) -> bass.AP:
    """Rebuild free-dims of a sliced AP with explicit [step, count] pairs."""
    return bass.AP(sliced.tensor, sliced.offset,
                   [list(sliced.ap[0])] + [[int(s), int(c)] for s, c in dims])


def build_program(cfg: Cfg):
    nc = bacc.Bacc("TRN2", target_bir_lowering=False, debug=False,
                   enable_asserts=True, num_devices=cfg.ncore)
    npc, ntile, cpt = cfg.npc, cfg.ntile, cfg.cpt
    S = cfg.slots
    nsub = S // P
    NPT = cfg.np_total
    LYR = cfg.layers

    dp = nc.declare_dram_parameter
    t_ntab0 = dp("ntab0", [NPT, GW0], BF16, isOutput=False)
    t_s0fm = dp("s0fm", [C0, npc], BF16, isOutput=False)
    t_em = dp("em", [P, nsub, EMW], BF16, isOutput=False)
    t_geom = dp("geom", [3, S], F32, isOutput=False)
    t_idxw = dp("idxw", [P, nsub], I32, isOutput=False)
    t_wq = dp("wq", [3, NB], F32, isOutput=False)
    t_wrad1 = dp("wrad1", [LYR, NB, RAD], BF16, isOutput=False)
    t_wrad2 = dp("wrad2", [LYR, RAD, RAD], BF16, isOutput=False)
    t_brad1 = dp("brad1", [LYR, RAD, 1], F32, isOutput=False)
    t_brad2 = dp("brad2", [LYR, RAD, 1], F32, isOutput=False)
    t_wwall = dp("wwall", [LYR, RAD, MW], BF16, isOutput=False)
    t_wo_s = dp("wo_s", [LYR, C0, C0], BF16, isOutput=False)
    t_wo_v = dp("wo_v", [LYR, C1, C1], BF16, isOutput=False)
    t_wo_t = dp("wo_t", [LYR, C2, C2], BF16, isOutput=False)
    t_wspack = dp("wspack", [LYR, C0, GW0], BF16, isOutput=False)
    t_wv_v = dp("wv_v", [LYR, C1, C1], BF16, isOutput=False)
    t_wt_t = dp("wt_t", [LYR, C2, C2], BF16, isOutput=False)
    t_lngs = dp("lngs", [LYR, P, C0], BF16, isOutput=False)
    t_lnbs = dp("lnbs", [LYR, P, C0], BF16, isOutput=False)
    t_lngv = dp("lngv", [LYR, P, C1], BF16, isOutput=False)
    t_lngt = dp("lngt", [LYR, P, C2], BF16, isOutput=False)
    t_wfeat = dp("wfeat", [C0, FD], BF16, isOutput=False)
    t_bfeatp = dp("bfeatp", [P, 4], F32, isOutput=False)
    t_wout1p = dp("wout1p", [P, 4], F32, isOutput=False)
    t_nidxb = dp("nidxb", [P, P], BF16, isOutput=False)
    t_identb = dp("identb", [P, P], BF16, isOutput=False)
    t_nodee = dp("node_e", [npc], F32, isOutput=True)

    own = [nc.dram_tensor(f"own{l}", [npc, GW if l < LYR - 2 else GW5], BF16)
           for l in range(LYR - 1)]
    ntab = [nc.dram_tensor(f"ntab{l + 1}", [NPT, GW if l < LYR - 2 else GW5],
                           BF16, addr_space="Shared")
            for l in range(LYR - 1)]

    def gwid(l):
        return GW0 if l == 0 else (GW5 if l == LYR - 1 else GW)

    def mwid(l):
        return MW0 if l == 0 else (MW5 if l == LYR - 1 else MW)

    def hwid(l):
        return HHW5 if l == LYR - 1 else HHW

    with tile.TileContext(nc) as tc, ExitStack() as ctx:
        pool1 = ctx.enter_context(tc.tile_pool(name="const", bufs=1))
        poolL = ctx.enter_context(tc.tile_pool(name="layerw", bufs=1))
        poolT = ctx.enter_context(tc.tile_pool(name="tilec", bufs=2))
        poolg = ctx.enter_context(tc.tile_pool(name="gath", bufs=4))
        poole = ctx.enter_context(tc.tile_pool(name="edge", bufs=3))
        poolx = ctx.enter_context(tc.tile_pool(name="edge1", bufs=1))
        poolr = ctx.enter_context(tc.tile_pool(name="rad", bufs=2))
        poolu = ctx.enter_context(tc.tile_pool(name="upd", bufs=1))
        psA = ctx.enter_context(tc.tile_pool(name="psA", bufs=3, space="PSUM"))
        psRp = ctx.enter_context(tc.tile_pool(name="psRp", bufs=1, space="PSUM"))
        psAgg = ctx.enter_context(tc.tile_pool(name="psAgg", bufs=1, space="PSUM"))

        def load1(dram, shape, dtype=F32):
            t = pool1.tile(shape, dtype, tag=dram.name)
            nc.sync.dma_start(out=t[:], in_=dram[:])
            return t

        wq_t = load1(t_wq, [3, NB])
        nidx_t = load1(t_nidxb, [P, P], BF16)
        ident_t = load1(t_identb, [P, P], BF16)
        wfeat_t = load1(t_wfeat, [C0, FD], BF16)
        bfeatp_t = load1(t_bfeatp, [P, 4])
        wout1p_t = load1(t_wout1p, [P, 4])

        eps_t = pool1.tile([P, 1], F32, tag="epsT")
        nc.vector.memset(eps_t[:], EPS)

        # feature-major state (bf16)
        sfm = pool1.tile([C0, npc], BF16, tag="sfm")
        nc.sync.dma_start(out=sfm[:], in_=t_s0fm[:])
        vfm_t = pool1.tile([C1, 3, npc], BF16, tag="vfm")
        nc.vector.memset(vfm_t[:], 0.0)
        tfm_t = pool1.tile([C2, 5, npc], BF16, tag="tfm")
        nc.vector.memset(tfm_t[:], 0.0)

        def loadL(dram, l, p, f, tag, dtype=BF16):
            t = poolL.tile([p, f], dtype, tag=tag)
            nc.sync.dma_start(out=t[:], in_=dram[l])
            return t

        rbf_ring = {}

        def radial_ph1(ti):
            """PH1 (layer-independent rbf) for tile slot ti."""
            rbf_b = poolr.tile([NB, cpt * 512], BF16, tag="rbfb")
            for k in range(cpt):
                gsl = poolT.tile([3, 512], F32, tag="geom_c")
                nc.sync.dma_start(
                    out=gsl[:],
                    in_=t_geom[:, (ti * cpt + k) * 512:(ti * cpt + k + 1) * 512])
                ps = psA.tile([NB, 512], F32, tag="mmA", space="PSUM")
                nc.tensor.matmul(ps[:], wq_t[:], gsl[:], start=True, stop=True)
                nc.scalar.activation(out=rbf_b[:, k * 512:(k + 1) * 512],
                                     in_=ps[:], func=AF.Exp)
            rbf_ring[ti % 2] = rbf_b

        def edge_tile(l, t, gsrc, lw, em_s, idx_s, ph1_next):
            """Edge phase for node-tile t of layer l -> returns agg psum.
            Emission is software-pipelined: stage_a(k+1) (gather+proj+evac)
            is emitted before stage_b(k) (DVE messages + agg matmuls) so the
            PE never head-of-line blocks behind the DVE chain."""
            gw, mw, hw = gwid(l), mwid(l), hwid(l)
            full = 0 < l < LYR - 1
            exoff = hw - 4
            rbf_b = rbf_ring[t % 2]
            w_b = poolr.tile([RAD, cpt * 512], BF16, tag="wb")
            for k in range(cpt):
                sl = slice(k * 512, (k + 1) * 512)
                ps = psA.tile([RAD, 512], F32, tag="mmA", space="PSUM")
                nc.tensor.matmul(ps[:RAD, :], lw["wrad1"][:], rbf_b[:, sl],
                                 start=True, stop=True)
                h1 = poolx.tile([RAD, 512], BF16, tag="h1")
                nc.scalar.activation(out=h1[:], in_=ps[:RAD, :], func=AF.Silu,
                                     bias=lw["brad1"][:])
                ps2 = psA.tile([RAD, 512], F32, tag="mmA", space="PSUM")
                nc.tensor.matmul(ps2[:RAD, :], lw["wrad2"][:], h1[:],
                                 start=True, stop=True)
                nc.scalar.activation(out=w_b[:, sl], in_=ps2[:RAD, :], func=AF.Silu,
                                     bias=lw["brad2"][:])
            agg = psAgg.tile([P, HHW], F32, tag="agg", space="PSUM")
            live = {}

            def stage_a(k):
                cs0 = k * 4
                gt = poolg.tile([P, 4, GW], BF16, tag="gt")
                for s in range(4):
                    nc.gpsimd.indirect_dma_start(
                        out=gt[:, s, 0:gw], out_offset=None, in_=gsrc[:, :],
                        in_offset=bass.IndirectOffsetOnAxis(
                            ap=idx_s[:, cs0 + s:cs0 + s + 1], axis=0))
                psp = psRp.tile([P, 4, 512], F32, tag="rp", space="PSUM")
                for s in range(4):
                    nc.tensor.matmul(
                        psp[:, s, 0:mw],
                        w_b[:, k * 512 + s * 128:k * 512 + (s + 1) * 128],
                        lw["wwall"][:, 0:mw], start=True, stop=True,
                        skip_group_check=True)
                rsb = poole.tile([P, 4, MW], BF16, tag="rsb")
                nc.scalar.copy(out=rsb[:, :, 0:mw],
                               in_=reap(psp[:, 0:1, 0:1], [(512, 4), (1, mw)]))
                live[k] = (gt, rsb)

            def stage_b(k):
                cs0 = k * 4
                gt, rsb = live.pop(k)
                hh = poole.tile([P, 4, HHW], BF16, tag="hh")
                # scr = G_s * R_sa ; logits ; ex -> hh ex cols (unmasked; pad
                # edges are killed by the one-hot dst=-1 row instead)
                scrt = poolx.tile([P, 4, C0], BF16, tag="scrt")
                nc.vector.tensor_tensor(
                    out=scrt[:], in0=reap(gt[:, 0:1, 0:1], [(GW, 4), (1, C0)]),
                    in1=reap(rsb[:, 0:1, W_SA:W_SA + 1], [(MW, 4), (1, C0)]),
                    op=OP.mult)
                lgt = poolx.tile([P, 4, H], BF16, tag="lgt")
                with nc.allow_low_precision(reason="logits are O(0.1); bf16 ok"):
                    nc.vector.tensor_reduce(
                        out=lgt[:],
                        in_=scrt[:].rearrange("p s (h c) -> p s h c", h=H),
                        axis=mybir.AxisListType.X, op=OP.add)
                nc.scalar.activation(
                    out=reap(hh[:, 0:1, exoff:exoff + 1], [(HHW, 4), (1, H)]),
                    in_=lgt[:], func=AF.Exp)
                # gxs = G_s * ex ; hh_s = gxs * R_s
                gxs = poolx.tile([P, 4, C0], BF16, tag="gxs")
                nc.vector.tensor_tensor(
                    out=reap(gxs[:, 0:1, 0:1], [(C0, 4), (32, H), (1, 32)]),
                    in0=reap(gt[:, 0:1, 0:1], [(GW, 4), (32, H), (1, 32)]),
                    in1=reap(hh[:, 0:1, exoff:exoff + 1], [(HHW, 4), (1, H), (0, 32)]),
                    op=OP.mult)
                nc.vector.tensor_tensor(
                    out=reap(hh[:, 0:1, 0:1], [(HHW, 4), (1, C0)]),
                    in0=reap(gxs[:, 0:1, 0:1], [(C0, 4), (1, C0)]),
                    in1=reap(rsb[:, 0:1, W_S:W_S + 1], [(MW, 4), (1, C0)]),
                    op=OP.mult)
                if l < LYR - 1:
                    wv_off = W0_V if l == 0 else W_V
                    wt_off = W0_T if l == 0 else W_T
                    rx = poolx.tile([P, 4, 192], BF16, tag="rx")
                    rxv_o, rxt_o = 0, 128 if full else 64
                    nc.vector.tensor_tensor(
                        out=reap(rx[:, 0:1, rxv_o:rxv_o + 1], [(192, 4), (16, H), (1, 16)]),
                        in0=reap(rsb[:, 0:1, wv_off:wv_off + 1], [(MW, 4), (16, H), (1, 16)]),
                        in1=reap(hh[:, 0:1, exoff:exoff + 1], [(HHW, 4), (1, H), (0, 16)]),
                        op=OP.mult)
                    nc.vector.tensor_tensor(
                        out=reap(rx[:, 0:1, rxt_o:rxt_o + 1], [(192, 4), (8, H), (1, 8)]),
                        in0=reap(rsb[:, 0:1, wt_off:wt_off + 1], [(MW, 4), (8, H), (1, 8)]),
                        in1=reap(hh[:, 0:1, exoff:exoff + 1], [(HHW, 4), (1, H), (0, 8)]),
                        op=OP.mult)
                    if full:
                        nc.vector.tensor_tensor(
                            out=reap(rx[:, 0:1, 64:65], [(192, 4), (16, H), (1, 16)]),
                            in0=reap(rsb[:, 0:1, W_VV:W_VV + 1], [(MW, 4), (16, H), (1, 16)]),
                            in1=reap(hh[:, 0:1, exoff:exoff + 1], [(HHW, 4), (1, H), (0, 16)]),
                            op=OP.mult)
                        nc.vector.tensor_tensor(
                            out=reap(rx[:, 0:1, 160:161], [(192, 4), (8, H), (1, 8)]),
                            in0=reap(rsb[:, 0:1, W_TT:W_TT + 1], [(MW, 4), (8, H), (1, 8)]),
                            in1=reap(hh[:, 0:1, exoff:exoff + 1], [(HHW, 4), (1, H), (0, 8)]),
                            op=OP.mult)
                    msv = poolx.tile([P, 4, C1], BF16, tag="msv")
                    nc.vector.tensor_tensor(
                        out=reap(msv[:, 0:1, 0:1], [(C1, 4), (1, C1)]),
                        in0=reap(gt[:, 0:1, G_SV:G_SV + 1], [(GW, 4), (1, C1)]),
                        in1=reap(rx[:, 0:1, rxv_o:rxv_o + 1], [(192, 4), (1, C1)]),
                        op=OP.mult)
                    mst = poolx.tile([P, 4, C2], BF16, tag="mst")
                    nc.vector.tensor_tensor(
                        out=reap(mst[:, 0:1, 0:1], [(C2, 4), (1, C2)]),
                        in0=reap(gt[:, 0:1, G_ST:G_ST + 1], [(GW, 4), (1, C2)]),
                        in1=reap(rx[:, 0:1, rxt_o:rxt_o + 1], [(192, 4), (1, C2)]),
                        op=OP.mult)
                    if full:
                        nc.vector.tensor_tensor(
                            out=reap(hh[:, 0:1, C0:C0 + 1], [(HHW, 4), (C1, 3), (1, C1)]),
                            in0=reap(gt[:, 0:1, G_PV:G_PV + 1], [(GW, 4), (C1, 3), (1, C1)]),
                            in1=reap(rx[:, 0:1, 64:65], [(192, 4), (0, 3), (1, C1)]),
                            op=OP.mult)
                        nc.vector.tensor_tensor(
                            out=reap(hh[:, 0:1, 320:321], [(HHW, 4), (C2, 5), (1, C2)]),
                            in0=reap(gt[:, 0:1, G_PT:G_PT + 1], [(GW, 4), (C2, 5), (1, C2)]),
                            in1=reap(rx[:, 0:1, 160:161], [(192, 4), (0, 5), (1, C2)]),
                            op=OP.mult)
                        scr2 = poolx.tile([P, 4, 352], BF16, tag="scr2")
                        nc.vector.tensor_tensor(
                            out=reap(scr2[:, 0:1, 0:1], [(352, 4), (C1, 3), (1, C1)]),
                            in0=reap(msv[:, 0:1, 0:1], [(C1, 4), (0, 3), (1, C1)]),
                            in1=reap(em_s[:, cs0:cs0 + 1, 2:3], [(EMW, 4), (1, 3), (0, C1)]),
                            op=OP.mult)
                        nc.vector.tensor_tensor(
                            out=reap(scr2[:, 0:1, 192:193], [(352, 4), (C2, 5), (1, C2)]),
                            in0=reap(mst[:, 0:1, 0:1], [(C2, 4), (0, 5), (1, C2)]),
                            in1=reap(em_s[:, cs0:cs0 + 1, 6:7], [(EMW, 4), (1, 5), (0, C2)]),
                            op=OP.mult)
                        nc.vector.tensor_tensor(
                            out=reap(hh[:, 0:1, C0:C0 + 1], [(HHW, 4), (1, 352)]),
                            in0=reap(hh[:, 0:1, C0:C0 + 1], [(HHW, 4), (1, 352)]),
                            in1=reap(scr2[:, 0:1, 0:1], [(352, 4), (1, 352)]),
                            op=OP.add)
                    else:
                        nc.vector.tensor_tensor(
                            out=reap(hh[:, 0:1, C0:C0 + 1], [(HHW, 4), (C1, 3), (1, C1)]),
                            in0=reap(msv[:, 0:1, 0:1], [(C1, 4), (0, 3), (1, C1)]),
                            in1=reap(em_s[:, cs0:cs0 + 1, 2:3], [(EMW, 4), (1, 3), (0, C1)]),
                            op=OP.mult)
                        nc.vector.tensor_tensor(
                            out=reap(hh[:, 0:1, 320:321], [(HHW, 4), (C2, 5), (1, C2)]),
                            in0=reap(mst[:, 0:1, 0:1], [(C2, 4), (0, 5), (1, C2)]),
                            in1=reap(em_s[:, cs0:cs0 + 1, 6:7], [(EMW, 4), (1, 5), (0, C2)]),
                            op=OP.mult)
                ohb = poole.tile([P, 4, P], BF16, tag="ohb")
                nc.vector.tensor_tensor(
                    out=ohb[:],
                    in0=reap(nidx_t[:, 0:1], [(0, 4), (1, P)]),
                    in1=reap(em_s[:, cs0:cs0 + 1, 12:13], [(EMW, 4), (0, P)]),
                    op=OP.is_equal)
                for s in range(4):
                    first = (k == 0 and s == 0)
                    last = (k == cpt - 1 and s == 3)
                    nc.tensor.matmul(agg[:, 0:hw], ohb[:, s, :], hh[:, s, 0:hw],
                                     start=first, stop=last, skip_group_check=True)

            stage_a(0)
            for k in range(cpt):
                if k + 1 < cpt:
                    stage_a(k + 1)
                elif ph1_next is not None:
                    ph1_next()
                stage_b(k)
            return agg

        def transpose_to(src_ap, kparts, ffree):
            """transpose bf16 src [kparts, ffree] sbuf -> psum [ffree, kparts]"""
            ps = psA.tile([P, 512], F32, tag="mmA", space="PSUM")
            psb = ps[:, 0:P].bitcast(BF16)
            nc.tensor.transpose(psb[:ffree, :kparts], src_ap,
                                ident_t[:kparts, :kparts])
            return psb

        def update_tile(l, t, agg, lw):
            tsl = slice(t * P, (t + 1) * P)
            hw = hwid(l)
            last_v = l < LYR - 1
            rden = poolu.tile([P, H], F32, tag="rden")
            nc.vector.tensor_scalar(out=rden[:], in0=agg[:, hw - 4:hw],
                                    scalar1=1e-9, scalar2=None, op0=OP.add)
            nc.vector.reciprocal(out=rden[:], in_=rden[:])
            aggnm = poolu.tile([P, 480], BF16, tag="aggnm")
            nc.vector.tensor_tensor(
                out=reap(aggnm[:, 0:1], [(32, H), (1, 32)]),
                in0=reap(agg[:, 0:1], [(32, H), (1, 32)]),
                in1=reap(rden[:, 0:1], [(1, H), (0, 32)]), op=OP.mult)
            if last_v:
                nc.vector.tensor_tensor(
                    out=reap(aggnm[:, 128:129], [(64, 3), (16, H), (1, 16)]),
                    in0=reap(agg[:, 128:129], [(64, 3), (16, H), (1, 16)]),
                    in1=reap(rden[:, 0:1], [(0, 3), (1, H), (0, 16)]), op=OP.mult)
                nc.vector.tensor_tensor(
                    out=reap(aggnm[:, 320:321], [(32, 5), (8, H), (1, 8)]),
                    in0=reap(agg[:, 320:321], [(32, 5), (8, H), (1, 8)]),
                    in1=reap(rden[:, 0:1], [(0, 5), (1, H), (0, 8)]), op=OP.mult)

            # s out-projection + residual (feature-major)
            psS = transpose_to(aggnm[:, 0:128], P, P)
            afs = poolu.tile([P, P], BF16, tag="afs")
            nc.scalar.copy(out=afs[:], in_=psS[:, :P])
            pso = psA.tile([P, 512], F32, tag="mmA", space="PSUM")
            nc.tensor.matmul(pso[:, 0:P], lw["wo_s"][:], afs[:], start=True, stop=True)
            upd_s = poolu.tile([P, P], BF16, tag="upd_s")
            nc.vector.tensor_tensor(out=upd_s[:], in0=sfm[:, tsl], in1=pso[:, 0:P],
                                    op=OP.add)

            upd_v = poolu.tile([C1, 3, P], BF16, tag="upd_v")
            upd_t = poolu.tile([C2, 5, P], BF16, tag="upd_t")
            if last_v:
                for i in range(3):
                    psV = transpose_to(aggnm[:, 128 + 64 * i:128 + 64 * i + 64], P, C1)
                    afv = poolu.tile([C1, P], BF16, tag="afv")
                    nc.scalar.copy(out=afv[:], in_=psV[:C1, :P])
                    psv2 = psA.tile([P, 512], F32, tag="mmA", space="PSUM")
                    nc.tensor.matmul(psv2[:C1, 0:P], lw["wo_v"][:], afv[:],
                                     start=True, stop=True)
                    nc.vector.tensor_tensor(out=upd_v[:, i, :], in0=vfm_t[:, i, tsl],
                                            in1=psv2[:C1, 0:P], op=OP.add)
                for m in range(5):
                    psT_ = transpose_to(aggnm[:, 320 + 32 * m:320 + 32 * m + 32], P, C2)
                    aft = poolu.tile([C2, P], BF16, tag="aft")
                    nc.scalar.copy(out=aft[:], in_=psT_[:C2, :P])
                    pst2 = psA.tile([P, 512], F32, tag="mmA", space="PSUM")
                    nc.tensor.matmul(pst2[:C2, 0:P], lw["wo_t"][:], aft[:],
                                     start=True, stop=True)
                    nc.vector.tensor_tensor(out=upd_t[:, m, :], in0=tfm_t[:, m, tsl],
                                            in1=pst2[:C2, 0:P], op=OP.add)

            # transpose to node-major
            snm = poolu.tile([P, C0], BF16, tag="snm")
            psn = transpose_to(upd_s[:], P, P)
            nc.scalar.copy(out=snm[:], in_=psn[:, :P])
            vnm = poolu.tile([P, C1, 3], BF16, tag="vnm")
            tnm = poolu.tile([P, C2, 5], BF16, tag="tnm")
            if last_v:
                for i in range(3):
                    psn = transpose_to(upd_v[:, i, :], C1, P)
                    nc.vector.tensor_copy(
                        out=reap(vnm[:, 0:1, i:i + 1], [(3, C1)]), in_=psn[:, :C1])
                for m in range(5):
                    psn = transpose_to(upd_t[:, m, :], C2, P)
                    nc.vector.tensor_copy(
                        out=reap(tnm[:, 0:1, m:m + 1], [(5, C2)]), in_=psn[:, :C2])

            # batched inverse-norms: nrm = [var_s, mean|v|^2, mean|t|^2] + eps
            stats = poolu.tile([P, 6], F32, tag="stats")
            nc.vector.bn_stats(out=stats[:], in_=snm[:])
            mv = poolu.tile([P, 2], F32, tag="mv")
            nc.vector.bn_aggr(out=mv[:], in_=stats[:])
            nrm = poolu.tile([P, 3], F32, tag="nrm")
            nc.vector.tensor_scalar(out=nrm[:, 0:1], in0=mv[:, 1:2],
                                    scalar1=EPS, scalar2=None, op0=OP.add)
            if last_v:
                vsq = poolu.tile([P, C1, 3], F32, tag="vsq")
                nc.vector.tensor_tensor(out=vsq[:], in0=vnm[:], in1=vnm[:], op=OP.mult)
                vr1 = poolu.tile([P, 1], F32, tag="vr1")
                nc.vector.tensor_reduce(out=vr1[:], in_=vsq[:],
                                        axis=mybir.AxisListType.XY, op=OP.add)
                tsq = poolu.tile([P, C2, 5], F32, tag="tsq")
                nc.vector.tensor_tensor(out=tsq[:], in0=tnm[:], in1=tnm[:], op=OP.mult)
                tr1 = poolu.tile([P, 1], F32, tag="tr1")
                nc.vector.tensor_reduce(out=tr1[:], in_=tsq[:],
                                        axis=mybir.AxisListType.XY, op=OP.add)
                nc.vector.tensor_scalar(out=nrm[:, 1:2], in0=vr1[:, 0:1],
                                        scalar1=1.0 / C1, scalar2=EPS,
                                        op0=OP.mult, op1=OP.add)
                nc.vector.tensor_scalar(out=nrm[:, 2:3], in0=tr1[:, 0:1],
                                        scalar1=1.0 / C2, scalar2=EPS,
                                        op0=OP.mult, op1=OP.add)
            nw = 3 if last_v else 1
            nc.vector.reciprocal(out=nrm[:, 0:nw], in_=nrm[:, 0:nw])
            nc.scalar.activation(out=nrm[:, 0:nw], in_=nrm[:, 0:nw], func=AF.Sqrt)
            # apply LN / norms
            nc.vector.tensor_scalar(out=snm[:], in0=snm[:], scalar1=mv[:, 0:1],
                                    scalar2=nrm[:, 0:1], op0=OP.subtract, op1=OP.mult)
            nc.vector.tensor_tensor(out=snm[:], in0=snm[:], in1=lw["lngs"][:], op=OP.mult)
            nc.vector.tensor_tensor(out=snm[:], in0=snm[:], in1=lw["lnbs"][:], op=OP.add)
            if last_v:
                nc.vector.tensor_scalar(out=vnm[:], in0=vnm[:], scalar1=nrm[:, 1:2],
                                        scalar2=None, op0=OP.mult)
                nc.vector.tensor_tensor(
                    out=vnm[:], in0=vnm[:],
                    in1=reap(lw["lngv"][:, 0:1], [(1, C1), (0, 3)]), op=OP.mult)
                nc.vector.tensor_scalar(out=tnm[:], in0=tnm[:], scalar1=nrm[:, 2:3],
                                        scalar2=None, op0=OP.mult)
                nc.vector.tensor_tensor(
                    out=tnm[:], in0=tnm[:],
                    in1=reap(lw["lngt"][:, 0:1], [(1, C2), (0, 5)]), op=OP.mult)

            # write back feature-major state
            psn = transpose_to(snm[:], P, P)
            nc.scalar.copy(out=sfm[:, tsl], in_=psn[:, :P])
            if last_v:
                for i in range(3):
                    psn = transpose_to(reap(vnm[:, 0:1, i:i + 1], [(3, C1)]), P, C1)
                    nc.scalar.copy(out=vfm_t[:, i, tsl], in_=psn[:C1, :P])
                for m in range(5):
                    psn = transpose_to(reap(tnm[:, 0:1, m:m + 1], [(5, C2)]), P, C2)
                    nc.scalar.copy(out=tfm_t[:, m, tsl], in_=psn[:C2, :P])

            if last_v:
                # next-layer table rows, node-major (stationary = fm state)
                gwn = gwid(l + 1)
                ntrow = poolu.tile([P, GW], BF16, tag="ntrow")
                if l < LYR - 2:
                    ntA = psRp.tile([P, 512], F32, tag="rp", space="PSUM")
                    nc.tensor.matmul(ntA[:, 0:GW0], sfm[:, tsl], lw["wspack"][:],
                                     start=True, stop=True)
                    nc.scalar.copy(out=ntrow[:, 0:GW0], in_=ntA[:, 0:GW0])
                    ntB = psRp.tile([P, 512], F32, tag="rp", space="PSUM")
                    for i in range(3):
                        nc.tensor.matmul(ntB[:, 64 * i:64 * i + 64],
                                         vfm_t[:, i, tsl], lw["wv_v"][:],
                                         start=True, stop=True, skip_group_check=True)
                    for m in range(5):
                        nc.tensor.matmul(ntB[:, 192 + 32 * m:192 + 32 * m + 32],
                                         tfm_t[:, m, tsl], lw["wt_t"][:],
                                         start=True, stop=True, skip_group_check=True)
                    nc.scalar.copy(out=ntrow[:, GW0:GW], in_=ntB[:, 0:352])
                    nc.sync.dma_start(out=own[l][tsl, :], in_=ntrow[:, 0:GW])
                else:
                    # l == 4: only P_s for layer 5
                    ntA = psRp.tile([P, 512], F32, tag="rp", space="PSUM")
                    nc.tensor.matmul(ntA[:, 0:C0], sfm[:, tsl],
                                     lw["wspack"][:, 0:C0], start=True, stop=True)
                    nc.scalar.copy(out=ntrow[:, 0:C0], in_=ntA[:, 0:C0])
                    nc.sync.dma_start(out=own[l][tsl, :], in_=ntrow[:, 0:GW5])
            else:
                # final readout head
                feat = poolu.tile([P, 4, P], F32, tag="feat")
                for b in range(4):
                    ps = psA.tile([P, 512], F32, tag="mmA", space="PSUM")
                    nc.tensor.matmul(ps[:, 0:P], wfeat_t[:, b * 128:(b + 1) * 128],
                                     sfm[:, tsl], start=True, stop=True)
                    nc.scalar.activation(out=feat[:, b, :], in_=ps[:, 0:P],
                                         func=AF.Gelu_apprx_tanh,
                                         bias=bfeatp_t[:, b:b + 1])
                pseT = psRp.tile([P, 512], F32, tag="rp", space="PSUM")
                pse = pseT[0:1, 0:P]
                for b in range(4):
                    nc.tensor.matmul(pse, wout1p_t[:, b:b + 1], feat[:, b, :],
                                     start=(b == 0), stop=(b == 3))
                ne = poolu.tile([1, P], F32, tag="ne")
                nc.vector.tensor_copy(out=ne[:], in_=pse)
                nc.sync.dma_start(out=t_nodee[tsl], in_=ne[0:1, :])

        for l in range(LYR):
            gsrc = t_ntab0 if l == 0 else ntab[l - 1]
            lw = dict(
                wrad1=loadL(t_wrad1, l, NB, RAD, "wrad1"),
                wrad2=loadL(t_wrad2, l, RAD, RAD, "wrad2"),
                brad1=loadL(t_brad1, l, RAD, 1, "brad1", F32),
                brad2=loadL(t_brad2, l, RAD, 1, "brad2", F32),
                wwall=loadL(t_wwall, l, RAD, MW, "wwall"),
                wo_s=loadL(t_wo_s, l, C0, C0, "wo_s"),
                lngs=loadL(t_lngs, l, P, C0, "lngs"),
                lnbs=loadL(t_lnbs, l, P, C0, "lnbs"),
            )
            if l < LYR - 1:
                lw["wo_v"] = loadL(t_wo_v, l, C1, C1, "wo_v")
                lw["wo_t"] = loadL(t_wo_t, l, C2, C2, "wo_t")
                lw["lngv"] = loadL(t_lngv, l, P, C1, "lngv")
                lw["lngt"] = loadL(t_lngt, l, P, C2, "lngt")
                lw["wspack"] = loadL(t_wspack, l + 1, C0, GW0, "wspack")
                lw["wv_v"] = loadL(t_wv_v, l + 1, C1, C1, "wv_v")
                lw["wt_t"] = loadL(t_wt_t, l + 1, C2, C2, "wt_t")
            for t in range(ntile):
                if l == 0 and t == 0:
                    radial_ph1(0)
                em_s = poolT.tile([P, cpt * 4, EMW], BF16, tag="em_s")
                nc.sync.dma_start(out=em_s[:],
                                  in_=t_em[:, t * cpt * 4:(t + 1) * cpt * 4, :])
                idx_s = poolT.tile([P, cpt * 4], I32, tag="idx_s")
                nc.sync.dma_start(out=idx_s[:],
                                  in_=t_idxw[:, t * cpt * 4:(t + 1) * cpt * 4])
                nt_t = t + 1 if t + 1 < ntile else 0
                more = (t + 1 < ntile) or (l + 1 < LYR)
                ph1_next = (lambda tt=nt_t: radial_ph1(tt)) if more else None
                agg = edge_tile(l, t, gsrc, lw, em_s, idx_s, ph1_next)
                update_tile(l, t, agg, lw)
            if l < LYR - 1:
                nc.gpsimd.collective_compute(
                    "AllGather", OP.bypass,
                    replica_groups=[list(range(cfg.ncore))],
                    ins=[own[l][:]], outs=[ntab[l][:]])

    nc.compile()
    return nc


# ---------------- entry point ----------------

def _ensure_profile_hook():
    try:
        import antenv  # noqa
        import antenv.axon_hooks  # noqa
        return
    except Exception:
        pass
    try:
        import antenv
        from trn_agent_boot.trn_boot import _ntff_profile_via_ctypes
        hook = _ntff_profile_via_ctypes("/opt/axon/libaxon_pjrt.so")
        mod = types.ModuleType("antenv.axon_hooks")
        mod.get_axon_ntff_profile_hook = lambda: hook
        mod.set_axon_ntff_profile_hook = lambda h: None
        sys.modules["antenv.axon_hooks"] = mod
        antenv.axon_hooks = mod
    except Exception:
        pass


_PROGRAM_CACHE = {}


def run_cfg(inp, cfg: Cfg, trace=False):
    in_maps, gid = host_preprocess(inp, cfg)
    key = (cfg.ncore, cfg.npc, cfg.cpt, cfg.layers)
    if key not in _PROGRAM_CACHE:
        _PROGRAM_CACHE[key] = build_program(cfg)
    nc = _PROGRAM_CACHE[key]
    if trace:
        _ensure_profile_hook()
    res = run_bass_kernel_spmd(nc, in_maps, list(range(cfg.ncore)), trace=trace)
    node_e_pad = np.concatenate(
        [np.asarray(res.results[c]["node_e"]) for c in range(cfg.ncore)])
    node_e = node_e_pad[gid]          # invert node permutation
    return node_e, res


def kernel(**inputs):
    cfg = Cfg()
    node_e, _ = run_cfg(inputs, cfg)
    node_e = node_e[:, None] + np.asarray(inputs["b_out1"], np.float32)[None, :]
    batch = np.asarray(inputs["batch"]).astype(np.int64)
    graph = np.zeros((G, 1), np.float32)
    np.add.at(graph, batch, node_e)
    out = graph @ np.asarray(inputs["W_read"], np.float32) + np.asarray(
        inputs["b_read"], np.float32)
    return out.astype(np.float32)
